# revision 41
# baseline (speedup 1.0000x reference)
"""Trainium2 Bass kernel for nn_DSSConf (DSS conformer GNN message passing).

Self-contained: hardcodes shapes/sharding for the real problem; exposes
kernel(**inputs) -> np.ndarray.
"""
import sys
import math
from dataclasses import dataclass

sys.path.insert(0, "/opt/trn_rl_repo")

import numpy as np
from concourse import bass, bacc, tile, mybir, bass_utils

F32 = mybir.dt.float32
I16 = mybir.dt.int16
ALU = mybir.AluOpType
ACTF = mybir.ActivationFunctionType
AX = mybir.AxisListType


@dataclass(frozen=True)
class Dims:
    N: int = 100000        # conformer nodes
    H: int = 256           # hidden
    NF: int = 128          # num filters
    NG: int = 50           # num gaussians
    G: int = 10000         # graph nodes
    E: int = 1000000       # conformer edges
    EG: int = 30000        # graph edges
    VOCAB: int = 5
    CUTOFF: float = 10.0
    cores: int = 8
    qsize: int = 25000     # src quadrant size for int16 gather indices
    B_E: int = 384         # fixed edges per (dst-window, src-quad) bucket
    gwin: int = 128        # GIN scatter window (<=128 segments)
    PW: int = 640          # padded GIN edges per (core, window) (multiple of 128)
    nchunk: int = 500      # node chunk for the h/out stage (divides NS, mult of 10)

    @property
    def NS(self):
        return self.N // self.cores

    @property
    def GS(self):
        return self.G // self.cores

    @property
    def CW(self):
        """dst windows of 128 nodes per core shard."""
        return (self.NS + 127) // 128

    @property
    def EH(self):
        """padded edges per src-half stream (2 quad buckets per window)."""
        return self.CW * 2 * self.B_E

    @property
    def E_pad(self):
        return 2 * self.EH

    @property
    def NWIN(self):
        return (self.GS + self.gwin - 1) // self.gwin

    @property
    def EG_pad(self):
        return self.NWIN * self.PW


REAL = Dims()


def _wrap16(arr, dtype=np.int16):
    """Edge i -> [i % 16, i // 16], replicated to 128 partitions."""
    a = np.asarray(arr).reshape(-1, 16).T.astype(dtype)  # [16, n/16]
    return np.tile(a, (8, 1)).copy()  # [128, n/16]


def _tile128(arr, dtype=np.float32):
    """Edge i -> [i % 128, i // 128] (per-partition scalar layout)."""
    return np.ascontiguousarray(np.asarray(arr).reshape(-1, 128).T.astype(dtype))


def host_prep(inputs, D: Dims):
    """Build per-core in_maps (list of dicts) for the SPMD kernel."""
    x = np.asarray(inputs["x"], np.float32)
    cnb = np.asarray(inputs["conf_node_batch"]).astype(np.int64)
    ei = np.asarray(inputs["edge_index_conf"]).astype(np.int64)
    ew = np.asarray(inputs["edge_weight_conf"], np.float32)
    ea = np.asarray(inputs["edge_attr_conf"], np.float32)
    eig = np.asarray(inputs["edge_index_graph"]).astype(np.int64)
    eag = np.asarray(inputs["edge_attr_graph"]).astype(np.int64)

    rep = D.N // D.G
    assert np.array_equal(cnb, np.repeat(np.arange(D.G), rep)), \
        "conf_node_batch structure mismatch"

    NS, GS = D.NS, D.GS
    src, dst = ei[0], ei[1]

    # ---- conformer edges: window-major one-hot layout ----
    # per core: two streams by src half (matching the split AllGather's
    # permuted xf layout); within a stream, buckets of fixed B_E edges per
    # (dst-window of 128, src quad within the half); in-quad src offset =
    # (src_core % 4)*NS/2 + (src % NS/2)
    B_E, CW, EH = D.B_E, D.CW, D.EH
    core = dst // NS
    sc = src // NS
    so = src % NS
    sh = so // (NS // 2)
    sj = (sc >= 4).astype(np.int64)
    inq = (sc % 4) * (NS // 2) + (so % (NS // 2))
    win = (dst % NS) // 128
    bucket = ((core * 2 + sh) * CW + win) * 2 + sj
    order = np.argsort(bucket, kind="stable")
    sb = bucket[order]
    nbuck = D.cores * 2 * CW * 2
    bounds = np.searchsorted(sb, np.arange(nbuck + 1))
    counts = bounds[1:] - bounds[:-1]
    assert counts.max() <= B_E, f"bucket overflow: {counts.max()} > {B_E}"
    rank = np.arange(len(sb)) - bounds[sb]
    c_b = sb // (2 * CW * 2)
    rem = sb % (2 * CW * 2)
    h_b = rem // (CW * 2)
    w_b = (rem % (CW * 2)) // 2
    j_b = rem % 2
    flat = (c_b * D.E_pad + h_b * EH + (w_b * 2 + j_b) * B_E + rank)

    src_pad = np.zeros(D.cores * D.E_pad, np.int64)
    drel_pad = np.full(D.cores * D.E_pad, -1.0, np.float32)
    w_pad = np.full(D.cores * D.E_pad, D.CUTOFF, np.float32)  # C(CUTOFF)=0
    a_pad = np.zeros((D.cores * D.E_pad, D.NG), np.float32)
    src_pad[flat] = inq[order]
    drel_pad[flat] = ((dst[order] % NS) - w_b * 128).astype(np.float32)
    w_pad[flat] = ew[order]
    a_pad[flat] = ea[order]

    AT = np.zeros((D.cores, D.NG, D.E_pad), np.float32)
    WT = np.zeros((D.cores, 128, D.E_pad // 128), np.float32)
    DRELC = np.zeros((D.cores, 128, D.E_pad // 128), np.float32)
    SRC = np.zeros((D.cores, 128, D.E_pad // 16), np.int16)
    for c in range(D.cores):
        sl = slice(c * D.E_pad, (c + 1) * D.E_pad)
        AT[c] = a_pad[sl].T
        WT[c] = _tile128(w_pad[sl])
        DRELC[c] = _tile128(drel_pad[sl])
        SRC[c] = _wrap16(src_pad[sl])

    # ---- graph edges: order by (core(dst), window(dst), dst) ----
    sg, dg = eig[0], eig[1]
    gcore = dg // GS
    gwin = (dg - gcore * GS) // D.gwin
    gorder = np.lexsort((dg, gwin, gcore))
    g_s, g_d, g_w, g_c = sg[gorder], dg[gorder], gwin[gorder], gcore[gorder]
    g_a = eag[gorder]

    SG = np.zeros((D.cores, 128, D.EG_pad // 16), np.int16)
    DREL = np.zeros((D.cores, 128, D.EG_pad // 128), np.float32)
    BHOT = np.zeros((D.cores, 3 * D.VOCAB, D.EG_pad), np.float32)

    gkeys = g_c * D.NWIN + g_w
    gbounds = np.searchsorted(gkeys, np.arange(D.cores * D.NWIN + 1))
    for c in range(D.cores):
        sg_pad = np.zeros(D.EG_pad, np.int64)
        dr_pad = np.full(D.EG_pad, -1.0, np.float32)  # -1 kills pads in one-hot
        bh_pad = np.zeros((3 * D.VOCAB, D.EG_pad), np.float32)
        for w in range(D.NWIN):
            lo, hi = gbounds[c * D.NWIN + w], gbounds[c * D.NWIN + w + 1]
            cnt = hi - lo
            assert cnt <= D.PW, f"PW overflow: core {c} win {w}: {cnt}"
            o = w * D.PW
            sg_pad[o:o + cnt] = g_s[lo:hi]
            dr_pad[o:o + cnt] = (g_d[lo:hi] - c * GS - w * D.gwin).astype(np.float32)
            for k in range(3):
                bh_pad[k * D.VOCAB + g_a[lo:hi, k], np.arange(o, o + cnt)] = 1.0
        SG[c] = _wrap16(sg_pad)
        DREL[c] = _tile128(dr_pad)
        BHOT[c] = bh_pad

    # ---- x^T shards ----
    xT = np.ascontiguousarray(x.T)  # [H, N]
    XT = xT.reshape(2, 128, D.N)

    # ---- weights (replicated) ----
    H2 = D.H // 128
    w = {k: np.asarray(inputs[k], np.float32) for k in (
        "mlp_w1", "mlp_b1", "mlp_w2", "mlp_b2", "cf_lin1", "cf_lin2",
        "cf_lin2_b", "lin_w", "lin_b", "bond_emb", "gin_eps", "gin_w1",
        "gin_w2", "bn1_g", "bn1_b", "bn2_g", "bn2_b")}
    const = dict(
        w1=w["mlp_w1"],                                   # [NG, NF]
        b1col=w["mlp_b1"].reshape(D.NF, 1),
        w2=w["mlp_w2"],                                   # [NF, NF]
        b2row=w["mlp_b2"].reshape(1, D.NF),
        b2row3=np.tile(w["mlp_b2"].reshape(1, D.NF), (1, 3)),
        ones1=np.ones((1, 128), np.float32),
        lin1=np.ascontiguousarray(w["cf_lin1"].reshape(H2, 128, D.NF)),
        lin2=w["cf_lin2"],                                # [NF, H]
        lin2b=w["cf_lin2_b"].reshape(H2, 128, 1),
        linw=np.ascontiguousarray(w["lin_w"].reshape(H2, 128, D.H)),
        linb=w["lin_b"].reshape(H2, 128, 1),
        gw1=np.ascontiguousarray(w["gin_w1"].reshape(H2, 128, D.H)),
        gw2=np.ascontiguousarray(w["gin_w2"].reshape(H2, 128, D.H)),
        bondcat=np.ascontiguousarray(
            w["bond_emb"].reshape(3 * D.VOCAB, D.H)),
        bn1g=w["bn1_g"].reshape(H2, 128, 1), bn1b=w["bn1_b"].reshape(H2, 128, 1),
        bn2g=w["bn2_g"].reshape(H2, 128, 1), bn2b=w["bn2_b"].reshape(H2, 128, 1),
        epsv=np.full((128, 1), 1.0 + float(w["gin_eps"]), np.float32),
        zerocol=np.zeros((128, 1), np.float32),
        eps5col=np.full((128, 1), 1e-5, np.float32),
        pihalf=np.full((128, 1), -math.pi / 2, np.float32),
        iota=np.tile(np.arange(128, dtype=np.float32), (128, 1)).copy(),
        ident=np.eye(128, dtype=np.float32),
    )

    in_maps = []
    for c in range(D.cores):
        m = dict(
            xT=np.ascontiguousarray(XT[:, :, c * NS:(c + 1) * NS]),
            AT=AT[c], WT=WT[c], SRC=SRC[c], DRELC=DRELC[c],
            SG=SG[c], DREL=DREL[c], BHOT=BHOT[c],
        )
        m.update(const)
        in_maps.append(m)
    return in_maps


def assemble(results, D: Dims):
    """Per-core outT [2,128,NS] -> full [N, H]."""
    parts = [r["outT"].reshape(D.H, D.NS) for r in results]
    outT = np.concatenate(parts, axis=1)  # [H, N]
    return np.ascontiguousarray(outT.T)


def _ts(i, n):
    return bass.ts(i, n)


def build_nc(D: Dims, flags: frozenset = frozenset()):
    nc = bacc.Bacc("TRN2", target_bir_lowering=False, debug=False,
                   num_devices=D.cores, num_swdge_queues=3)
    NS, GS, H, NF, NG = D.NS, D.GS, D.H, D.NF, D.NG
    H2 = H // 128

    I = {}
    def di(name, shape, dt=F32):
        I[name] = nc.dram_tensor(name, list(shape), dt, kind="ExternalInput")
        return I[name]

    di("xT", [2, 128, NS])
    di("AT", [NG, D.E_pad])
    di("WT", [128, D.E_pad // 128])
    di("SRC", [128, D.E_pad // 16], I16)
    di("DRELC", [128, D.E_pad // 128])
    di("SG", [128, D.EG_pad // 16], I16)
    di("DREL", [128, D.EG_pad // 128])
    di("BHOT", [3 * D.VOCAB, D.EG_pad])
    di("w1", [NG, NF]); di("b1col", [NF, 1]); di("w2", [NF, NF])
    di("b2row", [1, NF]); di("b2row3", [1, 3 * NF]); di("ones1", [1, 128])
    di("lin1", [H2, 128, NF]); di("lin2", [NF, H]); di("lin2b", [H2, 128, 1])
    di("linw", [H2, 128, H]); di("linb", [H2, 128, 1])
    di("gw1", [H2, 128, H]); di("gw2", [H2, 128, H])
    di("bondcat", [3 * D.VOCAB, H])
    di("bn1g", [H2, 128, 1]); di("bn1b", [H2, 128, 1])
    di("bn2g", [H2, 128, 1]); di("bn2b", [H2, 128, 1])
    di("epsv", [128, 1]); di("iota", [128, 128]); di("ident", [128, 128])
    di("zerocol", [128, 1]); di("eps5col", [128, 1]); di("pihalf", [128, 1])

    outT = nc.dram_tensor("outT", [2, 128, NS], F32, kind="ExternalOutput")

    groups = [list(range(D.cores))]

    with tile.TileContext(nc) as tc:
        with (
            tc.tile_pool(name="const", bufs=1) as cp,
            tc.tile_pool(name="work", bufs=2) as wp,
            tc.tile_pool(name="small", bufs=3) as sp,
            tc.tile_pool(name="gin", bufs=2) as ctx_gin_pool,
            tc.tile_pool(name="psum", bufs=2, space="PSUM") as pp,
            tc.tile_pool(name="dram", bufs=1, space="DRAM") as dp,
        ):
            # ---------- load constants ----------
            C = {}
            for nm, shp in [("w1", [NG, NF]), ("b1col", [NF, 1]),
                            ("w2", [NF, NF]), ("b2row", [1, NF]),
                            ("b2row3", [1, 3 * NF]),
                            ("ones1", [1, 128]), ("lin2", [NF, H]),
                            ("bondcat", [3 * D.VOCAB, H]),
                            ("epsv", [128, 1]), ("iota", [128, 128]),
                            ("ident", [128, 128]), ("zerocol", [128, 1]),
                            ("eps5col", [128, 1]), ("pihalf", [128, 1])]:
                t = cp.tile(shp, F32, name=f"c_{nm}")
                nc.sync.dma_start(t[:], I[nm].ap())
                C[nm] = t
            nc.const_aps.aps[(F32, 0.0)] = C["zerocol"][:]
            # [H2,128,*] constants: load as per-half tiles
            for nm in ("lin1", "lin2b", "linw", "linb", "gw1", "gw2",
                       "bn1g", "bn1b", "bn2g", "bn2b"):
                C[nm] = []
                inner = I[nm].shape[2]
                for k in range(H2):
                    t = cp.tile([128, inner], F32, name=f"c_{nm}{k}")
                    nc.sync.dma_start(t[:], I[nm].ap()[k])
                    C[nm].append(t)

            # ---------- DRAM scratch ----------
            # xf shard/full split in half so the AllGather pipelines with
            # compute: AG#h gathers every core's half-h shard; the gathered
            # layout is permuted (half-major), host_prep permutes src indices
            HSH = NS // 2
            xf_shard = [dp.tile([HSH, NF], F32, name=f"xf_shard{h}")
                        for h in range(2)]
            xf_full = [dp.tile([D.cores * HSH, NF], F32, name=f"xf_full{h}",
                               addr_space="Shared") for h in range(2)]
            xagg_shard = dp.tile([GS, H], F32, name="xagg_shard")
            xagg_full = dp.tile([D.cores * GS, H], F32, name="xagg_full",
                                addr_space="Shared")
            agg_ab = [dp.tile([NS, NF], F32, name=f"agg_{h}")
                      for h in range(2)]
            st1_in = dp.tile([128, 4], F32, name="st1_in")
            st1_out = dp.tile([128, 4], F32, name="st1_out", addr_space="Shared")
            st2_in = dp.tile([128, 4], F32, name="st2_in")
            st2_out = dp.tile([128, 4], F32, name="st2_out", addr_space="Shared")

            # =========== Phase A: segment-max pool first, then xf ===========
            # pass 1 pools so the (small) xagg AllGather is issued as early
            # as possible; pass 2 re-reads x and computes xf, issuing each
            # half's AllGather as soon as that half of the shard is written
            rep = D.N // D.G
            PCH = 250
            n_pch = NS // PCH
            half_chunks = n_pch // 2
            xaggT = [cp.tile([128, GS], F32, name=f"xaggT{k}") for k in range(2)]
            for j in range(n_pch):
                xt = [wp.tile([128, PCH], F32, tag=f"ph_a_xt{k}", name=f"ph_a_xt{k}")
                      for k in range(2)]
                for k in range(2):
                    nc.sync.dma_start(xt[k][:], I["xT"].ap()[k, :, _ts(j, PCH)])
                # pool: max over groups of 10 cols
                for k in range(2):
                    nc.vector.tensor_reduce(
                        xaggT[k][:, _ts(j, PCH // rep)],
                        xt[k][:].rearrange("p (g t) -> p g t", t=rep),
                        AX.X, ALU.max)

            # transpose x_aggT -> node-major x_agg shard
            GT = (GS + 127) // 128
            for t in range(GT):
                m = min(128, GS - t * 128)
                for k in range(2):
                    pst = pp.tile([128, 128], F32, tag="ps_tr", name="ps_tr")
                    nc.tensor.transpose(pst[:m, :], xaggT[k][:, t * 128:t * 128 + m],
                                        C["ident"][:])
                    sb = sp.tile([128, 128], F32, tag="ph_a_trsb", name="ph_a_trsb")
                    nc.scalar.copy(sb[:m, :], pst[:m, :])
                    nc.sync.dma_start(
                        xagg_shard[t * 128:t * 128 + m, _ts(k, 128)], sb[:m, :])

            if "no_coll" not in flags and "no_coll_xagg" not in flags:
                nc.gpsimd.collective_compute(
                    "AllGather", ALU.bypass, replica_groups=groups,
                    ins=[xagg_shard.opt()], outs=[xagg_full.opt()])

            # pass 2: xf = x @ cf_lin1, node-major tiles of <=128
            for j in range(n_pch):
                xt = [wp.tile([128, PCH], F32, tag=f"ph_a_xt{k}", name=f"ph_a2_xt{k}")
                      for k in range(2)]
                for k in range(2):
                    nc.sync.dma_start(xt[k][:], I["xT"].ap()[k, :, _ts(j, PCH)])
                h = j // half_chunks
                r0 = (j - h * half_chunks) * PCH
                nt = (PCH + 127) // 128
                for t in range(nt):
                    m = min(128, PCH - t * 128)
                    ps = pp.tile([128, NF], F32, tag="ps_mm", name="ps_mm")
                    for k in range(2):
                        nc.tensor.matmul(ps[:m, :], xt[k][:, t * 128:t * 128 + m],
                                         C["lin1"][k][:], start=(k == 0),
                                         stop=(k == 1))
                    sb = sp.tile([128, NF], F32, tag="ph_a_sb", name="ph_a_sb")
                    nc.scalar.copy(sb[:m, :], ps[:m, :])
                    nc.sync.dma_start(
                        xf_shard[h][r0 + t * 128: r0 + t * 128 + m, :],
                        sb[:m, :])
                if (j == half_chunks - 1 and "no_coll" not in flags
                        and "no_coll_xf" not in flags):
                    nc.gpsimd.collective_compute(
                        "AllGather", ALU.bypass, replica_groups=groups,
                        ins=[xf_shard[0].opt()], outs=[xf_full[0].opt()])
            # AG#1 is issued later (after st1's AllReduce) so the GIN stats
            # reduction isn't queued behind it on the collective engine

            # =========== Phase B: GIN branch (sharded by graph node) =========
            # gather x_agg[sg], edge_emb via bond one-hot matmul, relu,
            # one-hot scatter into agg_g windows
            sgidx = cp.tile([128, D.EG_pad // 16], I16, name="sgidx_sb")
            nc.sync.dma_start(sgidx[:], I["SG"].ap())
            drel = cp.tile([128, D.EG_pad // 128], F32, name="drel_sb")
            nc.sync.dma_start(drel[:], I["DREL"].ap())

            # t-buffer (node-major (1+eps)x_agg + agg_g), then transposed halves
            gp = ctx_gin_pool
            tT = [gp.tile([128, GS], F32, tag=f"ginbuf{k}", name=f"tT{k}")
                  for k in range(2)]

            assert D.EG_pad % 128 == 0
            tiles_per_win = D.PW // 128
            for w in range(0 if "no_b" in flags else D.NWIN):
                m = min(D.gwin, GS - w * D.gwin)
                # gather this window's source rows
                gath_g = wp.tile([128, tiles_per_win, H], F32,
                                 tag="ph_b_gath", name="ph_b_gath")
                nc.gpsimd.dma_gather(
                    gath_g[:], xagg_full[:],
                    sgidx[:, w * D.PW // 16:(w + 1) * D.PW // 16],
                    num_idxs=D.PW, num_idxs_reg=D.PW, elem_size=H)
                bhot = wp.tile([3 * D.VOCAB, D.PW], F32, tag="ph_b_bhot",
                               name="ph_b_bhot")
                nc.sync.dma_start(bhot[:],
                                  I["BHOT"].ap()[:, w * D.PW:(w + 1) * D.PW])
                ps_agg = pp.tile([128, H], F32, tag="ps_agg", name="ps_agg")
                for i in range(tiles_per_win):
                    t = w * tiles_per_win + i
                    # edge embedding: one-hot bond matmul (K=15)
                    ps_emb = pp.tile([128, H], F32, tag="ps_mm", name="ps_mm")
                    nc.tensor.matmul(ps_emb[:], bhot[:, _ts(i, 128)],
                                     C["bondcat"][:], start=True, stop=True)
                    # msg = relu(gathered + emb)
                    msg = sp.tile([128, H], F32, tag="ph_b_msg", name="ph_b_msg")
                    nc.vector.tensor_tensor(msg[:], gath_g[:, i, :], ps_emb[:],
                                            ALU.add)
                    nc.scalar.activation(msg[:], msg[:], ACTF.Relu)
                    # one-hot scatter
                    oh = sp.tile([128, D.gwin], F32, tag="ph_b_oh", name="ph_b_oh")
                    nc.vector.tensor_scalar(oh[:], C["iota"][:, :D.gwin],
                                            drel[:, t:t + 1], None, ALU.is_equal)
                    nc.tensor.matmul(ps_agg[:m, :], oh[:, :m], msg[:],
                                     start=(i == 0), stop=(i == tiles_per_win - 1))
                # t = (1+eps) * x_agg + agg_g  (node-major window rows)
                xa = sp.tile([128, H], F32, tag="ph_b_xa", name="ph_b_xa")
                nc.sync.dma_start(
                    xa[:m, :], xagg_shard[w * D.gwin:w * D.gwin + m, :])
                tn = sp.tile([128, H], F32, tag="ph_b_tn", name="ph_b_tn")
                nc.vector.tensor_scalar(tn[:m, :], xa[:m, :], C["epsv"][:m, :],
                                        None, ALU.mult)
                nc.vector.tensor_tensor(tn[:m, :], tn[:m, :], ps_agg[:m, :],
                                        ALU.add)
                # transpose to feature-major tT
                for k in range(2):
                    pst = pp.tile([128, 128], F32, tag="ps_tr", name="ps_tr")
                    nc.tensor.transpose(pst[:, :m], tn[:m, _ts(k, 128)],
                                        C["ident"][:m, :m])
                    nc.vector.tensor_copy(tT[k][:, w * D.gwin:w * D.gwin + m],
                                          pst[:, :m])

            def gin_mm_and_stats_issue(inT, Wc, uT, stats_in, stats_out, label):
                """u = in @ W (node-major tiles), transpose to uT, stats;
                issues the stats AllReduce but does NOT read the result."""
                for t in range(GT):
                    m = min(128, GS - t * 128)
                    ps = pp.tile([128, H], F32, tag="ps_mm", name="ps_mm")
                    for k in range(2):
                        nc.tensor.matmul(ps[:m, :],
                                         inT[k][:, t * 128:t * 128 + m],
                                         Wc[k][:], start=(k == 0), stop=(k == 1))
                    sb = sp.tile([128, H], F32, tag=f"{label}_sb", name=f"{label}_sb")
                    nc.scalar.copy(sb[:m, :], ps[:m, :])
                    for k in range(2):
                        pst = pp.tile([128, 128], F32, tag="ps_tr", name="ps_tr")
                        nc.tensor.transpose(pst[:, :m], sb[:m, _ts(k, 128)],
                                            C["ident"][:m, :m])
                        nc.vector.tensor_copy(uT[k][:, t * 128:t * 128 + m],
                                              pst[:, :m])
                st = sp.tile([128, 4], F32, tag=f"{label}_st", name=f"{label}_st")
                sq = sp.tile([128, GS], F32, tag="gin_sq", name="gin_sq",
                             bufs=1)
                for k in range(2):
                    nc.vector.tensor_reduce(st[:, 2 * k:2 * k + 1], uT[k][:],
                                            AX.X, ALU.add)
                    nc.vector.tensor_tensor(sq[:], uT[k][:], uT[k][:], ALU.mult)
                    nc.vector.tensor_reduce(st[:, 2 * k + 1:2 * k + 2], sq[:],
                                            AX.X, ALU.add)
                nc.sync.dma_start(stats_in[:], st[:])
                if "no_b_ar" not in flags:
                    nc.gpsimd.collective_compute(
                        "AllReduce", ALU.add, replica_groups=groups,
                        ins=[stats_in.opt()], outs=[stats_out.opt()])

            def gin_read_stf(stats_in, stats_out, label):
                stf = sp.tile([128, 4], F32, tag=f"{label}_stf", name=f"{label}_stf")
                nc.sync.dma_start(
                    stf[:], stats_in[:] if "no_b_ar" in flags else stats_out[:])
                return stf

            def bn_apply(stf, uT, g_c, b_c, outT_t, relu, label):
                """out = func((u - mu) * g / sqrt(var+eps) + b), feature-major."""
                inv_n = 1.0 / float(D.G)
                for k in range(2):
                    mu = sp.tile([128, 1], F32, tag=f"{label}_mu{k}", name=f"{label}_mu{k}")
                    nc.vector.tensor_scalar(mu[:], stf[:, 2 * k:2 * k + 1],
                                            inv_n, None, ALU.mult)
                    var = sp.tile([128, 1], F32, tag=f"{label}_va{k}", name=f"{label}_va{k}")
                    nc.vector.tensor_scalar(var[:], stf[:, 2 * k + 1:2 * k + 2],
                                            inv_n, None, ALU.mult)
                    mu2 = sp.tile([128, 1], F32, tag=f"{label}_m2{k}", name=f"{label}_m2{k}")
                    nc.vector.tensor_tensor(mu2[:], mu[:], mu[:], ALU.mult)
                    nc.vector.tensor_tensor(var[:], var[:], mu2[:], ALU.subtract)
                    sd = sp.tile([128, 1], F32, tag=f"{label}_sd{k}", name=f"{label}_sd{k}")
                    nc.scalar.activation(sd[:], var[:], ACTF.Sqrt,
                                         bias=C["eps5col"][:])
                    rs = sp.tile([128, 1], F32, tag=f"{label}_rs{k}", name=f"{label}_rs{k}")
                    nc.vector.reciprocal(rs[:], sd[:])
                    sc = sp.tile([128, 1], F32, tag=f"{label}_sc{k}", name=f"{label}_sc{k}")
                    nc.vector.tensor_tensor(sc[:], g_c[k][:], rs[:], ALU.mult)
                    sh = sp.tile([128, 1], F32, tag=f"{label}_sh{k}", name=f"{label}_sh{k}")
                    nc.vector.tensor_tensor(sh[:], mu[:], sc[:], ALU.mult)
                    nc.vector.tensor_tensor(sh[:], b_c[k][:], sh[:], ALU.subtract)
                    nc.scalar.activation(outT_t[k][:], uT[k][:],
                                         ACTF.Relu if relu else ACTF.Identity,
                                         bias=sh[:], scale=sc[:])

            def gin_buf(nm):
                return [gp.tile([128, GS], F32, tag=f"ginbuf{k}",
                                name=f"{nm}{k}") for k in range(2)]

            bstate = {}
            if "no_b" in flags:
                ginT = gin_buf("ginT")
                for k in range(2):
                    nc.vector.memset(ginT[k][:], 0.0)
                bstate["ginT"] = ginT
            else:
                uT = gin_buf("uT")
                gin_mm_and_stats_issue(tT, C["gw1"], uT, st1_in, st1_out,
                                       "gmm1")

            # xf AG#1 queued on the collective engine AFTER st1's AllReduce
            # so the (tiny) stats reduce isn't stuck behind the 25MB gather
            if "no_coll" not in flags and "no_coll_xf" not in flags:
                nc.gpsimd.collective_compute(
                    "AllGather", ALU.bypass, replica_groups=groups,
                    ins=[xf_shard[1].opt()], outs=[xf_full[1].opt()])

            def emit_b2():
                """bn1 + gmm2 + st2 AllReduce issue (mid phase C)."""
                if "no_b" in flags or "t2T" in bstate:
                    return
                stf1 = gin_read_stf(st1_in, st1_out, "gmm1")
                t1T = gin_buf("t1T")
                bn_apply(stf1, uT, C["bn1g"], C["bn1b"], t1T, True, "bn1")
                t2T = gin_buf("t2T")
                gin_mm_and_stats_issue(t1T, C["gw2"], t2T, st2_in, st2_out,
                                       "gmm2")
                bstate["t2T"] = t2T

            def emit_b3():
                """bn2 -> ginT (late in phase C)."""
                if "no_b" in flags or "ginT" in bstate:
                    return
                stf2 = gin_read_stf(st2_in, st2_out, "gmm2")
                ginT = gin_buf("ginT")
                bn_apply(stf2, bstate["t2T"], C["bn2g"], C["bn2b"], ginT,
                         False, "bn2")
                bstate["ginT"] = ginT

            # =========== Phase C: conformer edge pipeline ===========
            # window-major: per (src-half stream h, dst-window w of 128
            # nodes): gather the window's edges (one call per src quad
            # bucket of B_E), compute msg = (xf[src]) * (mlp(A) + b2), then
            # aggregate over dst via one-hot matmuls into PSUM (the one-hot
            # rows carry the cosine-cutoff C so no separate C-multiply),
            # and flush the window's 128 agg rows with a plain DMA write.
            # No scatter-add: each agg row is written exactly once.
            # resident: C row (cosine cutoff) and dst-rel row per edge
            crow = cp.tile([128, D.E_pad // 128], F32, name="crow_sb")
            for s0 in range(0, D.E_pad // 128, 512):
                sw = min(512, D.E_pad // 128 - s0)
                wt = wp.tile([128, 512], F32, tag="ph_c_wt", name="ph_c_wt")
                nc.sync.dma_start(wt[:, :sw], I["WT"].ap()[:, s0:s0 + sw])
                nc.scalar.activation(wt[:, :sw], wt[:, :sw], ACTF.Sin,
                                     bias=C["pihalf"][:],
                                     scale=math.pi / D.CUTOFF)
                nc.scalar.activation(crow[:, s0:s0 + sw], wt[:, :sw],
                                     ACTF.Copy, bias=0.5, scale=-0.5)
            drelc = cp.tile([128, D.E_pad // 128], F32, name="drelc_sb")
            nc.sync.dma_start(drelc[:], I["DRELC"].ap())

            B_E, CW, EH = D.B_E, D.CW, D.EH
            WE = 2 * B_E           # edges per (stream, window)
            NTW = WE // 128        # tiles per window (6)
            wstream = ([] if "no_c" in flags else
                       [(h, w) for h in range(2) for w in range(CW)])
            PFD = 4  # gather prefetch depth (windows issued ahead)
            gat_fifo = []
            F32R = mybir.dt.float32r

            def emit_gather(idx):
                h, w = wstream[idx]
                e0 = h * EH + w * WE
                if "no_gather" in flags:
                    if "no_cmm" in flags:
                        return None
                    gat = wp.tile([128, NTW, NF], F32, tag="ph_c_gat",
                                  name="ph_c_gat", bufs=PFD + 2)
                    nc.vector.memset(gat[:], 0.0)
                    return gat
                gat = wp.tile([128, NTW, NF], F32, tag="ph_c_gat",
                              name="ph_c_gat", bufs=PFD + 2)
                si = wp.tile([128, WE // 16], I16, tag="ph_c_si",
                             name="ph_c_si", bufs=PFD + 2)
                nc.sync.dma_start(
                    si[:], I["SRC"].ap()[:, e0 // 16:(e0 + WE) // 16])
                for j in range(2):
                    nc.gpsimd.dma_gather(
                        gat[:, j * (NTW // 2):(j + 1) * (NTW // 2), :],
                        xf_full[h][j * D.qsize:(j + 1) * D.qsize, :],
                        si[:, j * B_E // 16:(j + 1) * B_E // 16],
                        num_idxs=B_E, num_idxs_reg=B_E, elem_size=NF,
                        queue_num=idx % 2)
                return gat

            def emit_compute_flush(idx, gat):
                h, w = wstream[idx]
                e0 = h * EH + w * WE
                c0col = e0 // 128
                msg = None
                if "no_cmm" not in flags:
                    msg = wp.tile([128, NTW, NF], F32, tag="ph_c_msg",
                                  name="ph_c_msg")
                    at = wp.tile([NG, WE], F32, tag="ph_c_at",
                                 name="ph_c_at")
                    nc.sync.dma_start(at[:], I["AT"].ap()[:, e0:e0 + WE])
                    # per 384-edge group: mm1+relu (fp32r), mm2 + b2 packed
                    # 3 tiles into one PSUM bank; 384-wide msg-mul
                    for g in range(2):
                        s0 = g * 384
                        ps1 = pp.tile([128, 384], F32, tag="ps_mm", name="ps_mm")
                        nc.tensor.matmul(ps1[:], C["w1"][:],
                                         at[:, s0:s0 + 384],
                                         start=True, stop=True)
                        h1 = wp.tile([128, 384], F32, tag="ph_c_h1",
                                     name="ph_c_h1")
                        nc.scalar.activation(h1[:], ps1[:],
                                             ACTF.Relu, bias=C["b1col"][:])
                        psw = pp.tile([128, 3, NF], F32, tag="ps_w", name="ps_w")
                        for t3 in range(3):
                            nc.tensor.matmul(psw[:, t3, :], h1[:, _ts(t3, 128)],
                                             C["w2"][:], start=True, stop=False)
                            nc.tensor.matmul(psw[:, t3, :], C["ones1"][:],
                                             C["b2row"][:], start=False,
                                             stop=True)
                        nc.vector.tensor_tensor(msg[:, 3 * g:3 * g + 3, :],
                                                gat[:, 3 * g:3 * g + 3, :],
                                                psw[:], ALU.mult)
                if "no_scatter" in flags or "no_cmm" in flags:
                    return
                # one-hot aggregation: rows carry C; accumulate over tiles
                ps_agg = pp.tile([128, NF], F32, tag="ps_agg", name="ps_cagg")
                for t in range(NTW):
                    ohc = sp.tile([128, 128], F32, tag="ph_c_oh",
                                  name="ph_c_oh")
                    nc.vector.tensor_scalar(
                        ohc[:], C["iota"][:],
                        drelc[:, c0col + t:c0col + t + 1],
                        crow[:, c0col + t:c0col + t + 1],
                        ALU.is_equal, ALU.mult)
                    nc.tensor.matmul(ps_agg[:], ohc[:], msg[:, t, :],
                                     start=(t == 0), stop=(t == NTW - 1))
                stg = sp.tile([128, NF], F32, tag="ph_c_stg", name="ph_c_stg")
                nc.scalar.copy(stg[:], ps_agg[:])
                base = w * 128
                m = min(128, NS - base)
                nc.sync.dma_start(
                    agg_ab[h][base:base + m, :].rearrange(
                        "(t p) f -> p t f", p=m),
                    stg[:m, :].rearrange("p (t f) -> p t f", f=NF))

            for i in range(len(wstream) + PFD):
                if i < len(wstream):
                    gat_fifo.append(emit_gather(i))
                if i >= PFD:
                    done = i - PFD
                    emit_compute_flush(done, gat_fifo[done])
                    # GIN stage hooks: latency of the stats AllReduces and
                    # the serial BN chains hides under the window stream
                    if done == CW // 4:
                        emit_b2()
                    if done == CW + CW // 4:
                        emit_b3()
            emit_b2()  # no-op unless phase C was skipped
            emit_b3()
            ginT = bstate["ginT"]

            # =========== Phase D: h = relu(agg@lin2+b)@linw+b, residual =====
            NCH = D.nchunk
            n_nch = NS // NCH
            for j in range(0 if "no_d" in flags else n_nch):
                r0 = j * NCH
                # load agg rows, transpose to feature-major aggT [NF, NCH]
                aggT = wp.tile([NF, NCH], F32, tag="ph_d_aggT", name="ph_d_aggT")
                ntt = (NCH + 127) // 128
                for t in range(ntt):
                    m = min(128, NCH - t * 128)
                    asb = sp.tile([128, NF], F32, tag="ph_d_asb", name="ph_d_asb")
                    nc.sync.dma_start(asb[:m, :],
                                      agg_ab[0][r0 + t * 128:r0 + t * 128 + m, :])
                    bsb = sp.tile([128, NF], F32, tag="ph_d_bsb", name="ph_d_bsb")
                    nc.sync.dma_start(bsb[:m, :],
                                      agg_ab[1][r0 + t * 128:r0 + t * 128 + m, :])
                    nc.vector.tensor_tensor(asb[:m, :], asb[:m, :], bsb[:m, :],
                                            ALU.add)
                    pst = pp.tile([128, 128], F32, tag="ps_tr", name="ps_tr")
                    nc.tensor.transpose(pst[:, :m], asb[:m, :], C["ident"][:m, :m])
                    nc.vector.tensor_copy(aggT[:, t * 128:t * 128 + m],
                                          pst[:, :m])
                # h1T = relu(lin2^T @ aggT + b)  [2][128, NCH]
                h1T = [wp.tile([128, NCH], F32, tag=f"ph_d_h1T{k}", name=f"ph_d_h1T{k}")
                       for k in range(2)]
                for k in range(2):
                    ps = pp.tile([128, NCH], F32, tag="ps_mm", name="ps_mm")
                    nc.tensor.matmul(ps[:], C["lin2"][:, _ts(k, 128)], aggT[:],
                                     start=True, stop=True)
                    nc.scalar.activation(h1T[k][:], ps[:], ACTF.Relu,
                                         bias=C["lin2b"][k][:])
                # outT = linw^T @ h1T + linb + xT + gin[batch]
                for k in range(2):
                    ps = pp.tile([128, NCH], F32, tag="ps_mm", name="ps_mm")
                    for kk in range(2):
                        nc.tensor.matmul(ps[:], C["linw"][kk][:, _ts(k, 128)],
                                         h1T[kk][:], start=(kk == 0),
                                         stop=(kk == 1))
                    ob = sp.tile([128, NCH], F32, tag="ph_d_ob", name="ph_d_ob")
                    nc.scalar.activation(ob[:], ps[:], ACTF.Identity,
                                         bias=C["linb"][k][:])
                    xtc = sp.tile([128, NCH], F32, tag="ph_d_xtc", name="ph_d_xtc")
                    nc.sync.dma_start(xtc[:], I["xT"].ap()[k, :, r0:r0 + NCH])
                    nc.vector.tensor_tensor(ob[:], ob[:], xtc[:], ALU.add)
                    # + gin, each graph col repeated `rep` times
                    rep = D.N // D.G
                    g0 = r0 // rep
                    gin_rep = ginT[k][:, g0:g0 + NCH // rep].broadcast_to(
                        (128, NCH // rep, rep))
                    nc.vector.tensor_tensor(
                        ob[:].rearrange("p (g t) -> p g t", t=rep),
                        ob[:].rearrange("p (g t) -> p g t", t=rep),
                        gin_rep, ALU.add)
                    nc.sync.dma_start(outT.ap()[k, :, r0:r0 + NCH], ob[:])

    nc.compile()
    return nc


_CACHE = {}


def _get_nc(D: Dims):
    key = ("nc", D)
    if key not in _CACHE:
        _CACHE[key] = build_nc(D)
    return _CACHE[key]


def run_on_hw(inputs, D: Dims = REAL):
    nc = _get_nc(D)
    in_maps = host_prep(inputs, D)
    res = bass_utils.run_bass_kernel_spmd(nc, in_maps,
                                          core_ids=list(range(D.cores)))
    return assemble(res.results, D)


def kernel(**inputs):
    return run_on_hw(inputs, REAL)



# revision 42
# speedup vs baseline: 1.0669x; 1.0669x over previous
"""Trainium2 Bass kernel for nn_DSSConf (DSS conformer GNN message passing).

Self-contained: hardcodes shapes/sharding for the real problem; exposes
kernel(**inputs) -> np.ndarray.
"""
import sys
import math
from dataclasses import dataclass

sys.path.insert(0, "/opt/trn_rl_repo")

import numpy as np
from concourse import bass, bacc, tile, mybir, bass_utils

F32 = mybir.dt.float32
I16 = mybir.dt.int16
ALU = mybir.AluOpType
ACTF = mybir.ActivationFunctionType
AX = mybir.AxisListType


@dataclass(frozen=True)
class Dims:
    N: int = 100000        # conformer nodes
    H: int = 256           # hidden
    NF: int = 128          # num filters
    NG: int = 50           # num gaussians
    G: int = 10000         # graph nodes
    E: int = 1000000       # conformer edges
    EG: int = 30000        # graph edges
    VOCAB: int = 5
    CUTOFF: float = 10.0
    cores: int = 8
    qsize: int = 25000     # src quadrant size for int16 gather indices
    B_E: int = 384         # fixed edges per (dst-window, src-quad) bucket
    gwin: int = 128        # GIN scatter window (<=128 segments)
    PW: int = 640          # padded GIN edges per (core, window) (multiple of 128)
    nchunk: int = 500      # node chunk for the h/out stage (divides NS, mult of 10)

    @property
    def NS(self):
        return self.N // self.cores

    @property
    def GS(self):
        return self.G // self.cores

    @property
    def CW(self):
        """dst windows of 128 nodes per core shard."""
        return (self.NS + 127) // 128

    @property
    def EH(self):
        """padded edges per src-half stream (2 quad buckets per window)."""
        return self.CW * 2 * self.B_E

    @property
    def E_pad(self):
        return 2 * self.EH

    @property
    def NWIN(self):
        return (self.GS + self.gwin - 1) // self.gwin

    @property
    def EG_pad(self):
        return self.NWIN * self.PW


REAL = Dims()


def _wrap16(arr, dtype=np.int16):
    """Edge i -> [i % 16, i // 16], replicated to 128 partitions."""
    a = np.asarray(arr).reshape(-1, 16).T.astype(dtype)  # [16, n/16]
    return np.tile(a, (8, 1)).copy()  # [128, n/16]


def _tile128(arr, dtype=np.float32):
    """Edge i -> [i % 128, i // 128] (per-partition scalar layout)."""
    return np.ascontiguousarray(np.asarray(arr).reshape(-1, 128).T.astype(dtype))


def host_prep(inputs, D: Dims):
    """Build per-core in_maps (list of dicts) for the SPMD kernel."""
    x = np.asarray(inputs["x"], np.float32)
    cnb = np.asarray(inputs["conf_node_batch"]).astype(np.int64)
    ei = np.asarray(inputs["edge_index_conf"]).astype(np.int64)
    ew = np.asarray(inputs["edge_weight_conf"], np.float32)
    ea = np.asarray(inputs["edge_attr_conf"], np.float32)
    eig = np.asarray(inputs["edge_index_graph"]).astype(np.int64)
    eag = np.asarray(inputs["edge_attr_graph"]).astype(np.int64)

    rep = D.N // D.G
    assert np.array_equal(cnb, np.repeat(np.arange(D.G), rep)), \
        "conf_node_batch structure mismatch"

    NS, GS = D.NS, D.GS
    src, dst = ei[0], ei[1]

    # ---- conformer edges: window-major one-hot layout ----
    # per core: two streams by src half (matching the split AllGather's
    # permuted xf layout); within a stream, buckets of fixed B_E edges per
    # (dst-window of 128, src quad within the half); in-quad src offset =
    # (src_core % 4)*NS/2 + (src % NS/2)
    B_E, CW, EH = D.B_E, D.CW, D.EH
    core = dst // NS
    sc = src // NS
    so = src % NS
    sh = so // (NS // 2)
    sj = (sc >= 4).astype(np.int64)
    inq = (sc % 4) * (NS // 2) + (so % (NS // 2))
    win = (dst % NS) // 128
    bucket = ((core * 2 + sh) * CW + win) * 2 + sj
    order = np.argsort(bucket, kind="stable")
    sb = bucket[order]
    nbuck = D.cores * 2 * CW * 2
    bounds = np.searchsorted(sb, np.arange(nbuck + 1))
    counts = bounds[1:] - bounds[:-1]
    assert counts.max() <= B_E, f"bucket overflow: {counts.max()} > {B_E}"
    rank = np.arange(len(sb)) - bounds[sb]
    c_b = sb // (2 * CW * 2)
    rem = sb % (2 * CW * 2)
    h_b = rem // (CW * 2)
    w_b = (rem % (CW * 2)) // 2
    j_b = rem % 2
    flat = (c_b * D.E_pad + h_b * EH + (w_b * 2 + j_b) * B_E + rank)

    src_pad = np.zeros(D.cores * D.E_pad, np.int64)
    drel_pad = np.full(D.cores * D.E_pad, -1.0, np.float32)
    w_pad = np.full(D.cores * D.E_pad, D.CUTOFF, np.float32)  # C(CUTOFF)=0
    a_pad = np.zeros((D.cores * D.E_pad, D.NG), np.float32)
    src_pad[flat] = inq[order]
    drel_pad[flat] = ((dst[order] % NS) - w_b * 128).astype(np.float32)
    w_pad[flat] = ew[order]
    a_pad[flat] = ea[order]

    AT = np.zeros((D.cores, D.NG, D.E_pad), np.float32)
    WT = np.zeros((D.cores, 128, D.E_pad // 128), np.float32)
    DRELC = np.zeros((D.cores, 128, D.E_pad // 128), np.float32)
    SRC = np.zeros((D.cores, 128, D.E_pad // 16), np.int16)
    for c in range(D.cores):
        sl = slice(c * D.E_pad, (c + 1) * D.E_pad)
        AT[c] = a_pad[sl].T
        WT[c] = _tile128(w_pad[sl])
        DRELC[c] = _tile128(drel_pad[sl])
        SRC[c] = _wrap16(src_pad[sl])

    # ---- graph edges: order by (core(dst), window(dst), dst) ----
    sg, dg = eig[0], eig[1]
    gcore = dg // GS
    gwin = (dg - gcore * GS) // D.gwin
    gorder = np.lexsort((dg, gwin, gcore))
    g_s, g_d, g_w, g_c = sg[gorder], dg[gorder], gwin[gorder], gcore[gorder]
    g_a = eag[gorder]

    SG = np.zeros((D.cores, 128, D.EG_pad // 16), np.int16)
    DREL = np.zeros((D.cores, 128, D.EG_pad // 128), np.float32)
    BHOT = np.zeros((D.cores, 3 * D.VOCAB, D.EG_pad), np.float32)

    gkeys = g_c * D.NWIN + g_w
    gbounds = np.searchsorted(gkeys, np.arange(D.cores * D.NWIN + 1))
    for c in range(D.cores):
        sg_pad = np.zeros(D.EG_pad, np.int64)
        dr_pad = np.full(D.EG_pad, -1.0, np.float32)  # -1 kills pads in one-hot
        bh_pad = np.zeros((3 * D.VOCAB, D.EG_pad), np.float32)
        for w in range(D.NWIN):
            lo, hi = gbounds[c * D.NWIN + w], gbounds[c * D.NWIN + w + 1]
            cnt = hi - lo
            assert cnt <= D.PW, f"PW overflow: core {c} win {w}: {cnt}"
            o = w * D.PW
            sg_pad[o:o + cnt] = g_s[lo:hi]
            dr_pad[o:o + cnt] = (g_d[lo:hi] - c * GS - w * D.gwin).astype(np.float32)
            for k in range(3):
                bh_pad[k * D.VOCAB + g_a[lo:hi, k], np.arange(o, o + cnt)] = 1.0
        SG[c] = _wrap16(sg_pad)
        DREL[c] = _tile128(dr_pad)
        BHOT[c] = bh_pad

    # ---- x^T shards ----
    xT = np.ascontiguousarray(x.T)  # [H, N]
    XT = xT.reshape(2, 128, D.N)

    # ---- weights (replicated) ----
    H2 = D.H // 128
    w = {k: np.asarray(inputs[k], np.float32) for k in (
        "mlp_w1", "mlp_b1", "mlp_w2", "mlp_b2", "cf_lin1", "cf_lin2",
        "cf_lin2_b", "lin_w", "lin_b", "bond_emb", "gin_eps", "gin_w1",
        "gin_w2", "bn1_g", "bn1_b", "bn2_g", "bn2_b")}
    const = dict(
        w1=w["mlp_w1"],                                   # [NG, NF]
        b1col=w["mlp_b1"].reshape(D.NF, 1),
        w2=w["mlp_w2"],                                   # [NF, NF]
        b2row=w["mlp_b2"].reshape(1, D.NF),
        b2row3=np.tile(w["mlp_b2"].reshape(1, D.NF), (1, 3)),
        ones1=np.ones((1, 128), np.float32),
        lin1=np.ascontiguousarray(w["cf_lin1"].reshape(H2, 128, D.NF)),
        lin2=w["cf_lin2"],                                # [NF, H]
        lin2b=w["cf_lin2_b"].reshape(H2, 128, 1),
        linw=np.ascontiguousarray(w["lin_w"].reshape(H2, 128, D.H)),
        linb=w["lin_b"].reshape(H2, 128, 1),
        gw1=np.ascontiguousarray(w["gin_w1"].reshape(H2, 128, D.H)),
        gw2=np.ascontiguousarray(w["gin_w2"].reshape(H2, 128, D.H)),
        bondcat=np.ascontiguousarray(
            w["bond_emb"].reshape(3 * D.VOCAB, D.H)),
        bn1g=w["bn1_g"].reshape(H2, 128, 1), bn1b=w["bn1_b"].reshape(H2, 128, 1),
        bn2g=w["bn2_g"].reshape(H2, 128, 1), bn2b=w["bn2_b"].reshape(H2, 128, 1),
        epsv=np.full((128, 1), 1.0 + float(w["gin_eps"]), np.float32),
        zerocol=np.zeros((128, 1), np.float32),
        eps5col=np.full((128, 1), 1e-5, np.float32),
        pihalf=np.full((128, 1), -math.pi / 2, np.float32),
        iota=np.tile(np.arange(128, dtype=np.float32), (128, 1)).copy(),
        ident=np.eye(128, dtype=np.float32),
    )

    in_maps = []
    for c in range(D.cores):
        m = dict(
            xT=np.ascontiguousarray(XT[:, :, c * NS:(c + 1) * NS]),
            AT=AT[c], WT=WT[c], SRC=SRC[c], DRELC=DRELC[c],
            SG=SG[c], DREL=DREL[c], BHOT=BHOT[c],
        )
        m.update(const)
        in_maps.append(m)
    return in_maps


def assemble(results, D: Dims):
    """Per-core outT [2,128,NS] -> full [N, H]."""
    parts = [r["outT"].reshape(D.H, D.NS) for r in results]
    outT = np.concatenate(parts, axis=1)  # [H, N]
    return np.ascontiguousarray(outT.T)


def _ts(i, n):
    return bass.ts(i, n)


def build_nc(D: Dims, flags: frozenset = frozenset()):
    nc = bacc.Bacc("TRN2", target_bir_lowering=False, debug=False,
                   num_devices=D.cores, num_swdge_queues=3)
    NS, GS, H, NF, NG = D.NS, D.GS, D.H, D.NF, D.NG
    H2 = H // 128

    I = {}
    def di(name, shape, dt=F32):
        I[name] = nc.dram_tensor(name, list(shape), dt, kind="ExternalInput")
        return I[name]

    di("xT", [2, 128, NS])
    di("AT", [NG, D.E_pad])
    di("WT", [128, D.E_pad // 128])
    di("SRC", [128, D.E_pad // 16], I16)
    di("DRELC", [128, D.E_pad // 128])
    di("SG", [128, D.EG_pad // 16], I16)
    di("DREL", [128, D.EG_pad // 128])
    di("BHOT", [3 * D.VOCAB, D.EG_pad])
    di("w1", [NG, NF]); di("b1col", [NF, 1]); di("w2", [NF, NF])
    di("b2row", [1, NF]); di("b2row3", [1, 3 * NF]); di("ones1", [1, 128])
    di("lin1", [H2, 128, NF]); di("lin2", [NF, H]); di("lin2b", [H2, 128, 1])
    di("linw", [H2, 128, H]); di("linb", [H2, 128, 1])
    di("gw1", [H2, 128, H]); di("gw2", [H2, 128, H])
    di("bondcat", [3 * D.VOCAB, H])
    di("bn1g", [H2, 128, 1]); di("bn1b", [H2, 128, 1])
    di("bn2g", [H2, 128, 1]); di("bn2b", [H2, 128, 1])
    di("epsv", [128, 1]); di("iota", [128, 128]); di("ident", [128, 128])
    di("zerocol", [128, 1]); di("eps5col", [128, 1]); di("pihalf", [128, 1])

    outT = nc.dram_tensor("outT", [2, 128, NS], F32, kind="ExternalOutput")

    groups = [list(range(D.cores))]

    with tile.TileContext(nc) as tc:
        with (
            tc.tile_pool(name="const", bufs=1) as cp,
            tc.tile_pool(name="work", bufs=2) as wp,
            tc.tile_pool(name="small", bufs=3) as sp,
            tc.tile_pool(name="gin", bufs=2) as ctx_gin_pool,
            tc.tile_pool(name="psum", bufs=2, space="PSUM") as pp,
            tc.tile_pool(name="dram", bufs=1, space="DRAM") as dp,
        ):
            # ---------- load constants ----------
            C = {}
            for nm, shp in [("w1", [NG, NF]), ("b1col", [NF, 1]),
                            ("w2", [NF, NF]), ("b2row", [1, NF]),
                            ("b2row3", [1, 3 * NF]),
                            ("ones1", [1, 128]), ("lin2", [NF, H]),
                            ("bondcat", [3 * D.VOCAB, H]),
                            ("epsv", [128, 1]), ("iota", [128, 128]),
                            ("ident", [128, 128]), ("zerocol", [128, 1]),
                            ("eps5col", [128, 1]), ("pihalf", [128, 1])]:
                t = cp.tile(shp, F32, name=f"c_{nm}")
                nc.sync.dma_start(t[:], I[nm].ap())
                C[nm] = t
            nc.const_aps.aps[(F32, 0.0)] = C["zerocol"][:]
            # [H2,128,*] constants: load as per-half tiles
            for nm in ("lin1", "lin2b", "linw", "linb", "gw1", "gw2",
                       "bn1g", "bn1b", "bn2g", "bn2b"):
                C[nm] = []
                inner = I[nm].shape[2]
                for k in range(H2):
                    t = cp.tile([128, inner], F32, name=f"c_{nm}{k}")
                    nc.sync.dma_start(t[:], I[nm].ap()[k])
                    C[nm].append(t)

            # ---------- DRAM scratch ----------
            # xf shard/full split in half so the AllGather pipelines with
            # compute: AG#h gathers every core's half-h shard; the gathered
            # layout is permuted (half-major), host_prep permutes src indices
            HSH = NS // 2
            xf_shard = [dp.tile([HSH, NF], F32, name=f"xf_shard{h}")
                        for h in range(2)]
            xf_full = [dp.tile([D.cores * HSH, NF], F32, name=f"xf_full{h}",
                               addr_space="Shared") for h in range(2)]
            xagg_shard = dp.tile([GS, H], F32, name="xagg_shard")
            xagg_full = dp.tile([D.cores * GS, H], F32, name="xagg_full",
                                addr_space="Shared")
            agg_ab = [dp.tile([NS, NF], F32, name=f"agg_{h}")
                      for h in range(2)]
            st1_in = dp.tile([128, 4], F32, name="st1_in")
            st1_out = dp.tile([128, 4], F32, name="st1_out", addr_space="Shared")
            st2_in = dp.tile([128, 4], F32, name="st2_in")
            st2_out = dp.tile([128, 4], F32, name="st2_out", addr_space="Shared")

            # =========== Phase A: segment-max pool first, then xf ===========
            # pass 1 pools so the (small) xagg AllGather is issued as early
            # as possible; pass 2 re-reads x and computes xf, issuing each
            # half's AllGather as soon as that half of the shard is written
            rep = D.N // D.G
            PCH = 250
            n_pch = NS // PCH
            half_chunks = n_pch // 2
            xaggT = [cp.tile([128, GS], F32, name=f"xaggT{k}") for k in range(2)]
            for j in range(n_pch):
                xt = [wp.tile([128, PCH], F32, tag=f"ph_a_xt{k}", name=f"ph_a_xt{k}")
                      for k in range(2)]
                for k in range(2):
                    nc.sync.dma_start(xt[k][:], I["xT"].ap()[k, :, _ts(j, PCH)])
                # pool: max over groups of 10 cols
                for k in range(2):
                    nc.vector.tensor_reduce(
                        xaggT[k][:, _ts(j, PCH // rep)],
                        xt[k][:].rearrange("p (g t) -> p g t", t=rep),
                        AX.X, ALU.max)

            # transpose x_aggT -> node-major x_agg shard
            GT = (GS + 127) // 128
            for t in range(GT):
                m = min(128, GS - t * 128)
                for k in range(2):
                    pst = pp.tile([128, 128], F32, tag="ps_tr", name="ps_tr")
                    nc.tensor.transpose(pst[:m, :], xaggT[k][:, t * 128:t * 128 + m],
                                        C["ident"][:])
                    sb = sp.tile([128, 128], F32, tag="ph_a_trsb", name="ph_a_trsb")
                    nc.scalar.copy(sb[:m, :], pst[:m, :])
                    nc.sync.dma_start(
                        xagg_shard[t * 128:t * 128 + m, _ts(k, 128)], sb[:m, :])

            if "no_coll" not in flags and "no_coll_xagg" not in flags:
                nc.gpsimd.collective_compute(
                    "AllGather", ALU.bypass, replica_groups=groups,
                    ins=[xagg_shard.opt()], outs=[xagg_full.opt()])

            # pass 2: xf = x @ cf_lin1, node-major tiles of <=128
            for j in range(n_pch):
                xt = [wp.tile([128, PCH], F32, tag=f"ph_a_xt{k}", name=f"ph_a2_xt{k}")
                      for k in range(2)]
                for k in range(2):
                    nc.sync.dma_start(xt[k][:], I["xT"].ap()[k, :, _ts(j, PCH)])
                h = j // half_chunks
                r0 = (j - h * half_chunks) * PCH
                nt = (PCH + 127) // 128
                for t in range(nt):
                    m = min(128, PCH - t * 128)
                    ps = pp.tile([128, NF], F32, tag="ps_mm", name="ps_mm")
                    for k in range(2):
                        nc.tensor.matmul(ps[:m, :], xt[k][:, t * 128:t * 128 + m],
                                         C["lin1"][k][:], start=(k == 0),
                                         stop=(k == 1))
                    sb = sp.tile([128, NF], F32, tag="ph_a_sb", name="ph_a_sb")
                    nc.scalar.copy(sb[:m, :], ps[:m, :])
                    nc.sync.dma_start(
                        xf_shard[h][r0 + t * 128: r0 + t * 128 + m, :],
                        sb[:m, :])
                if (j == half_chunks - 1 and "no_coll" not in flags
                        and "no_coll_xf" not in flags):
                    nc.gpsimd.collective_compute(
                        "AllGather", ALU.bypass, replica_groups=groups,
                        ins=[xf_shard[0].opt()], outs=[xf_full[0].opt()])
            # AG#1 is issued later (after st1's AllReduce) so the GIN stats
            # reduction isn't queued behind it on the collective engine

            # =========== Phase B: GIN branch (sharded by graph node) =========
            # gather x_agg[sg], edge_emb via bond one-hot matmul, relu,
            # one-hot scatter into agg_g windows
            sgidx = cp.tile([128, D.EG_pad // 16], I16, name="sgidx_sb")
            nc.sync.dma_start(sgidx[:], I["SG"].ap())
            drel = cp.tile([128, D.EG_pad // 128], F32, name="drel_sb")
            nc.sync.dma_start(drel[:], I["DREL"].ap())

            # t-buffer (node-major (1+eps)x_agg + agg_g), then transposed halves
            gp = ctx_gin_pool
            tT = [gp.tile([128, GS], F32, tag=f"ginbuf{k}", name=f"tT{k}")
                  for k in range(2)]

            assert D.EG_pad % 128 == 0
            tiles_per_win = D.PW // 128
            for w in range(0 if "no_b" in flags else D.NWIN):
                m = min(D.gwin, GS - w * D.gwin)
                # gather this window's source rows
                gath_g = wp.tile([128, tiles_per_win, H], F32,
                                 tag="ph_b_gath", name="ph_b_gath")
                nc.gpsimd.dma_gather(
                    gath_g[:], xagg_full[:],
                    sgidx[:, w * D.PW // 16:(w + 1) * D.PW // 16],
                    num_idxs=D.PW, num_idxs_reg=D.PW, elem_size=H)
                bhot = wp.tile([3 * D.VOCAB, D.PW], F32, tag="ph_b_bhot",
                               name="ph_b_bhot")
                nc.sync.dma_start(bhot[:],
                                  I["BHOT"].ap()[:, w * D.PW:(w + 1) * D.PW])
                ps_agg = pp.tile([128, H], F32, tag="ps_agg", name="ps_agg")
                for i in range(tiles_per_win):
                    t = w * tiles_per_win + i
                    # edge embedding: one-hot bond matmul (K=15)
                    ps_emb = pp.tile([128, H], F32, tag="ps_mm", name="ps_mm")
                    nc.tensor.matmul(ps_emb[:], bhot[:, _ts(i, 128)],
                                     C["bondcat"][:], start=True, stop=True)
                    # msg = relu(gathered + emb)
                    msg = sp.tile([128, H], F32, tag="ph_b_msg", name="ph_b_msg")
                    nc.vector.tensor_tensor(msg[:], gath_g[:, i, :], ps_emb[:],
                                            ALU.add)
                    nc.scalar.activation(msg[:], msg[:], ACTF.Relu)
                    # one-hot scatter
                    oh = sp.tile([128, D.gwin], F32, tag="ph_b_oh", name="ph_b_oh")
                    nc.vector.tensor_scalar(oh[:], C["iota"][:, :D.gwin],
                                            drel[:, t:t + 1], None, ALU.is_equal)
                    nc.tensor.matmul(ps_agg[:m, :], oh[:, :m], msg[:],
                                     start=(i == 0), stop=(i == tiles_per_win - 1))
                # t = (1+eps) * x_agg + agg_g  (node-major window rows)
                xa = sp.tile([128, H], F32, tag="ph_b_xa", name="ph_b_xa")
                nc.sync.dma_start(
                    xa[:m, :], xagg_shard[w * D.gwin:w * D.gwin + m, :])
                tn = sp.tile([128, H], F32, tag="ph_b_tn", name="ph_b_tn")
                nc.vector.tensor_scalar(tn[:m, :], xa[:m, :], C["epsv"][:m, :],
                                        None, ALU.mult)
                nc.vector.tensor_tensor(tn[:m, :], tn[:m, :], ps_agg[:m, :],
                                        ALU.add)
                # transpose to feature-major tT
                for k in range(2):
                    pst = pp.tile([128, 128], F32, tag="ps_tr", name="ps_tr")
                    nc.tensor.transpose(pst[:, :m], tn[:m, _ts(k, 128)],
                                        C["ident"][:m, :m])
                    nc.vector.tensor_copy(tT[k][:, w * D.gwin:w * D.gwin + m],
                                          pst[:, :m])

            def gin_mm_and_stats_issue(inT, Wc, uT, stats_in, stats_out, label):
                """u = in @ W (node-major tiles), transpose to uT, stats;
                issues the stats AllReduce but does NOT read the result."""
                for t in range(GT):
                    m = min(128, GS - t * 128)
                    ps = pp.tile([128, H], F32, tag="ps_mm", name="ps_mm")
                    for k in range(2):
                        nc.tensor.matmul(ps[:m, :],
                                         inT[k][:, t * 128:t * 128 + m],
                                         Wc[k][:], start=(k == 0), stop=(k == 1))
                    sb = sp.tile([128, H], F32, tag=f"{label}_sb", name=f"{label}_sb")
                    nc.scalar.copy(sb[:m, :], ps[:m, :])
                    for k in range(2):
                        pst = pp.tile([128, 128], F32, tag="ps_tr", name="ps_tr")
                        nc.tensor.transpose(pst[:, :m], sb[:m, _ts(k, 128)],
                                            C["ident"][:m, :m])
                        nc.vector.tensor_copy(uT[k][:, t * 128:t * 128 + m],
                                              pst[:, :m])
                st = sp.tile([128, 4], F32, tag=f"{label}_st", name=f"{label}_st")
                sq = sp.tile([128, GS], F32, tag="gin_sq", name="gin_sq",
                             bufs=1)
                for k in range(2):
                    nc.vector.tensor_reduce(st[:, 2 * k:2 * k + 1], uT[k][:],
                                            AX.X, ALU.add)
                    nc.vector.tensor_tensor(sq[:], uT[k][:], uT[k][:], ALU.mult)
                    nc.vector.tensor_reduce(st[:, 2 * k + 1:2 * k + 2], sq[:],
                                            AX.X, ALU.add)
                nc.sync.dma_start(stats_in[:], st[:])
                if "no_b_ar" not in flags:
                    nc.gpsimd.collective_compute(
                        "AllReduce", ALU.add, replica_groups=groups,
                        ins=[stats_in.opt()], outs=[stats_out.opt()])

            def gin_read_stf(stats_in, stats_out, label):
                stf = sp.tile([128, 4], F32, tag=f"{label}_stf", name=f"{label}_stf")
                nc.sync.dma_start(
                    stf[:], stats_in[:] if "no_b_ar" in flags else stats_out[:])
                return stf

            def bn_apply(stf, uT, g_c, b_c, outT_t, relu, label):
                """out = func((u - mu) * g / sqrt(var+eps) + b), feature-major."""
                inv_n = 1.0 / float(D.G)
                for k in range(2):
                    mu = sp.tile([128, 1], F32, tag=f"{label}_mu{k}", name=f"{label}_mu{k}")
                    nc.vector.tensor_scalar(mu[:], stf[:, 2 * k:2 * k + 1],
                                            inv_n, None, ALU.mult)
                    var = sp.tile([128, 1], F32, tag=f"{label}_va{k}", name=f"{label}_va{k}")
                    nc.vector.tensor_scalar(var[:], stf[:, 2 * k + 1:2 * k + 2],
                                            inv_n, None, ALU.mult)
                    mu2 = sp.tile([128, 1], F32, tag=f"{label}_m2{k}", name=f"{label}_m2{k}")
                    nc.vector.tensor_tensor(mu2[:], mu[:], mu[:], ALU.mult)
                    nc.vector.tensor_tensor(var[:], var[:], mu2[:], ALU.subtract)
                    sd = sp.tile([128, 1], F32, tag=f"{label}_sd{k}", name=f"{label}_sd{k}")
                    nc.scalar.activation(sd[:], var[:], ACTF.Sqrt,
                                         bias=C["eps5col"][:])
                    rs = sp.tile([128, 1], F32, tag=f"{label}_rs{k}", name=f"{label}_rs{k}")
                    nc.vector.reciprocal(rs[:], sd[:])
                    sc = sp.tile([128, 1], F32, tag=f"{label}_sc{k}", name=f"{label}_sc{k}")
                    nc.vector.tensor_tensor(sc[:], g_c[k][:], rs[:], ALU.mult)
                    sh = sp.tile([128, 1], F32, tag=f"{label}_sh{k}", name=f"{label}_sh{k}")
                    nc.vector.tensor_tensor(sh[:], mu[:], sc[:], ALU.mult)
                    nc.vector.tensor_tensor(sh[:], b_c[k][:], sh[:], ALU.subtract)
                    nc.scalar.activation(outT_t[k][:], uT[k][:],
                                         ACTF.Relu if relu else ACTF.Identity,
                                         bias=sh[:], scale=sc[:])

            def gin_buf(nm):
                return [gp.tile([128, GS], F32, tag=f"ginbuf{k}",
                                name=f"{nm}{k}") for k in range(2)]

            bstate = {}
            if "no_b" in flags:
                ginT = gin_buf("ginT")
                for k in range(2):
                    nc.vector.memset(ginT[k][:], 0.0)
                bstate["ginT"] = ginT
            else:
                uT = gin_buf("uT")
                gin_mm_and_stats_issue(tT, C["gw1"], uT, st1_in, st1_out,
                                       "gmm1")

            # xf AG#1 queued on the collective engine AFTER st1's AllReduce
            # so the (tiny) stats reduce isn't stuck behind the 25MB gather
            if "no_coll" not in flags and "no_coll_xf" not in flags:
                nc.gpsimd.collective_compute(
                    "AllGather", ALU.bypass, replica_groups=groups,
                    ins=[xf_shard[1].opt()], outs=[xf_full[1].opt()])

            def emit_b2():
                """bn1 + gmm2 + st2 AllReduce issue (mid phase C)."""
                if "no_b" in flags or "t2T" in bstate:
                    return
                stf1 = gin_read_stf(st1_in, st1_out, "gmm1")
                t1T = gin_buf("t1T")
                bn_apply(stf1, uT, C["bn1g"], C["bn1b"], t1T, True, "bn1")
                t2T = gin_buf("t2T")
                gin_mm_and_stats_issue(t1T, C["gw2"], t2T, st2_in, st2_out,
                                       "gmm2")
                bstate["t2T"] = t2T

            def emit_b3():
                """bn2 -> ginT (late in phase C)."""
                if "no_b" in flags or "ginT" in bstate:
                    return
                stf2 = gin_read_stf(st2_in, st2_out, "gmm2")
                ginT = gin_buf("ginT")
                bn_apply(stf2, bstate["t2T"], C["bn2g"], C["bn2b"], ginT,
                         False, "bn2")
                bstate["ginT"] = ginT

            # =========== Phase C: conformer edge pipeline ===========
            # window-major: per (src-half stream h, dst-window w of 128
            # nodes): gather the window's edges (one call per src quad
            # bucket of B_E), compute msg = (xf[src]) * (mlp(A) + b2), then
            # aggregate over dst via one-hot matmuls into PSUM (the one-hot
            # rows carry the cosine-cutoff C so no separate C-multiply),
            # and flush the window's 128 agg rows with a plain DMA write.
            # No scatter-add: each agg row is written exactly once.
            # resident: C row (cosine cutoff) and dst-rel row per edge
            crow = cp.tile([128, D.E_pad // 128], F32, name="crow_sb")
            for s0 in range(0, D.E_pad // 128, 512):
                sw = min(512, D.E_pad // 128 - s0)
                wt = wp.tile([128, 512], F32, tag="ph_c_wt", name="ph_c_wt")
                nc.sync.dma_start(wt[:, :sw], I["WT"].ap()[:, s0:s0 + sw])
                nc.scalar.activation(wt[:, :sw], wt[:, :sw], ACTF.Sin,
                                     bias=C["pihalf"][:],
                                     scale=math.pi / D.CUTOFF)
                nc.scalar.activation(crow[:, s0:s0 + sw], wt[:, :sw],
                                     ACTF.Copy, bias=0.5, scale=-0.5)
            drelc = cp.tile([128, D.E_pad // 128], F32, name="drelc_sb")
            nc.sync.dma_start(drelc[:], I["DRELC"].ap())

            B_E, CW, EH = D.B_E, D.CW, D.EH
            WE = 2 * B_E           # edges per (stream, window)
            NTW = WE // 128        # tiles per window (6)
            wstream = ([] if "no_c" in flags else
                       [(h, w) for h in range(2) for w in range(CW)])
            PFD = 4  # gather prefetch depth (windows issued ahead)
            gat_fifo = []
            F32R = mybir.dt.float32r

            def emit_gather(idx):
                h, w = wstream[idx]
                e0 = h * EH + w * WE
                if "no_gather" in flags:
                    if "no_cmm" in flags:
                        return None
                    gat = wp.tile([128, NTW, NF], F32, tag="ph_c_gat",
                                  name="ph_c_gat", bufs=PFD + 2)
                    nc.vector.memset(gat[:], 0.0)
                    return gat
                gat = wp.tile([128, NTW, NF], F32, tag="ph_c_gat",
                              name="ph_c_gat", bufs=PFD + 2)
                si = wp.tile([128, WE // 16], I16, tag="ph_c_si",
                             name="ph_c_si", bufs=PFD + 2)
                nc.sync.dma_start(
                    si[:], I["SRC"].ap()[:, e0 // 16:(e0 + WE) // 16])
                for j in range(2):
                    nc.gpsimd.dma_gather(
                        gat[:, j * (NTW // 2):(j + 1) * (NTW // 2), :],
                        xf_full[h][j * D.qsize:(j + 1) * D.qsize, :],
                        si[:, j * B_E // 16:(j + 1) * B_E // 16],
                        num_idxs=B_E, num_idxs_reg=B_E, elem_size=NF,
                        queue_num=idx % 2)
                return gat

            def emit_compute_flush(idx, gat):
                h, w = wstream[idx]
                e0 = h * EH + w * WE
                c0col = e0 // 128
                msg = None
                if "no_cmm" not in flags:
                    msg = wp.tile([128, NTW, NF], F32, tag="ph_c_msg",
                                  name="ph_c_msg")
                    at = wp.tile([NG, WE], F32, tag="ph_c_at",
                                 name="ph_c_at")
                    nc.sync.dma_start(at[:], I["AT"].ap()[:, e0:e0 + WE])
                    # per 384-edge group: mm1+relu (fp32r), mm2 + b2 packed
                    # 3 tiles into one PSUM bank; 384-wide msg-mul
                    for g in range(2):
                        s0 = g * 384
                        ps1 = pp.tile([128, 384], F32, tag="ps_mm", name="ps_mm")
                        nc.tensor.matmul(ps1[:], C["w1"][:],
                                         at[:, s0:s0 + 384],
                                         start=True, stop=True)
                        h1 = wp.tile([128, 384], F32, tag="ph_c_h1",
                                     name="ph_c_h1")
                        nc.scalar.activation(h1[:], ps1[:],
                                             ACTF.Relu, bias=C["b1col"][:])
                        psw = pp.tile([128, 3, NF], F32, tag="ps_w", name="ps_w")
                        for t3 in range(3):
                            nc.tensor.matmul(psw[:, t3, :], h1[:, _ts(t3, 128)],
                                             C["w2"][:], start=True,
                                             stop="b2zero" in flags)
                            if "b2zero" not in flags:
                                nc.tensor.matmul(psw[:, t3, :], C["ones1"][:],
                                                 C["b2row"][:], start=False,
                                                 stop=True)
                        nc.vector.tensor_tensor(msg[:, 3 * g:3 * g + 3, :],
                                                gat[:, 3 * g:3 * g + 3, :],
                                                psw[:], ALU.mult)
                if "no_scatter" in flags or "no_cmm" in flags:
                    return
                # one-hot aggregation: rows carry C; accumulate over tiles
                ps_agg = pp.tile([128, NF], F32, tag="ps_agg", name="ps_cagg")
                for t in range(NTW):
                    ohc = sp.tile([128, 128], F32, tag="ph_c_oh",
                                  name="ph_c_oh")
                    nc.vector.tensor_scalar(
                        ohc[:], C["iota"][:],
                        drelc[:, c0col + t:c0col + t + 1],
                        crow[:, c0col + t:c0col + t + 1],
                        ALU.is_equal, ALU.mult)
                    nc.tensor.matmul(ps_agg[:], ohc[:], msg[:, t, :],
                                     start=(t == 0), stop=(t == NTW - 1))
                stg = sp.tile([128, NF], F32, tag="ph_c_stg", name="ph_c_stg")
                nc.scalar.copy(stg[:], ps_agg[:])
                base = w * 128
                m = min(128, NS - base)
                nc.sync.dma_start(
                    agg_ab[h][base:base + m, :].rearrange(
                        "(t p) f -> p t f", p=m),
                    stg[:m, :].rearrange("p (t f) -> p t f", f=NF))

            for i in range(len(wstream) + PFD):
                if i < len(wstream):
                    gat_fifo.append(emit_gather(i))
                if i >= PFD:
                    done = i - PFD
                    emit_compute_flush(done, gat_fifo[done])
                    # GIN stage hooks: latency of the stats AllReduces and
                    # the serial BN chains hides under the window stream
                    if done == CW // 4:
                        emit_b2()
                    if done == CW + CW // 4:
                        emit_b3()
            emit_b2()  # no-op unless phase C was skipped
            emit_b3()
            ginT = bstate["ginT"]

            # =========== Phase D: h = relu(agg@lin2+b)@linw+b, residual =====
            NCH = D.nchunk
            n_nch = NS // NCH
            for j in range(0 if "no_d" in flags else n_nch):
                r0 = j * NCH
                # load agg rows, transpose to feature-major aggT [NF, NCH]
                aggT = wp.tile([NF, NCH], F32, tag="ph_d_aggT", name="ph_d_aggT")
                ntt = (NCH + 127) // 128
                for t in range(ntt):
                    m = min(128, NCH - t * 128)
                    asb = sp.tile([128, NF], F32, tag="ph_d_asb", name="ph_d_asb")
                    nc.sync.dma_start(asb[:m, :],
                                      agg_ab[0][r0 + t * 128:r0 + t * 128 + m, :])
                    bsb = sp.tile([128, NF], F32, tag="ph_d_bsb", name="ph_d_bsb")
                    nc.sync.dma_start(bsb[:m, :],
                                      agg_ab[1][r0 + t * 128:r0 + t * 128 + m, :])
                    nc.vector.tensor_tensor(asb[:m, :], asb[:m, :], bsb[:m, :],
                                            ALU.add)
                    pst = pp.tile([128, 128], F32, tag="ps_tr", name="ps_tr")
                    nc.tensor.transpose(pst[:, :m], asb[:m, :], C["ident"][:m, :m])
                    nc.vector.tensor_copy(aggT[:, t * 128:t * 128 + m],
                                          pst[:, :m])
                # h1T = relu(lin2^T @ aggT + b)  [2][128, NCH]
                h1T = [wp.tile([128, NCH], F32, tag=f"ph_d_h1T{k}", name=f"ph_d_h1T{k}")
                       for k in range(2)]
                for k in range(2):
                    ps = pp.tile([128, NCH], F32, tag="ps_mm", name="ps_mm")
                    nc.tensor.matmul(ps[:], C["lin2"][:, _ts(k, 128)], aggT[:],
                                     start=True, stop=True)
                    nc.scalar.activation(h1T[k][:], ps[:], ACTF.Relu,
                                         bias=C["lin2b"][k][:])
                # outT = linw^T @ h1T + linb + xT + gin[batch]
                for k in range(2):
                    ps = pp.tile([128, NCH], F32, tag="ps_mm", name="ps_mm")
                    for kk in range(2):
                        nc.tensor.matmul(ps[:], C["linw"][kk][:, _ts(k, 128)],
                                         h1T[kk][:], start=(kk == 0),
                                         stop=(kk == 1))
                    ob = sp.tile([128, NCH], F32, tag="ph_d_ob", name="ph_d_ob")
                    nc.scalar.activation(ob[:], ps[:], ACTF.Identity,
                                         bias=C["linb"][k][:])
                    xtc = sp.tile([128, NCH], F32, tag="ph_d_xtc", name="ph_d_xtc")
                    nc.sync.dma_start(xtc[:], I["xT"].ap()[k, :, r0:r0 + NCH])
                    nc.vector.tensor_tensor(ob[:], ob[:], xtc[:], ALU.add)
                    # + gin, each graph col repeated `rep` times
                    rep = D.N // D.G
                    g0 = r0 // rep
                    gin_rep = ginT[k][:, g0:g0 + NCH // rep].broadcast_to(
                        (128, NCH // rep, rep))
                    nc.vector.tensor_tensor(
                        ob[:].rearrange("p (g t) -> p g t", t=rep),
                        ob[:].rearrange("p (g t) -> p g t", t=rep),
                        gin_rep, ALU.add)
                    nc.sync.dma_start(outT.ap()[k, :, r0:r0 + NCH], ob[:])

    nc.compile()
    return nc


_CACHE = {}


def _get_nc(D: Dims, flags: frozenset = frozenset()):
    key = ("nc", D, flags)
    if key not in _CACHE:
        _CACHE[key] = build_nc(D, flags)
    return _CACHE[key]


def run_on_hw(inputs, D: Dims = REAL):
    flags = (frozenset({"b2zero"})
             if not np.any(np.asarray(inputs["mlp_b2"])) else frozenset())
    nc = _get_nc(D, flags)
    in_maps = host_prep(inputs, D)
    res = bass_utils.run_bass_kernel_spmd(nc, in_maps,
                                          core_ids=list(range(D.cores)))
    return assemble(res.results, D)


def kernel(**inputs):
    return run_on_hw(inputs, REAL)



# revision 43
# speedup vs baseline: 1.7061x; 1.5992x over previous
"""Trainium2 Bass kernel for nn_DSSConf (DSS conformer GNN message passing).

Self-contained: hardcodes shapes/sharding for the real problem; exposes
kernel(**inputs) -> np.ndarray.
"""
import sys
import math
from dataclasses import dataclass

sys.path.insert(0, "/opt/trn_rl_repo")

import numpy as np
from concourse import bass, bacc, tile, mybir, bass_utils

F32 = mybir.dt.float32
I16 = mybir.dt.int16
ALU = mybir.AluOpType
ACTF = mybir.ActivationFunctionType
AX = mybir.AxisListType


@dataclass(frozen=True)
class Dims:
    N: int = 100000        # conformer nodes
    H: int = 256           # hidden
    NF: int = 128          # num filters
    NG: int = 50           # num gaussians
    G: int = 10000         # graph nodes
    E: int = 1000000       # conformer edges
    EG: int = 30000        # graph edges
    VOCAB: int = 5
    CUTOFF: float = 10.0
    cores: int = 8
    qsize: int = 25000     # src quadrant size for int16 gather indices
    B_E: int = 384         # fixed edges per (dst-window, src-quad) bucket
    gwin: int = 128        # GIN scatter window (<=128 segments)
    PW: int = 640          # padded GIN edges per (core, window) (multiple of 128)
    nchunk: int = 500      # node chunk for the h/out stage (divides NS, mult of 10)

    @property
    def NS(self):
        return self.N // self.cores

    @property
    def GS(self):
        return self.G // self.cores

    @property
    def CW(self):
        """dst windows of 128 nodes per core shard."""
        return (self.NS + 127) // 128

    @property
    def EH(self):
        """padded edges per src-half stream (2 quad buckets per window)."""
        return self.CW * 2 * self.B_E

    @property
    def E_pad(self):
        return 2 * self.EH

    @property
    def NWIN(self):
        return (self.GS + self.gwin - 1) // self.gwin

    @property
    def EG_pad(self):
        return self.NWIN * self.PW


REAL = Dims()


def _wrap16(arr, dtype=np.int16):
    """Edge i -> [i % 16, i // 16], replicated to 128 partitions."""
    a = np.asarray(arr).reshape(-1, 16).T.astype(dtype)  # [16, n/16]
    return np.tile(a, (8, 1)).copy()  # [128, n/16]


def _tile128(arr, dtype=np.float32):
    """Edge i -> [i % 128, i // 128] (per-partition scalar layout)."""
    return np.ascontiguousarray(np.asarray(arr).reshape(-1, 128).T.astype(dtype))


def host_prep(inputs, D: Dims):
    """Build per-core in_maps (list of dicts) for the SPMD kernel."""
    x = np.asarray(inputs["x"], np.float32)
    cnb = np.asarray(inputs["conf_node_batch"]).astype(np.int64)
    ei = np.asarray(inputs["edge_index_conf"]).astype(np.int64)
    ew = np.asarray(inputs["edge_weight_conf"], np.float32)
    ea = np.asarray(inputs["edge_attr_conf"], np.float32)
    eig = np.asarray(inputs["edge_index_graph"]).astype(np.int64)
    eag = np.asarray(inputs["edge_attr_graph"]).astype(np.int64)

    rep = D.N // D.G
    assert np.array_equal(cnb, np.repeat(np.arange(D.G), rep)), \
        "conf_node_batch structure mismatch"

    NS, GS = D.NS, D.GS
    src, dst = ei[0], ei[1]

    # ---- conformer edges: window-major one-hot layout ----
    # per core: two streams by src half (matching the split AllGather's
    # permuted xf layout); within a stream, buckets of fixed B_E edges per
    # (dst-window of 128, src quad within the half); in-quad src offset =
    # (src_core % 4)*NS/2 + (src % NS/2)
    B_E, CW, EH = D.B_E, D.CW, D.EH
    core = dst // NS
    sc = src // NS
    so = src % NS
    sh = so // (NS // 2)
    sj = (sc >= 4).astype(np.int64)
    inq = (sc % 4) * (NS // 2) + (so % (NS // 2))
    win = (dst % NS) // 128
    bucket = ((core * 2 + sh) * CW + win) * 2 + sj
    order = np.argsort(bucket, kind="stable")
    sb = bucket[order]
    nbuck = D.cores * 2 * CW * 2
    bounds = np.searchsorted(sb, np.arange(nbuck + 1))
    counts = bounds[1:] - bounds[:-1]
    assert counts.max() <= B_E, f"bucket overflow: {counts.max()} > {B_E}"
    rank = np.arange(len(sb)) - bounds[sb]
    c_b = sb // (2 * CW * 2)
    rem = sb % (2 * CW * 2)
    h_b = rem // (CW * 2)
    w_b = (rem % (CW * 2)) // 2
    j_b = rem % 2
    flat = (c_b * D.E_pad + h_b * EH + (w_b * 2 + j_b) * B_E + rank)

    src_pad = np.zeros(D.cores * D.E_pad, np.int64)
    drel_pad = np.full(D.cores * D.E_pad, -1.0, np.float32)
    w_pad = np.full(D.cores * D.E_pad, D.CUTOFF, np.float32)  # C(CUTOFF)=0
    a_pad = np.zeros((D.cores * D.E_pad, D.NG), np.float32)
    src_pad[flat] = inq[order]
    drel_pad[flat] = ((dst[order] % NS) - w_b * 128).astype(np.float32)
    w_pad[flat] = ew[order]
    a_pad[flat] = ea[order]

    AT = np.zeros((D.cores, D.NG, D.E_pad), np.float32)
    WT = np.zeros((D.cores, 128, D.E_pad // 128), np.float32)
    DRELC = np.zeros((D.cores, 128, D.E_pad // 128), np.float32)
    SRC = np.zeros((D.cores, 128, D.E_pad // 16), np.int16)
    for c in range(D.cores):
        sl = slice(c * D.E_pad, (c + 1) * D.E_pad)
        AT[c] = a_pad[sl].T
        WT[c] = _tile128(w_pad[sl])
        DRELC[c] = _tile128(drel_pad[sl])
        SRC[c] = _wrap16(src_pad[sl])

    # ---- graph edges: order by (core(dst), window(dst), dst) ----
    sg, dg = eig[0], eig[1]
    gcore = dg // GS
    gwin = (dg - gcore * GS) // D.gwin
    gorder = np.lexsort((dg, gwin, gcore))
    g_s, g_d, g_w, g_c = sg[gorder], dg[gorder], gwin[gorder], gcore[gorder]
    g_a = eag[gorder]

    SG = np.zeros((D.cores, 128, D.EG_pad // 16), np.int16)
    DREL = np.zeros((D.cores, 128, D.EG_pad // 128), np.float32)
    BHOT = np.zeros((D.cores, 3 * D.VOCAB, D.EG_pad), np.float32)

    gkeys = g_c * D.NWIN + g_w
    gbounds = np.searchsorted(gkeys, np.arange(D.cores * D.NWIN + 1))
    for c in range(D.cores):
        sg_pad = np.zeros(D.EG_pad, np.int64)
        dr_pad = np.full(D.EG_pad, -1.0, np.float32)  # -1 kills pads in one-hot
        bh_pad = np.zeros((3 * D.VOCAB, D.EG_pad), np.float32)
        for w in range(D.NWIN):
            lo, hi = gbounds[c * D.NWIN + w], gbounds[c * D.NWIN + w + 1]
            cnt = hi - lo
            assert cnt <= D.PW, f"PW overflow: core {c} win {w}: {cnt}"
            o = w * D.PW
            sg_pad[o:o + cnt] = g_s[lo:hi]
            dr_pad[o:o + cnt] = (g_d[lo:hi] - c * GS - w * D.gwin).astype(np.float32)
            for k in range(3):
                bh_pad[k * D.VOCAB + g_a[lo:hi, k], np.arange(o, o + cnt)] = 1.0
        SG[c] = _wrap16(sg_pad)
        DREL[c] = _tile128(dr_pad)
        BHOT[c] = bh_pad

    # ---- x^T shards ----
    xT = np.ascontiguousarray(x.T)  # [H, N]
    XT = xT.reshape(2, 128, D.N)

    # ---- weights (replicated) ----
    H2 = D.H // 128
    w = {k: np.asarray(inputs[k], np.float32) for k in (
        "mlp_w1", "mlp_b1", "mlp_w2", "mlp_b2", "cf_lin1", "cf_lin2",
        "cf_lin2_b", "lin_w", "lin_b", "bond_emb", "gin_eps", "gin_w1",
        "gin_w2", "bn1_g", "bn1_b", "bn2_g", "bn2_b")}
    const = dict(
        w1=w["mlp_w1"],                                   # [NG, NF]
        b1col=w["mlp_b1"].reshape(D.NF, 1),
        w2=w["mlp_w2"],                                   # [NF, NF]
        b2row=w["mlp_b2"].reshape(1, D.NF),
        b2row3=np.tile(w["mlp_b2"].reshape(1, D.NF), (1, 3)),
        ones1=np.ones((1, 128), np.float32),
        lin1=np.ascontiguousarray(w["cf_lin1"].reshape(H2, 128, D.NF)),
        lin2=w["cf_lin2"],                                # [NF, H]
        lin2b=w["cf_lin2_b"].reshape(H2, 128, 1),
        linw=np.ascontiguousarray(w["lin_w"].reshape(H2, 128, D.H)),
        linb=w["lin_b"].reshape(H2, 128, 1),
        gw1=np.ascontiguousarray(w["gin_w1"].reshape(H2, 128, D.H)),
        gw2=np.ascontiguousarray(w["gin_w2"].reshape(H2, 128, D.H)),
        bondcat=np.ascontiguousarray(
            w["bond_emb"].reshape(3 * D.VOCAB, D.H)),
        bn1g=w["bn1_g"].reshape(H2, 128, 1), bn1b=w["bn1_b"].reshape(H2, 128, 1),
        bn2g=w["bn2_g"].reshape(H2, 128, 1), bn2b=w["bn2_b"].reshape(H2, 128, 1),
        epsv=np.full((128, 1), 1.0 + float(w["gin_eps"]), np.float32),
        zerocol=np.zeros((128, 1), np.float32),
        eps5col=np.full((128, 1), 1e-5, np.float32),
        pihalf=np.full((128, 1), -math.pi / 2, np.float32),
        iota=np.tile(np.arange(128, dtype=np.float32), (128, 1)).copy(),
        iota6=np.tile(np.arange(128, dtype=np.float32), (128, 6)).copy(),
        ident=np.eye(128, dtype=np.float32),
    )

    in_maps = []
    for c in range(D.cores):
        m = dict(
            xT=np.ascontiguousarray(XT[:, :, c * NS:(c + 1) * NS]),
            AT=AT[c], WT=WT[c], SRC=SRC[c], DRELC=DRELC[c],
            SG=SG[c], DREL=DREL[c], BHOT=BHOT[c],
        )
        m.update(const)
        in_maps.append(m)
    return in_maps


def assemble(results, D: Dims):
    """Per-core outT [2,128,NS] -> full [N, H]."""
    parts = [r["outT"].reshape(D.H, D.NS) for r in results]
    outT = np.concatenate(parts, axis=1)  # [H, N]
    return np.ascontiguousarray(outT.T)


def _ts(i, n):
    return bass.ts(i, n)


def build_nc(D: Dims, flags: frozenset = frozenset()):
    nc = bacc.Bacc("TRN2", target_bir_lowering=False, debug=False,
                   num_devices=D.cores, num_swdge_queues=3)
    NS, GS, H, NF, NG = D.NS, D.GS, D.H, D.NF, D.NG
    H2 = H // 128

    I = {}
    def di(name, shape, dt=F32):
        I[name] = nc.dram_tensor(name, list(shape), dt, kind="ExternalInput")
        return I[name]

    di("xT", [2, 128, NS])
    di("AT", [NG, D.E_pad])
    di("WT", [128, D.E_pad // 128])
    di("SRC", [128, D.E_pad // 16], I16)
    di("DRELC", [128, D.E_pad // 128])
    di("SG", [128, D.EG_pad // 16], I16)
    di("DREL", [128, D.EG_pad // 128])
    di("BHOT", [3 * D.VOCAB, D.EG_pad])
    di("w1", [NG, NF]); di("b1col", [NF, 1]); di("w2", [NF, NF])
    di("b2row", [1, NF]); di("b2row3", [1, 3 * NF]); di("ones1", [1, 128])
    di("lin1", [H2, 128, NF]); di("lin2", [NF, H]); di("lin2b", [H2, 128, 1])
    di("linw", [H2, 128, H]); di("linb", [H2, 128, 1])
    di("gw1", [H2, 128, H]); di("gw2", [H2, 128, H])
    di("bondcat", [3 * D.VOCAB, H])
    di("bn1g", [H2, 128, 1]); di("bn1b", [H2, 128, 1])
    di("bn2g", [H2, 128, 1]); di("bn2b", [H2, 128, 1])
    di("epsv", [128, 1]); di("iota", [128, 128]); di("iota6", [128, 768])
    di("ident", [128, 128])
    di("zerocol", [128, 1]); di("eps5col", [128, 1]); di("pihalf", [128, 1])

    outT = nc.dram_tensor("outT", [2, 128, NS], F32, kind="ExternalOutput")

    groups = [list(range(D.cores))]

    with tile.TileContext(nc) as tc:
        with (
            tc.tile_pool(name="const", bufs=1) as cp,
            tc.tile_pool(name="work", bufs=2) as wp,
            tc.tile_pool(name="small", bufs=3) as sp,
            tc.tile_pool(name="gin", bufs=2) as ctx_gin_pool,
            tc.tile_pool(name="psum", bufs=2, space="PSUM") as pp,
            tc.tile_pool(name="dram", bufs=1, space="DRAM") as dp,
        ):
            # ---------- load constants ----------
            C = {}
            for nm, shp in [("w1", [NG, NF]), ("b1col", [NF, 1]),
                            ("w2", [NF, NF]), ("b2row", [1, NF]),
                            ("b2row3", [1, 3 * NF]),
                            ("ones1", [1, 128]), ("lin2", [NF, H]),
                            ("bondcat", [3 * D.VOCAB, H]),
                            ("epsv", [128, 1]), ("iota", [128, 128]),
                            ("iota6", [128, 768]),
                            ("ident", [128, 128]), ("zerocol", [128, 1]),
                            ("eps5col", [128, 1]), ("pihalf", [128, 1])]:
                t = cp.tile(shp, F32, name=f"c_{nm}")
                nc.sync.dma_start(t[:], I[nm].ap())
                C[nm] = t
            nc.const_aps.aps[(F32, 0.0)] = C["zerocol"][:]
            # [H2,128,*] constants: load as per-half tiles
            for nm in ("lin1", "lin2b", "linw", "linb", "gw1", "gw2",
                       "bn1g", "bn1b", "bn2g", "bn2b"):
                C[nm] = []
                inner = I[nm].shape[2]
                for k in range(H2):
                    t = cp.tile([128, inner], F32, name=f"c_{nm}{k}")
                    nc.sync.dma_start(t[:], I[nm].ap()[k])
                    C[nm].append(t)

            # ---------- DRAM scratch ----------
            # xf shard/full split in half so the AllGather pipelines with
            # compute: AG#h gathers every core's half-h shard; the gathered
            # layout is permuted (half-major), host_prep permutes src indices
            HSH = NS // 2
            xf_shard = [dp.tile([HSH, NF], F32, name=f"xf_shard{h}")
                        for h in range(2)]
            xf_full = [dp.tile([D.cores * HSH, NF], F32, name=f"xf_full{h}",
                               addr_space="Shared") for h in range(2)]
            xagg_shard = dp.tile([GS, H], F32, name="xagg_shard")
            xagg_full = dp.tile([D.cores * GS, H], F32, name="xagg_full",
                                addr_space="Shared")
            agg_ab = [dp.tile([NS, NF], F32, name=f"agg_{h}")
                      for h in range(2)]
            st1_in = dp.tile([128, 4], F32, name="st1_in")
            st1_out = dp.tile([128, 4], F32, name="st1_out", addr_space="Shared")
            st2_in = dp.tile([128, 4], F32, name="st2_in")
            st2_out = dp.tile([128, 4], F32, name="st2_out", addr_space="Shared")

            # =========== Phase A: segment-max pool first, then xf ===========
            # pass 1 pools so the (small) xagg AllGather is issued as early
            # as possible; pass 2 re-reads x and computes xf, issuing each
            # half's AllGather as soon as that half of the shard is written
            rep = D.N // D.G
            PCH = 250
            n_pch = NS // PCH
            half_chunks = n_pch // 2
            xaggT = [cp.tile([128, GS], F32, name=f"xaggT{k}") for k in range(2)]
            for j in range(n_pch):
                xt = [wp.tile([128, PCH], F32, tag=f"ph_a_xt{k}", name=f"ph_a_xt{k}")
                      for k in range(2)]
                for k in range(2):
                    nc.sync.dma_start(xt[k][:], I["xT"].ap()[k, :, _ts(j, PCH)])
                # pool: max over groups of 10 cols
                for k in range(2):
                    nc.vector.tensor_reduce(
                        xaggT[k][:, _ts(j, PCH // rep)],
                        xt[k][:].rearrange("p (g t) -> p g t", t=rep),
                        AX.X, ALU.max)

            # transpose x_aggT -> node-major x_agg shard
            GT = (GS + 127) // 128
            for t in range(GT):
                m = min(128, GS - t * 128)
                for k in range(2):
                    pst = pp.tile([128, 128], F32, tag="ps_tr", name="ps_tr")
                    nc.tensor.transpose(pst[:m, :], xaggT[k][:, t * 128:t * 128 + m],
                                        C["ident"][:])
                    sb = sp.tile([128, 128], F32, tag="ph_a_trsb", name="ph_a_trsb")
                    nc.scalar.copy(sb[:m, :], pst[:m, :])
                    nc.sync.dma_start(
                        xagg_shard[t * 128:t * 128 + m, _ts(k, 128)], sb[:m, :])

            if "no_coll" not in flags and "no_coll_xagg" not in flags:
                nc.gpsimd.collective_compute(
                    "AllGather", ALU.bypass, replica_groups=groups,
                    ins=[xagg_shard.opt()], outs=[xagg_full.opt()])

            # pass 2: xf = x @ cf_lin1, node-major tiles of <=128
            for j in range(n_pch):
                xt = [wp.tile([128, PCH], F32, tag=f"ph_a_xt{k}", name=f"ph_a2_xt{k}")
                      for k in range(2)]
                for k in range(2):
                    nc.sync.dma_start(xt[k][:], I["xT"].ap()[k, :, _ts(j, PCH)])
                h = j // half_chunks
                r0 = (j - h * half_chunks) * PCH
                nt = (PCH + 127) // 128
                for t in range(nt):
                    m = min(128, PCH - t * 128)
                    ps = pp.tile([128, NF], F32, tag="ps_mm", name="ps_mm")
                    for k in range(2):
                        nc.tensor.matmul(ps[:m, :], xt[k][:, t * 128:t * 128 + m],
                                         C["lin1"][k][:], start=(k == 0),
                                         stop=(k == 1))
                    sb = sp.tile([128, NF], F32, tag="ph_a_sb", name="ph_a_sb")
                    nc.scalar.copy(sb[:m, :], ps[:m, :])
                    nc.sync.dma_start(
                        xf_shard[h][r0 + t * 128: r0 + t * 128 + m, :],
                        sb[:m, :])
                if (j == half_chunks - 1 and "no_coll" not in flags
                        and "no_coll_xf" not in flags):
                    nc.gpsimd.collective_compute(
                        "AllGather", ALU.bypass, replica_groups=groups,
                        ins=[xf_shard[0].opt()], outs=[xf_full[0].opt()])
            # AG#1 is issued later (after st1's AllReduce) so the GIN stats
            # reduction isn't queued behind it on the collective engine

            # =========== Phase B: GIN branch (sharded by graph node) =========
            # gather x_agg[sg], edge_emb via bond one-hot matmul, relu,
            # one-hot scatter into agg_g windows
            sgidx = cp.tile([128, D.EG_pad // 16], I16, name="sgidx_sb")
            nc.sync.dma_start(sgidx[:], I["SG"].ap())
            drel = cp.tile([128, D.EG_pad // 128], F32, name="drel_sb")
            nc.sync.dma_start(drel[:], I["DREL"].ap())

            # t-buffer (node-major (1+eps)x_agg + agg_g), then transposed halves
            gp = ctx_gin_pool
            tT = [gp.tile([128, GS], F32, tag=f"ginbuf{k}", name=f"tT{k}")
                  for k in range(2)]

            assert D.EG_pad % 128 == 0
            tiles_per_win = D.PW // 128
            for w in range(0 if "no_b" in flags else D.NWIN):
                m = min(D.gwin, GS - w * D.gwin)
                # gather this window's source rows
                gath_g = wp.tile([128, tiles_per_win, H], F32,
                                 tag="ph_b_gath", name="ph_b_gath")
                nc.gpsimd.dma_gather(
                    gath_g[:], xagg_full[:],
                    sgidx[:, w * D.PW // 16:(w + 1) * D.PW // 16],
                    num_idxs=D.PW, num_idxs_reg=D.PW, elem_size=H)
                bhot = wp.tile([3 * D.VOCAB, D.PW], F32, tag="ph_b_bhot",
                               name="ph_b_bhot")
                nc.sync.dma_start(bhot[:],
                                  I["BHOT"].ap()[:, w * D.PW:(w + 1) * D.PW])
                ps_agg = pp.tile([128, H], F32, tag="ps_agg", name="ps_agg")
                for i in range(tiles_per_win):
                    t = w * tiles_per_win + i
                    # edge embedding: one-hot bond matmul (K=15)
                    ps_emb = pp.tile([128, H], F32, tag="ps_mm", name="ps_mm")
                    nc.tensor.matmul(ps_emb[:], bhot[:, _ts(i, 128)],
                                     C["bondcat"][:], start=True, stop=True)
                    # msg = relu(gathered + emb)
                    msg = sp.tile([128, H], F32, tag="ph_b_msg", name="ph_b_msg")
                    nc.vector.tensor_tensor(msg[:], gath_g[:, i, :], ps_emb[:],
                                            ALU.add)
                    nc.scalar.activation(msg[:], msg[:], ACTF.Relu)
                    # one-hot scatter
                    oh = sp.tile([128, D.gwin], F32, tag="ph_b_oh", name="ph_b_oh")
                    nc.vector.tensor_scalar(oh[:], C["iota"][:, :D.gwin],
                                            drel[:, t:t + 1], None, ALU.is_equal)
                    nc.tensor.matmul(ps_agg[:m, :], oh[:, :m], msg[:],
                                     start=(i == 0), stop=(i == tiles_per_win - 1))
                # t = (1+eps) * x_agg + agg_g  (node-major window rows)
                xa = sp.tile([128, H], F32, tag="ph_b_xa", name="ph_b_xa")
                nc.sync.dma_start(
                    xa[:m, :], xagg_shard[w * D.gwin:w * D.gwin + m, :])
                tn = sp.tile([128, H], F32, tag="ph_b_tn", name="ph_b_tn")
                nc.vector.tensor_scalar(tn[:m, :], xa[:m, :], C["epsv"][:m, :],
                                        None, ALU.mult)
                nc.vector.tensor_tensor(tn[:m, :], tn[:m, :], ps_agg[:m, :],
                                        ALU.add)
                # transpose to feature-major tT
                for k in range(2):
                    pst = pp.tile([128, 128], F32, tag="ps_tr", name="ps_tr")
                    nc.tensor.transpose(pst[:, :m], tn[:m, _ts(k, 128)],
                                        C["ident"][:m, :m])
                    nc.vector.tensor_copy(tT[k][:, w * D.gwin:w * D.gwin + m],
                                          pst[:, :m])

            def gin_mm_and_stats_issue(inT, Wc, uT, stats_in, stats_out, label):
                """u = in @ W (node-major tiles), transpose to uT, stats;
                issues the stats AllReduce but does NOT read the result."""
                for t in range(GT):
                    m = min(128, GS - t * 128)
                    ps = pp.tile([128, H], F32, tag="ps_mm", name="ps_mm")
                    for k in range(2):
                        nc.tensor.matmul(ps[:m, :],
                                         inT[k][:, t * 128:t * 128 + m],
                                         Wc[k][:], start=(k == 0), stop=(k == 1))
                    sb = sp.tile([128, H], F32, tag=f"{label}_sb", name=f"{label}_sb")
                    nc.scalar.copy(sb[:m, :], ps[:m, :])
                    for k in range(2):
                        pst = pp.tile([128, 128], F32, tag="ps_tr", name="ps_tr")
                        nc.tensor.transpose(pst[:, :m], sb[:m, _ts(k, 128)],
                                            C["ident"][:m, :m])
                        nc.vector.tensor_copy(uT[k][:, t * 128:t * 128 + m],
                                              pst[:, :m])
                st = sp.tile([128, 4], F32, tag=f"{label}_st", name=f"{label}_st")
                sq = sp.tile([128, GS], F32, tag="gin_sq", name="gin_sq",
                             bufs=1)
                for k in range(2):
                    nc.vector.tensor_reduce(st[:, 2 * k:2 * k + 1], uT[k][:],
                                            AX.X, ALU.add)
                    nc.vector.tensor_tensor(sq[:], uT[k][:], uT[k][:], ALU.mult)
                    nc.vector.tensor_reduce(st[:, 2 * k + 1:2 * k + 2], sq[:],
                                            AX.X, ALU.add)
                nc.sync.dma_start(stats_in[:], st[:])
                if "no_b_ar" not in flags:
                    nc.gpsimd.collective_compute(
                        "AllReduce", ALU.add, replica_groups=groups,
                        ins=[stats_in.opt()], outs=[stats_out.opt()])

            def gin_read_stf(stats_in, stats_out, label):
                stf = sp.tile([128, 4], F32, tag=f"{label}_stf", name=f"{label}_stf")
                nc.sync.dma_start(
                    stf[:], stats_in[:] if "no_b_ar" in flags else stats_out[:])
                return stf

            def bn_apply(stf, uT, g_c, b_c, outT_t, relu, label):
                """out = func((u - mu) * g / sqrt(var+eps) + b), feature-major."""
                inv_n = 1.0 / float(D.G)
                for k in range(2):
                    mu = sp.tile([128, 1], F32, tag=f"{label}_mu{k}", name=f"{label}_mu{k}")
                    nc.vector.tensor_scalar(mu[:], stf[:, 2 * k:2 * k + 1],
                                            inv_n, None, ALU.mult)
                    var = sp.tile([128, 1], F32, tag=f"{label}_va{k}", name=f"{label}_va{k}")
                    nc.vector.tensor_scalar(var[:], stf[:, 2 * k + 1:2 * k + 2],
                                            inv_n, None, ALU.mult)
                    mu2 = sp.tile([128, 1], F32, tag=f"{label}_m2{k}", name=f"{label}_m2{k}")
                    nc.vector.tensor_tensor(mu2[:], mu[:], mu[:], ALU.mult)
                    nc.vector.tensor_tensor(var[:], var[:], mu2[:], ALU.subtract)
                    sd = sp.tile([128, 1], F32, tag=f"{label}_sd{k}", name=f"{label}_sd{k}")
                    nc.scalar.activation(sd[:], var[:], ACTF.Sqrt,
                                         bias=C["eps5col"][:])
                    rs = sp.tile([128, 1], F32, tag=f"{label}_rs{k}", name=f"{label}_rs{k}")
                    nc.vector.reciprocal(rs[:], sd[:])
                    sc = sp.tile([128, 1], F32, tag=f"{label}_sc{k}", name=f"{label}_sc{k}")
                    nc.vector.tensor_tensor(sc[:], g_c[k][:], rs[:], ALU.mult)
                    sh = sp.tile([128, 1], F32, tag=f"{label}_sh{k}", name=f"{label}_sh{k}")
                    nc.vector.tensor_tensor(sh[:], mu[:], sc[:], ALU.mult)
                    nc.vector.tensor_tensor(sh[:], b_c[k][:], sh[:], ALU.subtract)
                    nc.scalar.activation(outT_t[k][:], uT[k][:],
                                         ACTF.Relu if relu else ACTF.Identity,
                                         bias=sh[:], scale=sc[:])

            def gin_buf(nm):
                return [gp.tile([128, GS], F32, tag=f"ginbuf{k}",
                                name=f"{nm}{k}") for k in range(2)]

            bstate = {}
            if "no_b" in flags:
                ginT = gin_buf("ginT")
                for k in range(2):
                    nc.vector.memset(ginT[k][:], 0.0)
                bstate["ginT"] = ginT
            else:
                uT = gin_buf("uT")
                gin_mm_and_stats_issue(tT, C["gw1"], uT, st1_in, st1_out,
                                       "gmm1")

            # xf AG#1 queued on the collective engine AFTER st1's AllReduce
            # so the (tiny) stats reduce isn't stuck behind the 25MB gather
            if "no_coll" not in flags and "no_coll_xf" not in flags:
                nc.gpsimd.collective_compute(
                    "AllGather", ALU.bypass, replica_groups=groups,
                    ins=[xf_shard[1].opt()], outs=[xf_full[1].opt()])

            def emit_b2():
                """bn1 + gmm2 + st2 AllReduce issue (mid phase C)."""
                if "no_b" in flags or "t2T" in bstate:
                    return
                stf1 = gin_read_stf(st1_in, st1_out, "gmm1")
                t1T = gin_buf("t1T")
                bn_apply(stf1, uT, C["bn1g"], C["bn1b"], t1T, True, "bn1")
                t2T = gin_buf("t2T")
                gin_mm_and_stats_issue(t1T, C["gw2"], t2T, st2_in, st2_out,
                                       "gmm2")
                bstate["t2T"] = t2T

            def emit_b3():
                """bn2 -> ginT (late in phase C)."""
                if "no_b" in flags or "ginT" in bstate:
                    return
                stf2 = gin_read_stf(st2_in, st2_out, "gmm2")
                ginT = gin_buf("ginT")
                bn_apply(stf2, bstate["t2T"], C["bn2g"], C["bn2b"], ginT,
                         False, "bn2")
                bstate["ginT"] = ginT

            # =========== Phase C: conformer edge pipeline ===========
            # window-major: per (src-half stream h, dst-window w of 128
            # nodes): gather the window's edges (one call per src quad
            # bucket of B_E), compute msg = (xf[src]) * (mlp(A) + b2), then
            # aggregate over dst via one-hot matmuls into PSUM (the one-hot
            # rows carry the cosine-cutoff C so no separate C-multiply),
            # and flush the window's 128 agg rows with a plain DMA write.
            # No scatter-add: each agg row is written exactly once.
            # resident: C row (cosine cutoff) and dst-rel row per edge
            crow = cp.tile([128, D.E_pad // 128], F32, name="crow_sb")
            for s0 in range(0, D.E_pad // 128, 512):
                sw = min(512, D.E_pad // 128 - s0)
                wt = wp.tile([128, 512], F32, tag="ph_c_wt", name="ph_c_wt")
                nc.sync.dma_start(wt[:, :sw], I["WT"].ap()[:, s0:s0 + sw])
                nc.scalar.activation(wt[:, :sw], wt[:, :sw], ACTF.Sin,
                                     bias=C["pihalf"][:],
                                     scale=math.pi / D.CUTOFF)
                nc.scalar.activation(crow[:, s0:s0 + sw], wt[:, :sw],
                                     ACTF.Copy, bias=0.5, scale=-0.5)
            drelc = cp.tile([128, D.E_pad // 128], F32, name="drelc_sb")
            nc.sync.dma_start(drelc[:], I["DRELC"].ap())

            B_E, CW, EH = D.B_E, D.CW, D.EH
            WE = 2 * B_E           # edges per (stream, window)
            NTW = WE // 128        # tiles per window (6)
            wstream = ([] if "no_c" in flags else
                       [(h, w) for h in range(2) for w in range(CW)])
            PFD = 6  # gather prefetch depth (windows issued ahead)
            gat_fifo = []
            F32R = mybir.dt.float32r

            def emit_gather(idx):
                h, w = wstream[idx]
                e0 = h * EH + w * WE
                if "no_gather" in flags:
                    if "no_cmm" in flags:
                        return None
                    gat = wp.tile([128, NTW, NF], F32, tag="ph_c_gat",
                                  name="ph_c_gat", bufs=PFD + 2)
                    nc.vector.memset(gat[:], 0.0)
                    return gat
                gat = wp.tile([128, NTW, NF], F32, tag="ph_c_gat",
                              name="ph_c_gat", bufs=PFD + 2)
                si = wp.tile([128, WE // 16], I16, tag="ph_c_si",
                             name="ph_c_si", bufs=PFD + 2)
                nc.sync.dma_start(
                    si[:], I["SRC"].ap()[:, e0 // 16:(e0 + WE) // 16])
                for j in range(2):
                    nc.gpsimd.dma_gather(
                        gat[:, j * (NTW // 2):(j + 1) * (NTW // 2), :],
                        xf_full[h][j * D.qsize:(j + 1) * D.qsize, :],
                        si[:, j * B_E // 16:(j + 1) * B_E // 16],
                        num_idxs=B_E, num_idxs_reg=B_E, elem_size=NF,
                        queue_num=idx % 2)
                return gat

            def emit_compute_flush(idx, gat):
                h, w = wstream[idx]
                e0 = h * EH + w * WE
                c0col = e0 // 128
                msg = None
                if "no_cmm" not in flags:
                    msg = wp.tile([128, NTW, NF], F32, tag="ph_c_msg",
                                  name="ph_c_msg")
                    at = wp.tile([NG, WE], F32, tag="ph_c_at",
                                 name="ph_c_at")
                    nc.sync.dma_start(at[:], I["AT"].ap()[:, e0:e0 + WE])
                    # per 384-edge group: mm1+relu (fp32r), mm2 + b2 packed
                    # 3 tiles into one PSUM bank; 384-wide msg-mul
                    for g in range(2):
                        s0 = g * 384
                        ps1 = pp.tile([128, 384], F32, tag="ps_mm", name="ps_mm")
                        nc.tensor.matmul(ps1[:], C["w1"][:],
                                         at[:, s0:s0 + 384],
                                         start=True, stop=True)
                        h1 = wp.tile([128, 384], F32, tag="ph_c_h1",
                                     name="ph_c_h1")
                        nc.scalar.activation(h1[:], ps1[:],
                                             ACTF.Relu, bias=C["b1col"][:])
                        psw = pp.tile([128, 3, NF], F32, tag="ps_w", name="ps_w")
                        for t3 in range(3):
                            nc.tensor.matmul(psw[:, t3, :], h1[:, _ts(t3, 128)],
                                             C["w2"][:], start=True,
                                             stop="b2zero" in flags)
                            if "b2zero" not in flags:
                                nc.tensor.matmul(psw[:, t3, :], C["ones1"][:],
                                                 C["b2row"][:], start=False,
                                                 stop=True)
                        nc.vector.tensor_tensor(msg[:, 3 * g:3 * g + 3, :],
                                                gat[:, 3 * g:3 * g + 3, :],
                                                psw[:], ALU.mult)
                if "no_scatter" in flags or "no_cmm" in flags:
                    return
                # one-hot aggregation: rows carry C; accumulate over tiles
                ps_agg = pp.tile([128, NF], F32, tag="ps_agg", name="ps_cagg")
                ohc = sp.tile([128, NTW, 128], F32, tag="ph_c_oh",
                              name="ph_c_oh")
                nc.vector.tensor_tensor(
                    ohc[:],
                    C["iota6"][:].rearrange("p (t j) -> p t j", j=128),
                    drelc[:, c0col:c0col + NTW].broadcast_to((128, NTW, 128)),
                    ALU.is_equal)
                nc.vector.tensor_tensor(
                    ohc[:], ohc[:],
                    crow[:, c0col:c0col + NTW].broadcast_to((128, NTW, 128)),
                    ALU.mult)
                for t in range(NTW):
                    nc.tensor.matmul(ps_agg[:], ohc[:, t, :], msg[:, t, :],
                                     start=(t == 0), stop=(t == NTW - 1))
                stg = sp.tile([128, NF], F32, tag="ph_c_stg", name="ph_c_stg")
                nc.scalar.copy(stg[:], ps_agg[:])
                base = w * 128
                m = min(128, NS - base)
                nc.sync.dma_start(
                    agg_ab[h][base:base + m, :].rearrange(
                        "(t p) f -> p t f", p=m),
                    stg[:m, :].rearrange("p (t f) -> p t f", f=NF))

            for i in range(len(wstream) + PFD):
                if i < len(wstream):
                    gat_fifo.append(emit_gather(i))
                if i >= PFD:
                    done = i - PFD
                    emit_compute_flush(done, gat_fifo[done])
                    # GIN stage hooks: latency of the stats AllReduces and
                    # the serial BN chains hides under the window stream
                    if done == CW // 4:
                        emit_b2()
                    if done == CW + CW // 4:
                        emit_b3()
            emit_b2()  # no-op unless phase C was skipped
            emit_b3()
            ginT = bstate["ginT"]

            # =========== Phase D: h = relu(agg@lin2+b)@linw+b, residual =====
            NCH = D.nchunk
            n_nch = NS // NCH
            for j in range(0 if "no_d" in flags else n_nch):
                r0 = j * NCH
                # load agg rows, transpose to feature-major aggT [NF, NCH]
                aggT = wp.tile([NF, NCH], F32, tag="ph_d_aggT", name="ph_d_aggT")
                ntt = (NCH + 127) // 128
                for t in range(ntt):
                    m = min(128, NCH - t * 128)
                    asb = sp.tile([128, NF], F32, tag="ph_d_asb", name="ph_d_asb")
                    nc.sync.dma_start(asb[:m, :],
                                      agg_ab[0][r0 + t * 128:r0 + t * 128 + m, :])
                    bsb = sp.tile([128, NF], F32, tag="ph_d_bsb", name="ph_d_bsb")
                    nc.sync.dma_start(bsb[:m, :],
                                      agg_ab[1][r0 + t * 128:r0 + t * 128 + m, :])
                    nc.vector.tensor_tensor(asb[:m, :], asb[:m, :], bsb[:m, :],
                                            ALU.add)
                    pst = pp.tile([128, 128], F32, tag="ps_tr", name="ps_tr")
                    nc.tensor.transpose(pst[:, :m], asb[:m, :], C["ident"][:m, :m])
                    nc.vector.tensor_copy(aggT[:, t * 128:t * 128 + m],
                                          pst[:, :m])
                # h1T = relu(lin2^T @ aggT + b)  [2][128, NCH]
                h1T = [wp.tile([128, NCH], F32, tag=f"ph_d_h1T{k}", name=f"ph_d_h1T{k}")
                       for k in range(2)]
                for k in range(2):
                    ps = pp.tile([128, NCH], F32, tag="ps_mm", name="ps_mm")
                    nc.tensor.matmul(ps[:], C["lin2"][:, _ts(k, 128)], aggT[:],
                                     start=True, stop=True)
                    nc.scalar.activation(h1T[k][:], ps[:], ACTF.Relu,
                                         bias=C["lin2b"][k][:])
                # outT = linw^T @ h1T + linb + xT + gin[batch]
                for k in range(2):
                    ps = pp.tile([128, NCH], F32, tag="ps_mm", name="ps_mm")
                    for kk in range(2):
                        nc.tensor.matmul(ps[:], C["linw"][kk][:, _ts(k, 128)],
                                         h1T[kk][:], start=(kk == 0),
                                         stop=(kk == 1))
                    ob = sp.tile([128, NCH], F32, tag="ph_d_ob", name="ph_d_ob")
                    nc.scalar.activation(ob[:], ps[:], ACTF.Identity,
                                         bias=C["linb"][k][:])
                    xtc = sp.tile([128, NCH], F32, tag="ph_d_xtc", name="ph_d_xtc")
                    nc.sync.dma_start(xtc[:], I["xT"].ap()[k, :, r0:r0 + NCH])
                    nc.vector.tensor_tensor(ob[:], ob[:], xtc[:], ALU.add)
                    # + gin, each graph col repeated `rep` times
                    rep = D.N // D.G
                    g0 = r0 // rep
                    gin_rep = ginT[k][:, g0:g0 + NCH // rep].broadcast_to(
                        (128, NCH // rep, rep))
                    nc.vector.tensor_tensor(
                        ob[:].rearrange("p (g t) -> p g t", t=rep),
                        ob[:].rearrange("p (g t) -> p g t", t=rep),
                        gin_rep, ALU.add)
                    nc.sync.dma_start(outT.ap()[k, :, r0:r0 + NCH], ob[:])

    nc.compile()
    return nc


_CACHE = {}


def _get_nc(D: Dims, flags: frozenset = frozenset()):
    key = ("nc", D, flags)
    if key not in _CACHE:
        _CACHE[key] = build_nc(D, flags)
    return _CACHE[key]


def run_on_hw(inputs, D: Dims = REAL):
    flags = (frozenset({"b2zero"})
             if not np.any(np.asarray(inputs["mlp_b2"])) else frozenset())
    nc = _get_nc(D, flags)
    in_maps = host_prep(inputs, D)
    res = bass_utils.run_bass_kernel_spmd(nc, in_maps,
                                          core_ids=list(range(D.cores)))
    return assemble(res.results, D)


def kernel(**inputs):
    return run_on_hw(inputs, REAL)



# revision 46
# speedup vs baseline: 1.7899x; 1.0491x over previous
"""Trainium2 Bass kernel for nn_DSSConf (DSS conformer GNN message passing).

Self-contained: hardcodes shapes/sharding for the real problem; exposes
kernel(**inputs) -> np.ndarray.
"""
import sys
import math
from dataclasses import dataclass

sys.path.insert(0, "/opt/trn_rl_repo")

import numpy as np
from concourse import bass, bacc, tile, mybir, bass_utils

F32 = mybir.dt.float32
I16 = mybir.dt.int16
ALU = mybir.AluOpType
ACTF = mybir.ActivationFunctionType
AX = mybir.AxisListType


@dataclass(frozen=True)
class Dims:
    N: int = 100000        # conformer nodes
    H: int = 256           # hidden
    NF: int = 128          # num filters
    NG: int = 50           # num gaussians
    G: int = 10000         # graph nodes
    E: int = 1000000       # conformer edges
    EG: int = 30000        # graph edges
    VOCAB: int = 5
    CUTOFF: float = 10.0
    cores: int = 8
    qsize: int = 25000     # src quadrant size for int16 gather indices
    B_E: int = 384         # fixed edges per (dst-window, src-quad) bucket
    gwin: int = 128        # GIN scatter window (<=128 segments)
    PW: int = 640          # padded GIN edges per (core, window) (multiple of 128)
    nchunk: int = 500      # node chunk for the h/out stage (divides NS, mult of 10)

    @property
    def NS(self):
        return self.N // self.cores

    @property
    def GS(self):
        return self.G // self.cores

    @property
    def CW(self):
        """dst windows of 128 nodes per core shard."""
        return (self.NS + 127) // 128

    @property
    def EH(self):
        """padded edges per src-half stream (2 quad buckets per window)."""
        return self.CW * 2 * self.B_E

    @property
    def E_pad(self):
        return 2 * self.EH

    @property
    def NWIN(self):
        return (self.GS + self.gwin - 1) // self.gwin

    @property
    def EG_pad(self):
        return self.NWIN * self.PW


REAL = Dims()


def _wrap16(arr, dtype=np.int16):
    """Edge i -> [i % 16, i // 16], replicated to 128 partitions."""
    a = np.asarray(arr).reshape(-1, 16).T.astype(dtype)  # [16, n/16]
    return np.tile(a, (8, 1)).copy()  # [128, n/16]


def _tile128(arr, dtype=np.float32):
    """Edge i -> [i % 128, i // 128] (per-partition scalar layout)."""
    return np.ascontiguousarray(np.asarray(arr).reshape(-1, 128).T.astype(dtype))


def host_prep(inputs, D: Dims):
    """Build per-core in_maps (list of dicts) for the SPMD kernel."""
    x = np.asarray(inputs["x"], np.float32)
    cnb = np.asarray(inputs["conf_node_batch"]).astype(np.int64)
    ei = np.asarray(inputs["edge_index_conf"]).astype(np.int64)
    ew = np.asarray(inputs["edge_weight_conf"], np.float32)
    ea = np.asarray(inputs["edge_attr_conf"], np.float32)
    eig = np.asarray(inputs["edge_index_graph"]).astype(np.int64)
    eag = np.asarray(inputs["edge_attr_graph"]).astype(np.int64)

    rep = D.N // D.G
    assert np.array_equal(cnb, np.repeat(np.arange(D.G), rep)), \
        "conf_node_batch structure mismatch"

    NS, GS = D.NS, D.GS
    src, dst = ei[0], ei[1]

    # ---- conformer edges: window-major one-hot layout ----
    # per core: two streams by src half (matching the split AllGather's
    # permuted xf layout); within a stream, buckets of fixed B_E edges per
    # (dst-window of 128, src quad within the half); in-quad src offset =
    # (src_core % 4)*NS/2 + (src % NS/2)
    B_E, CW, EH = D.B_E, D.CW, D.EH
    core = dst // NS
    sc = src // NS
    so = src % NS
    sh = so // (NS // 2)
    sj = (sc >= 4).astype(np.int64)
    inq = (sc % 4) * (NS // 2) + (so % (NS // 2))
    win = (dst % NS) // 128
    bucket = ((core * 2 + sh) * CW + win) * 2 + sj
    order = np.argsort(bucket, kind="stable")
    sb = bucket[order]
    nbuck = D.cores * 2 * CW * 2
    bounds = np.searchsorted(sb, np.arange(nbuck + 1))
    counts = bounds[1:] - bounds[:-1]
    assert counts.max() <= B_E, f"bucket overflow: {counts.max()} > {B_E}"
    rank = np.arange(len(sb)) - bounds[sb]
    c_b = sb // (2 * CW * 2)
    rem = sb % (2 * CW * 2)
    h_b = rem // (CW * 2)
    w_b = (rem % (CW * 2)) // 2
    j_b = rem % 2
    flat = (c_b * D.E_pad + h_b * EH + (w_b * 2 + j_b) * B_E + rank)

    src_pad = np.zeros(D.cores * D.E_pad, np.int64)
    drel_pad = np.full(D.cores * D.E_pad, -1.0, np.float32)
    w_pad = np.full(D.cores * D.E_pad, D.CUTOFF, np.float32)  # C(CUTOFF)=0
    a_pad = np.zeros((D.cores * D.E_pad, D.NG), np.float32)
    src_pad[flat] = inq[order]
    drel_pad[flat] = ((dst[order] % NS) - w_b * 128).astype(np.float32)
    w_pad[flat] = ew[order]
    a_pad[flat] = ea[order]

    AT = np.zeros((D.cores, D.NG, D.E_pad), np.float32)
    WT = np.zeros((D.cores, 128, D.E_pad // 128), np.float32)
    DRELC = np.zeros((D.cores, 128, D.E_pad // 128), np.float32)
    SRC = np.zeros((D.cores, 128, D.E_pad // 16), np.int16)
    for c in range(D.cores):
        sl = slice(c * D.E_pad, (c + 1) * D.E_pad)
        AT[c] = a_pad[sl].T
        WT[c] = _tile128(w_pad[sl])
        DRELC[c] = _tile128(drel_pad[sl])
        SRC[c] = _wrap16(src_pad[sl])

    # ---- graph edges: order by (core(dst), window(dst), dst) ----
    sg, dg = eig[0], eig[1]
    gcore = dg // GS
    gwin = (dg - gcore * GS) // D.gwin
    gorder = np.lexsort((dg, gwin, gcore))
    g_s, g_d, g_w, g_c = sg[gorder], dg[gorder], gwin[gorder], gcore[gorder]
    g_a = eag[gorder]

    SG = np.zeros((D.cores, 128, D.EG_pad // 16), np.int16)
    DREL = np.zeros((D.cores, 128, D.EG_pad // 128), np.float32)
    BHOT = np.zeros((D.cores, 3 * D.VOCAB, D.EG_pad), np.float32)

    gkeys = g_c * D.NWIN + g_w
    gbounds = np.searchsorted(gkeys, np.arange(D.cores * D.NWIN + 1))
    for c in range(D.cores):
        sg_pad = np.zeros(D.EG_pad, np.int64)
        dr_pad = np.full(D.EG_pad, -1.0, np.float32)  # -1 kills pads in one-hot
        bh_pad = np.zeros((3 * D.VOCAB, D.EG_pad), np.float32)
        for w in range(D.NWIN):
            lo, hi = gbounds[c * D.NWIN + w], gbounds[c * D.NWIN + w + 1]
            cnt = hi - lo
            assert cnt <= D.PW, f"PW overflow: core {c} win {w}: {cnt}"
            o = w * D.PW
            sg_pad[o:o + cnt] = g_s[lo:hi]
            dr_pad[o:o + cnt] = (g_d[lo:hi] - c * GS - w * D.gwin).astype(np.float32)
            for k in range(3):
                bh_pad[k * D.VOCAB + g_a[lo:hi, k], np.arange(o, o + cnt)] = 1.0
        SG[c] = _wrap16(sg_pad)
        DREL[c] = _tile128(dr_pad)
        BHOT[c] = bh_pad

    # ---- x^T shards ----
    xT = np.ascontiguousarray(x.T)  # [H, N]
    XT = xT.reshape(2, 128, D.N)

    # ---- weights (replicated) ----
    H2 = D.H // 128
    w = {k: np.asarray(inputs[k], np.float32) for k in (
        "mlp_w1", "mlp_b1", "mlp_w2", "mlp_b2", "cf_lin1", "cf_lin2",
        "cf_lin2_b", "lin_w", "lin_b", "bond_emb", "gin_eps", "gin_w1",
        "gin_w2", "bn1_g", "bn1_b", "bn2_g", "bn2_b")}
    const = dict(
        w1=w["mlp_w1"],                                   # [NG, NF]
        b1col=w["mlp_b1"].reshape(D.NF, 1),
        w2=w["mlp_w2"],                                   # [NF, NF]
        b2row=w["mlp_b2"].reshape(1, D.NF),
        b2row3=np.tile(w["mlp_b2"].reshape(1, D.NF), (1, 3)),
        ones1=np.ones((1, 128), np.float32),
        lin1=np.ascontiguousarray(w["cf_lin1"].reshape(H2, 128, D.NF)),
        lin2=w["cf_lin2"],                                # [NF, H]
        lin2b=w["cf_lin2_b"].reshape(H2, 128, 1),
        linw=np.ascontiguousarray(w["lin_w"].reshape(H2, 128, D.H)),
        linb=w["lin_b"].reshape(H2, 128, 1),
        gw1=np.ascontiguousarray(w["gin_w1"].reshape(H2, 128, D.H)),
        gw2=np.ascontiguousarray(w["gin_w2"].reshape(H2, 128, D.H)),
        bondcat=np.ascontiguousarray(
            w["bond_emb"].reshape(3 * D.VOCAB, D.H)),
        bn1g=w["bn1_g"].reshape(H2, 128, 1), bn1b=w["bn1_b"].reshape(H2, 128, 1),
        bn2g=w["bn2_g"].reshape(H2, 128, 1), bn2b=w["bn2_b"].reshape(H2, 128, 1),
        epsv=np.full((128, 1), 1.0 + float(w["gin_eps"]), np.float32),
        zerocol=np.zeros((128, 1), np.float32),
        eps5col=np.full((128, 1), 1e-5, np.float32),
        pihalf=np.full((128, 1), -math.pi / 2, np.float32),
        iota=np.tile(np.arange(128, dtype=np.float32), (128, 1)).copy(),
        iota6=np.tile(np.arange(128, dtype=np.float32), (128, 6)).copy(),
        ident=np.eye(128, dtype=np.float32),
    )

    in_maps = []
    for c in range(D.cores):
        m = dict(
            xT=np.ascontiguousarray(XT[:, :, c * NS:(c + 1) * NS]),
            AT=AT[c], WT=WT[c], SRC=SRC[c], DRELC=DRELC[c],
            SG=SG[c], DREL=DREL[c], BHOT=BHOT[c],
        )
        m.update(const)
        in_maps.append(m)
    return in_maps


def assemble(results, D: Dims):
    """Per-core outT [2,128,NS] -> full [N, H]."""
    parts = [r["outT"].reshape(D.H, D.NS) for r in results]
    outT = np.concatenate(parts, axis=1)  # [H, N]
    return np.ascontiguousarray(outT.T)


def _ts(i, n):
    return bass.ts(i, n)


def build_nc(D: Dims, flags: frozenset = frozenset()):
    nc = bacc.Bacc("TRN2", target_bir_lowering=False, debug=False,
                   num_devices=D.cores, num_swdge_queues=3)
    NS, GS, H, NF, NG = D.NS, D.GS, D.H, D.NF, D.NG
    H2 = H // 128

    I = {}
    def di(name, shape, dt=F32):
        I[name] = nc.dram_tensor(name, list(shape), dt, kind="ExternalInput")
        return I[name]

    di("xT", [2, 128, NS])
    di("AT", [NG, D.E_pad])
    di("WT", [128, D.E_pad // 128])
    di("SRC", [128, D.E_pad // 16], I16)
    di("DRELC", [128, D.E_pad // 128])
    di("SG", [128, D.EG_pad // 16], I16)
    di("DREL", [128, D.EG_pad // 128])
    di("BHOT", [3 * D.VOCAB, D.EG_pad])
    di("w1", [NG, NF]); di("b1col", [NF, 1]); di("w2", [NF, NF])
    di("b2row", [1, NF]); di("b2row3", [1, 3 * NF]); di("ones1", [1, 128])
    di("lin1", [H2, 128, NF]); di("lin2", [NF, H]); di("lin2b", [H2, 128, 1])
    di("linw", [H2, 128, H]); di("linb", [H2, 128, 1])
    di("gw1", [H2, 128, H]); di("gw2", [H2, 128, H])
    di("bondcat", [3 * D.VOCAB, H])
    di("bn1g", [H2, 128, 1]); di("bn1b", [H2, 128, 1])
    di("bn2g", [H2, 128, 1]); di("bn2b", [H2, 128, 1])
    di("epsv", [128, 1]); di("iota", [128, 128]); di("iota6", [128, 768])
    di("ident", [128, 128])
    di("zerocol", [128, 1]); di("eps5col", [128, 1]); di("pihalf", [128, 1])

    outT = nc.dram_tensor("outT", [2, 128, NS], F32, kind="ExternalOutput")

    groups = [list(range(D.cores))]

    with tile.TileContext(nc) as tc:
        with (
            tc.tile_pool(name="const", bufs=1) as cp,
            tc.tile_pool(name="work", bufs=2) as wp,
            tc.tile_pool(name="small", bufs=3) as sp,
            tc.tile_pool(name="gin", bufs=2) as ctx_gin_pool,
            tc.tile_pool(name="psum", bufs=2, space="PSUM") as pp,
            tc.tile_pool(name="dram", bufs=1, space="DRAM") as dp,
        ):
            # ---------- load constants ----------
            C = {}
            for nm, shp in [("w1", [NG, NF]), ("b1col", [NF, 1]),
                            ("w2", [NF, NF]), ("b2row", [1, NF]),
                            ("b2row3", [1, 3 * NF]),
                            ("ones1", [1, 128]), ("lin2", [NF, H]),
                            ("bondcat", [3 * D.VOCAB, H]),
                            ("epsv", [128, 1]), ("iota", [128, 128]),
                            ("iota6", [128, 768]),
                            ("ident", [128, 128]), ("zerocol", [128, 1]),
                            ("eps5col", [128, 1]), ("pihalf", [128, 1])]:
                t = cp.tile(shp, F32, name=f"c_{nm}")
                nc.sync.dma_start(t[:], I[nm].ap())
                C[nm] = t
            nc.const_aps.aps[(F32, 0.0)] = C["zerocol"][:]
            # [H2,128,*] constants: load as per-half tiles
            for nm in ("lin1", "lin2b", "linw", "linb", "gw1", "gw2",
                       "bn1g", "bn1b", "bn2g", "bn2b"):
                C[nm] = []
                inner = I[nm].shape[2]
                for k in range(H2):
                    t = cp.tile([128, inner], F32, name=f"c_{nm}{k}")
                    nc.sync.dma_start(t[:], I[nm].ap()[k])
                    C[nm].append(t)

            # ---------- DRAM scratch ----------
            # xf shard/full split in half so the AllGather pipelines with
            # compute: AG#h gathers every core's half-h shard; the gathered
            # layout is permuted (half-major), host_prep permutes src indices
            HSH = NS // 2
            xf_shard = [dp.tile([HSH, NF], F32, name=f"xf_shard{h}")
                        for h in range(2)]
            xf_full = [dp.tile([D.cores * HSH, NF], F32, name=f"xf_full{h}",
                               addr_space="Shared") for h in range(2)]
            xagg_shard = dp.tile([GS, H], F32, name="xagg_shard")
            xagg_full = dp.tile([D.cores * GS, H], F32, name="xagg_full",
                                addr_space="Shared")
            agg_ab = [dp.tile([NS, NF], F32, name=f"agg_{h}")
                      for h in range(2)]
            st1_in = dp.tile([128, 4], F32, name="st1_in")
            st1_out = dp.tile([128, 4], F32, name="st1_out", addr_space="Shared")
            st2_in = dp.tile([128, 4], F32, name="st2_in")
            st2_out = dp.tile([128, 4], F32, name="st2_out", addr_space="Shared")

            # =========== Phase A: segment-max pool first, then xf ===========
            # pass 1 pools so the (small) xagg AllGather is issued as early
            # as possible; pass 2 re-reads x and computes xf, issuing each
            # half's AllGather as soon as that half of the shard is written
            rep = D.N // D.G
            PCH = 1250
            n_pch = NS // PCH
            half_chunks = n_pch // 2
            xaggT = [cp.tile([128, GS], F32, name=f"xaggT{k}") for k in range(2)]
            for j in range(n_pch):
                xt = [wp.tile([128, PCH], F32, tag=f"ph_a_xt{k}", name=f"ph_a_xt{k}")
                      for k in range(2)]
                for k in range(2):
                    nc.sync.dma_start(xt[k][:], I["xT"].ap()[k, :, _ts(j, PCH)])
                # pool: max over groups of 10 cols
                for k in range(2):
                    nc.vector.tensor_reduce(
                        xaggT[k][:, _ts(j, PCH // rep)],
                        xt[k][:].rearrange("p (g t) -> p g t", t=rep),
                        AX.X, ALU.max)

            # transpose x_aggT -> node-major x_agg shard
            GT = (GS + 127) // 128
            for t in range(GT):
                m = min(128, GS - t * 128)
                for k in range(2):
                    pst = pp.tile([128, 128], F32, tag="ps_tr", name="ps_tr")
                    nc.tensor.transpose(pst[:m, :], xaggT[k][:, t * 128:t * 128 + m],
                                        C["ident"][:])
                    sb = sp.tile([128, 128], F32, tag="ph_a_trsb", name="ph_a_trsb")
                    nc.scalar.copy(sb[:m, :], pst[:m, :])
                    nc.sync.dma_start(
                        xagg_shard[t * 128:t * 128 + m, _ts(k, 128)], sb[:m, :])

            if "no_coll" not in flags and "no_coll_xagg" not in flags:
                nc.gpsimd.collective_compute(
                    "AllGather", ALU.bypass, replica_groups=groups,
                    ins=[xagg_shard.opt()], outs=[xagg_full.opt()])

            # pass 2: xf = x @ cf_lin1, node-major tiles of <=128
            for j in range(n_pch):
                xt = [wp.tile([128, PCH], F32, tag=f"ph_a_xt{k}", name=f"ph_a2_xt{k}")
                      for k in range(2)]
                for k in range(2):
                    nc.sync.dma_start(xt[k][:], I["xT"].ap()[k, :, _ts(j, PCH)])
                h = j // half_chunks
                r0 = (j - h * half_chunks) * PCH
                nt = (PCH + 127) // 128
                for t in range(nt):
                    m = min(128, PCH - t * 128)
                    ps = pp.tile([128, NF], F32, tag="ps_mm", name="ps_mm")
                    for k in range(2):
                        nc.tensor.matmul(ps[:m, :], xt[k][:, t * 128:t * 128 + m],
                                         C["lin1"][k][:], start=(k == 0),
                                         stop=(k == 1))
                    sb = sp.tile([128, NF], F32, tag="ph_a_sb", name="ph_a_sb")
                    nc.scalar.copy(sb[:m, :], ps[:m, :])
                    nc.sync.dma_start(
                        xf_shard[h][r0 + t * 128: r0 + t * 128 + m, :],
                        sb[:m, :])
                if (j == half_chunks - 1 and "no_coll" not in flags
                        and "no_coll_xf" not in flags):
                    nc.gpsimd.collective_compute(
                        "AllGather", ALU.bypass, replica_groups=groups,
                        ins=[xf_shard[0].opt()], outs=[xf_full[0].opt()])
            # AG#1 is issued later (after st1's AllReduce) so the GIN stats
            # reduction isn't queued behind it on the collective engine

            # =========== Phase B: GIN branch (sharded by graph node) =========
            # gather x_agg[sg], edge_emb via bond one-hot matmul, relu,
            # one-hot scatter into agg_g windows
            sgidx = cp.tile([128, D.EG_pad // 16], I16, name="sgidx_sb")
            nc.sync.dma_start(sgidx[:], I["SG"].ap())
            drel = cp.tile([128, D.EG_pad // 128], F32, name="drel_sb")
            nc.sync.dma_start(drel[:], I["DREL"].ap())

            # t-buffer (node-major (1+eps)x_agg + agg_g), then transposed halves
            gp = ctx_gin_pool
            tT = [gp.tile([128, GS], F32, tag=f"ginbuf{k}", name=f"tT{k}")
                  for k in range(2)]

            assert D.EG_pad % 128 == 0
            tiles_per_win = D.PW // 128
            for w in range(0 if "no_b" in flags else D.NWIN):
                m = min(D.gwin, GS - w * D.gwin)
                # gather this window's source rows
                gath_g = wp.tile([128, tiles_per_win, H], F32,
                                 tag="ph_b_gath", name="ph_b_gath")
                nc.gpsimd.dma_gather(
                    gath_g[:], xagg_full[:],
                    sgidx[:, w * D.PW // 16:(w + 1) * D.PW // 16],
                    num_idxs=D.PW, num_idxs_reg=D.PW, elem_size=H)
                bhot = wp.tile([3 * D.VOCAB, D.PW], F32, tag="ph_b_bhot",
                               name="ph_b_bhot")
                nc.sync.dma_start(bhot[:],
                                  I["BHOT"].ap()[:, w * D.PW:(w + 1) * D.PW])
                ps_agg = pp.tile([128, H], F32, tag="ps_agg", name="ps_agg")
                for i in range(tiles_per_win):
                    t = w * tiles_per_win + i
                    # edge embedding: one-hot bond matmul (K=15)
                    ps_emb = pp.tile([128, H], F32, tag="ps_mm", name="ps_mm")
                    nc.tensor.matmul(ps_emb[:], bhot[:, _ts(i, 128)],
                                     C["bondcat"][:], start=True, stop=True)
                    # msg = relu(gathered + emb)
                    msg = sp.tile([128, H], F32, tag="ph_b_msg", name="ph_b_msg")
                    nc.vector.tensor_tensor(msg[:], gath_g[:, i, :], ps_emb[:],
                                            ALU.add)
                    nc.scalar.activation(msg[:], msg[:], ACTF.Relu)
                    # one-hot scatter
                    oh = sp.tile([128, D.gwin], F32, tag="ph_b_oh", name="ph_b_oh")
                    nc.vector.tensor_scalar(oh[:], C["iota"][:, :D.gwin],
                                            drel[:, t:t + 1], None, ALU.is_equal)
                    nc.tensor.matmul(ps_agg[:m, :], oh[:, :m], msg[:],
                                     start=(i == 0), stop=(i == tiles_per_win - 1))
                # t = (1+eps) * x_agg + agg_g  (node-major window rows)
                xa = sp.tile([128, H], F32, tag="ph_b_xa", name="ph_b_xa")
                nc.sync.dma_start(
                    xa[:m, :], xagg_shard[w * D.gwin:w * D.gwin + m, :])
                tn = sp.tile([128, H], F32, tag="ph_b_tn", name="ph_b_tn")
                nc.vector.tensor_scalar(tn[:m, :], xa[:m, :], C["epsv"][:m, :],
                                        None, ALU.mult)
                nc.vector.tensor_tensor(tn[:m, :], tn[:m, :], ps_agg[:m, :],
                                        ALU.add)
                # transpose to feature-major tT
                for k in range(2):
                    pst = pp.tile([128, 128], F32, tag="ps_tr", name="ps_tr")
                    nc.tensor.transpose(pst[:, :m], tn[:m, _ts(k, 128)],
                                        C["ident"][:m, :m])
                    nc.vector.tensor_copy(tT[k][:, w * D.gwin:w * D.gwin + m],
                                          pst[:, :m])

            def gin_mm_and_stats_issue(inT, Wc, uT, stats_in, stats_out, label):
                """u = in @ W (node-major tiles), transpose to uT, stats;
                issues the stats AllReduce but does NOT read the result."""
                for t in range(GT):
                    m = min(128, GS - t * 128)
                    ps = pp.tile([128, H], F32, tag="ps_mm", name="ps_mm")
                    for k in range(2):
                        nc.tensor.matmul(ps[:m, :],
                                         inT[k][:, t * 128:t * 128 + m],
                                         Wc[k][:], start=(k == 0), stop=(k == 1))
                    sb = sp.tile([128, H], F32, tag=f"{label}_sb", name=f"{label}_sb")
                    nc.scalar.copy(sb[:m, :], ps[:m, :])
                    for k in range(2):
                        pst = pp.tile([128, 128], F32, tag="ps_tr", name="ps_tr")
                        nc.tensor.transpose(pst[:, :m], sb[:m, _ts(k, 128)],
                                            C["ident"][:m, :m])
                        nc.vector.tensor_copy(uT[k][:, t * 128:t * 128 + m],
                                              pst[:, :m])
                st = sp.tile([128, 4], F32, tag=f"{label}_st", name=f"{label}_st")
                sq = sp.tile([128, GS], F32, tag="gin_sq", name="gin_sq",
                             bufs=1)
                for k in range(2):
                    nc.vector.tensor_reduce(st[:, 2 * k:2 * k + 1], uT[k][:],
                                            AX.X, ALU.add)
                    nc.vector.tensor_tensor(sq[:], uT[k][:], uT[k][:], ALU.mult)
                    nc.vector.tensor_reduce(st[:, 2 * k + 1:2 * k + 2], sq[:],
                                            AX.X, ALU.add)
                nc.sync.dma_start(stats_in[:], st[:])
                if "no_b_ar" not in flags:
                    nc.gpsimd.collective_compute(
                        "AllReduce", ALU.add, replica_groups=groups,
                        ins=[stats_in.opt()], outs=[stats_out.opt()])

            def gin_read_stf(stats_in, stats_out, label):
                stf = sp.tile([128, 4], F32, tag=f"{label}_stf", name=f"{label}_stf")
                nc.sync.dma_start(
                    stf[:], stats_in[:] if "no_b_ar" in flags else stats_out[:])
                return stf

            def bn_apply(stf, uT, g_c, b_c, outT_t, relu, label):
                """out = func((u - mu) * g / sqrt(var+eps) + b), feature-major."""
                inv_n = 1.0 / float(D.G)
                for k in range(2):
                    mu = sp.tile([128, 1], F32, tag=f"{label}_mu{k}", name=f"{label}_mu{k}")
                    nc.vector.tensor_scalar(mu[:], stf[:, 2 * k:2 * k + 1],
                                            inv_n, None, ALU.mult)
                    var = sp.tile([128, 1], F32, tag=f"{label}_va{k}", name=f"{label}_va{k}")
                    nc.vector.tensor_scalar(var[:], stf[:, 2 * k + 1:2 * k + 2],
                                            inv_n, None, ALU.mult)
                    mu2 = sp.tile([128, 1], F32, tag=f"{label}_m2{k}", name=f"{label}_m2{k}")
                    nc.vector.tensor_tensor(mu2[:], mu[:], mu[:], ALU.mult)
                    nc.vector.tensor_tensor(var[:], var[:], mu2[:], ALU.subtract)
                    sd = sp.tile([128, 1], F32, tag=f"{label}_sd{k}", name=f"{label}_sd{k}")
                    nc.scalar.activation(sd[:], var[:], ACTF.Sqrt,
                                         bias=C["eps5col"][:])
                    rs = sp.tile([128, 1], F32, tag=f"{label}_rs{k}", name=f"{label}_rs{k}")
                    nc.vector.reciprocal(rs[:], sd[:])
                    sc = sp.tile([128, 1], F32, tag=f"{label}_sc{k}", name=f"{label}_sc{k}")
                    nc.vector.tensor_tensor(sc[:], g_c[k][:], rs[:], ALU.mult)
                    sh = sp.tile([128, 1], F32, tag=f"{label}_sh{k}", name=f"{label}_sh{k}")
                    nc.vector.tensor_tensor(sh[:], mu[:], sc[:], ALU.mult)
                    nc.vector.tensor_tensor(sh[:], b_c[k][:], sh[:], ALU.subtract)
                    nc.scalar.activation(outT_t[k][:], uT[k][:],
                                         ACTF.Relu if relu else ACTF.Identity,
                                         bias=sh[:], scale=sc[:])

            def gin_buf(nm):
                return [gp.tile([128, GS], F32, tag=f"ginbuf{k}",
                                name=f"{nm}{k}") for k in range(2)]

            bstate = {}
            if "no_b" in flags:
                ginT = gin_buf("ginT")
                for k in range(2):
                    nc.vector.memset(ginT[k][:], 0.0)
                bstate["ginT"] = ginT
            else:
                uT = gin_buf("uT")
                gin_mm_and_stats_issue(tT, C["gw1"], uT, st1_in, st1_out,
                                       "gmm1")

            # xf AG#1 queued on the collective engine AFTER st1's AllReduce
            # so the (tiny) stats reduce isn't stuck behind the 25MB gather
            if "no_coll" not in flags and "no_coll_xf" not in flags:
                nc.gpsimd.collective_compute(
                    "AllGather", ALU.bypass, replica_groups=groups,
                    ins=[xf_shard[1].opt()], outs=[xf_full[1].opt()])

            def emit_b2():
                """bn1 + gmm2 + st2 AllReduce issue (mid phase C)."""
                if "no_b" in flags or "t2T" in bstate:
                    return
                stf1 = gin_read_stf(st1_in, st1_out, "gmm1")
                t1T = gin_buf("t1T")
                bn_apply(stf1, uT, C["bn1g"], C["bn1b"], t1T, True, "bn1")
                t2T = gin_buf("t2T")
                gin_mm_and_stats_issue(t1T, C["gw2"], t2T, st2_in, st2_out,
                                       "gmm2")
                bstate["t2T"] = t2T

            def emit_b3():
                """bn2 -> ginT (late in phase C)."""
                if "no_b" in flags or "ginT" in bstate:
                    return
                stf2 = gin_read_stf(st2_in, st2_out, "gmm2")
                ginT = gin_buf("ginT")
                bn_apply(stf2, bstate["t2T"], C["bn2g"], C["bn2b"], ginT,
                         False, "bn2")
                bstate["ginT"] = ginT

            # =========== Phase C: conformer edge pipeline ===========
            # window-major: per (src-half stream h, dst-window w of 128
            # nodes): gather the window's edges (one call per src quad
            # bucket of B_E), compute msg = (xf[src]) * (mlp(A) + b2), then
            # aggregate over dst via one-hot matmuls into PSUM (the one-hot
            # rows carry the cosine-cutoff C so no separate C-multiply),
            # and flush the window's 128 agg rows with a plain DMA write.
            # No scatter-add: each agg row is written exactly once.
            # resident: C row (cosine cutoff) and dst-rel row per edge
            crow = cp.tile([128, D.E_pad // 128], F32, name="crow_sb")
            for s0 in range(0, D.E_pad // 128, 512):
                sw = min(512, D.E_pad // 128 - s0)
                wt = wp.tile([128, 512], F32, tag="ph_c_wt", name="ph_c_wt")
                nc.sync.dma_start(wt[:, :sw], I["WT"].ap()[:, s0:s0 + sw])
                nc.scalar.activation(wt[:, :sw], wt[:, :sw], ACTF.Sin,
                                     bias=C["pihalf"][:],
                                     scale=math.pi / D.CUTOFF)
                nc.scalar.activation(crow[:, s0:s0 + sw], wt[:, :sw],
                                     ACTF.Copy, bias=0.5, scale=-0.5)
            drelc = cp.tile([128, D.E_pad // 128], F32, name="drelc_sb")
            nc.sync.dma_start(drelc[:], I["DRELC"].ap())

            B_E, CW, EH = D.B_E, D.CW, D.EH
            WE = 2 * B_E           # edges per (stream, window)
            NTW = WE // 128        # tiles per window (6)
            wstream = ([] if "no_c" in flags else
                       [(h, w) for h in range(2) for w in range(CW)])
            PFD = 8  # gather prefetch depth (windows issued ahead)
            gat_fifo = []
            F32R = mybir.dt.float32r

            def emit_gather(idx):
                h, w = wstream[idx]
                e0 = h * EH + w * WE
                if "no_gather" in flags:
                    if "no_cmm" in flags:
                        return None
                    gat = wp.tile([128, NTW, NF], F32, tag="ph_c_gat",
                                  name="ph_c_gat", bufs=PFD + 2)
                    nc.vector.memset(gat[:], 0.0)
                    return gat
                gat = wp.tile([128, NTW, NF], F32, tag="ph_c_gat",
                              name="ph_c_gat", bufs=PFD + 2)
                si = wp.tile([128, WE // 16], I16, tag="ph_c_si",
                             name="ph_c_si", bufs=PFD + 2)
                nc.sync.dma_start(
                    si[:], I["SRC"].ap()[:, e0 // 16:(e0 + WE) // 16])
                for j in range(2):
                    nc.gpsimd.dma_gather(
                        gat[:, j * (NTW // 2):(j + 1) * (NTW // 2), :],
                        xf_full[h][j * D.qsize:(j + 1) * D.qsize, :],
                        si[:, j * B_E // 16:(j + 1) * B_E // 16],
                        num_idxs=B_E, num_idxs_reg=B_E, elem_size=NF,
                        queue_num=(2 * idx + j) % 3)
                return gat

            def emit_compute_flush(idx, gat):
                h, w = wstream[idx]
                e0 = h * EH + w * WE
                c0col = e0 // 128
                msg = None
                if "no_cmm" not in flags:
                    msg = wp.tile([128, NTW, NF], F32, tag="ph_c_msg",
                                  name="ph_c_msg")
                    at = wp.tile([NG, WE], F32, tag="ph_c_at",
                                 name="ph_c_at")
                    nc.sync.dma_start(at[:], I["AT"].ap()[:, e0:e0 + WE])
                    # per 384-edge group: mm1+relu (fp32r), mm2 + b2 packed
                    # 3 tiles into one PSUM bank; 384-wide msg-mul
                    for g in range(2):
                        s0 = g * 384
                        ps1 = pp.tile([128, 384], F32, tag="ps_mm", name="ps_mm")
                        nc.tensor.matmul(ps1[:], C["w1"][:],
                                         at[:, s0:s0 + 384],
                                         start=True, stop=True)
                        h1 = wp.tile([128, 384], F32, tag="ph_c_h1",
                                     name="ph_c_h1")
                        nc.scalar.activation(h1[:], ps1[:],
                                             ACTF.Relu, bias=C["b1col"][:])
                        psw = pp.tile([128, 3, NF], F32, tag="ps_w", name="ps_w")
                        for t3 in range(3):
                            nc.tensor.matmul(psw[:, t3, :], h1[:, _ts(t3, 128)],
                                             C["w2"][:], start=True,
                                             stop="b2zero" in flags)
                            if "b2zero" not in flags:
                                nc.tensor.matmul(psw[:, t3, :], C["ones1"][:],
                                                 C["b2row"][:], start=False,
                                                 stop=True)
                        nc.vector.tensor_tensor(msg[:, 3 * g:3 * g + 3, :],
                                                gat[:, 3 * g:3 * g + 3, :],
                                                psw[:], ALU.mult)
                if "no_scatter" in flags or "no_cmm" in flags:
                    return
                # one-hot aggregation: rows carry C; accumulate over tiles
                ps_agg = pp.tile([128, NF], F32, tag="ps_agg", name="ps_cagg")
                ohc = sp.tile([128, NTW, 128], F32, tag="ph_c_oh",
                              name="ph_c_oh")
                nc.vector.tensor_tensor(
                    ohc[:],
                    C["iota6"][:].rearrange("p (t j) -> p t j", j=128),
                    drelc[:, c0col:c0col + NTW].broadcast_to((128, NTW, 128)),
                    ALU.is_equal)
                nc.vector.tensor_tensor(
                    ohc[:], ohc[:],
                    crow[:, c0col:c0col + NTW].broadcast_to((128, NTW, 128)),
                    ALU.mult)
                for t in range(NTW):
                    nc.tensor.matmul(ps_agg[:], ohc[:, t, :], msg[:, t, :],
                                     start=(t == 0), stop=(t == NTW - 1))
                stg = sp.tile([128, NF], F32, tag="ph_c_stg", name="ph_c_stg")
                nc.scalar.copy(stg[:], ps_agg[:])
                base = w * 128
                m = min(128, NS - base)
                nc.sync.dma_start(
                    agg_ab[h][base:base + m, :].rearrange(
                        "(t p) f -> p t f", p=m),
                    stg[:m, :].rearrange("p (t f) -> p t f", f=NF))

            # =========== Phase D: h = relu(agg@lin2+b)@linw+b, residual =====
            # emitted as a closure so node chunks interleave into the tail
            # of the window stream (chunk j only needs agg rows already
            # flushed by both streams, plus ginT)
            NCH = D.nchunk
            n_nch = NS // NCH

            def emit_d(j):
                r0 = j * NCH
                # load agg rows, transpose to feature-major aggT [NF, NCH]
                aggT = wp.tile([NF, NCH], F32, tag="ph_d_aggT", name="ph_d_aggT")
                ntt = (NCH + 127) // 128
                for t in range(ntt):
                    m = min(128, NCH - t * 128)
                    asb = sp.tile([128, NF], F32, tag="ph_d_asb", name="ph_d_asb")
                    nc.sync.dma_start(asb[:m, :],
                                      agg_ab[0][r0 + t * 128:r0 + t * 128 + m, :])
                    bsb = sp.tile([128, NF], F32, tag="ph_d_bsb", name="ph_d_bsb")
                    nc.sync.dma_start(bsb[:m, :],
                                      agg_ab[1][r0 + t * 128:r0 + t * 128 + m, :])
                    nc.vector.tensor_tensor(asb[:m, :], asb[:m, :], bsb[:m, :],
                                            ALU.add)
                    pst = pp.tile([128, 128], F32, tag="ps_tr", name="ps_tr")
                    nc.tensor.transpose(pst[:, :m], asb[:m, :], C["ident"][:m, :m])
                    nc.vector.tensor_copy(aggT[:, t * 128:t * 128 + m],
                                          pst[:, :m])
                # h1T = relu(lin2^T @ aggT + b)  [2][128, NCH]
                h1T = [wp.tile([128, NCH], F32, tag=f"ph_d_h1T{k}", name=f"ph_d_h1T{k}")
                       for k in range(2)]
                for k in range(2):
                    ps = pp.tile([128, NCH], F32, tag="ps_mm", name="ps_mm")
                    nc.tensor.matmul(ps[:], C["lin2"][:, _ts(k, 128)], aggT[:],
                                     start=True, stop=True)
                    nc.scalar.activation(h1T[k][:], ps[:], ACTF.Relu,
                                         bias=C["lin2b"][k][:])
                # outT = linw^T @ h1T + linb + xT + gin[batch]
                for k in range(2):
                    ps = pp.tile([128, NCH], F32, tag="ps_mm", name="ps_mm")
                    for kk in range(2):
                        nc.tensor.matmul(ps[:], C["linw"][kk][:, _ts(k, 128)],
                                         h1T[kk][:], start=(kk == 0),
                                         stop=(kk == 1))
                    ob = sp.tile([128, NCH], F32, tag="ph_d_ob", name="ph_d_ob")
                    nc.scalar.activation(ob[:], ps[:], ACTF.Identity,
                                         bias=C["linb"][k][:])
                    xtc = sp.tile([128, NCH], F32, tag="ph_d_xtc", name="ph_d_xtc")
                    nc.sync.dma_start(xtc[:], I["xT"].ap()[k, :, r0:r0 + NCH])
                    nc.vector.tensor_tensor(ob[:], ob[:], xtc[:], ALU.add)
                    # + gin, each graph col repeated `rep` times
                    rep = D.N // D.G
                    g0 = r0 // rep
                    gin_rep = bstate["ginT"][k][:, g0:g0 + NCH // rep] \
                        .broadcast_to((128, NCH // rep, rep))
                    nc.vector.tensor_tensor(
                        ob[:].rearrange("p (g t) -> p g t", t=rep),
                        ob[:].rearrange("p (g t) -> p g t", t=rep),
                        gin_rep, ALU.add)
                    nc.sync.dma_start(outT.ap()[k, :, r0:r0 + NCH], ob[:])

            next_d = [0]

            def emit_d_ready(done):
                """Emit phase D chunks whose agg rows are fully flushed."""
                if "no_d" in flags or "ginT" not in bstate:
                    return
                while (next_d[0] < n_nch
                       and (done - CW + 1) * 128 >= NCH * (next_d[0] + 1)):
                    emit_d(next_d[0])
                    next_d[0] += 1

            for i in range(len(wstream) + PFD):
                if i < len(wstream):
                    gat_fifo.append(emit_gather(i))
                if i >= PFD:
                    done = i - PFD
                    emit_compute_flush(done, gat_fifo[done])
                    # GIN stage hooks: latency of the stats AllReduces and
                    # the serial BN chains hides under the window stream
                    if done == CW // 4:
                        emit_b2()
                    if done == CW + CW // 4:
                        emit_b3()
                    if done > CW:
                        emit_d_ready(done)
            emit_b2()  # no-op unless phase C was skipped
            emit_b3()
            if "no_d" not in flags:
                for j in range(next_d[0], n_nch):
                    emit_d(j)

    nc.compile()
    return nc


_CACHE = {}


def _get_nc(D: Dims, flags: frozenset = frozenset()):
    key = ("nc", D, flags)
    if key not in _CACHE:
        _CACHE[key] = build_nc(D, flags)
    return _CACHE[key]


def run_on_hw(inputs, D: Dims = REAL):
    flags = (frozenset({"b2zero"})
             if not np.any(np.asarray(inputs["mlp_b2"])) else frozenset())
    nc = _get_nc(D, flags)
    in_maps = host_prep(inputs, D)
    res = bass_utils.run_bass_kernel_spmd(nc, in_maps,
                                          core_ids=list(range(D.cores)))
    return assemble(res.results, D)


def kernel(**inputs):
    return run_on_hw(inputs, REAL)



# revision 51
# speedup vs baseline: 2.1362x; 1.1935x over previous
"""Trainium2 Bass kernel for nn_DSSConf (DSS conformer GNN message passing).

Self-contained: hardcodes shapes/sharding for the real problem; exposes
kernel(**inputs) -> np.ndarray.
"""
import sys
import math
from dataclasses import dataclass

sys.path.insert(0, "/opt/trn_rl_repo")

import numpy as np
from concourse import bass, bacc, tile, mybir, bass_utils

F32 = mybir.dt.float32
I16 = mybir.dt.int16
ALU = mybir.AluOpType
ACTF = mybir.ActivationFunctionType
AX = mybir.AxisListType


@dataclass(frozen=True)
class Dims:
    N: int = 100000        # conformer nodes
    H: int = 256           # hidden
    NF: int = 128          # num filters
    NG: int = 50           # num gaussians
    G: int = 10000         # graph nodes
    E: int = 1000000       # conformer edges
    EG: int = 30000        # graph edges
    VOCAB: int = 5
    CUTOFF: float = 10.0
    cores: int = 8
    qsize: int = 25000     # src quadrant size for int16 gather indices
    B_E: int = 384         # fixed edges per (dst-window, src-quad) bucket
    gwin: int = 128        # GIN scatter window (<=128 segments)
    PW: int = 640          # padded GIN edges per (core, window) (multiple of 128)
    nchunk: int = 500      # node chunk for the h/out stage (divides NS, mult of 10)

    @property
    def NS(self):
        return self.N // self.cores

    @property
    def GS(self):
        return self.G // self.cores

    @property
    def CW(self):
        """dst windows of 128 nodes per core shard."""
        return (self.NS + 127) // 128

    @property
    def EH(self):
        """padded edges per src-half stream (2 quad buckets per window)."""
        return self.CW * 2 * self.B_E

    @property
    def E_pad(self):
        return 2 * self.EH

    @property
    def NWIN(self):
        return (self.GS + self.gwin - 1) // self.gwin

    @property
    def EG_pad(self):
        return self.NWIN * self.PW


REAL = Dims()


def _wrap16(arr, dtype=np.int16):
    """Edge i -> [i % 16, i // 16], replicated to 128 partitions."""
    a = np.asarray(arr).reshape(-1, 16).T.astype(dtype)  # [16, n/16]
    return np.tile(a, (8, 1)).copy()  # [128, n/16]


def _tile128(arr, dtype=np.float32):
    """Edge i -> [i % 128, i // 128] (per-partition scalar layout)."""
    return np.ascontiguousarray(np.asarray(arr).reshape(-1, 128).T.astype(dtype))


def host_prep(inputs, D: Dims):
    """Build per-core in_maps (list of dicts) for the SPMD kernel."""
    x = np.asarray(inputs["x"], np.float32)
    cnb = np.asarray(inputs["conf_node_batch"]).astype(np.int64)
    ei = np.asarray(inputs["edge_index_conf"]).astype(np.int64)
    ew = np.asarray(inputs["edge_weight_conf"], np.float32)
    ea = np.asarray(inputs["edge_attr_conf"], np.float32)
    eig = np.asarray(inputs["edge_index_graph"]).astype(np.int64)
    eag = np.asarray(inputs["edge_attr_graph"]).astype(np.int64)

    rep = D.N // D.G
    assert np.array_equal(cnb, np.repeat(np.arange(D.G), rep)), \
        "conf_node_batch structure mismatch"

    NS, GS = D.NS, D.GS
    src, dst = ei[0], ei[1]

    # ---- conformer edges: window-major one-hot layout ----
    # per core: two streams by src half (matching the split AllGather's
    # permuted xf layout); within a stream, buckets of fixed B_E edges per
    # (dst-window of 128, src quad within the half); in-quad src offset =
    # (src_core % 4)*NS/2 + (src % NS/2)
    B_E, CW, EH = D.B_E, D.CW, D.EH
    core = dst // NS
    sc = src // NS
    so = src % NS
    sh = so // (NS // 2)
    sj = (sc >= 4).astype(np.int64)
    inq = (sc % 4) * (NS // 2) + (so % (NS // 2))
    win = (dst % NS) // 128
    bucket = ((core * 2 + sh) * CW + win) * 2 + sj
    order = np.argsort(bucket, kind="stable")
    sb = bucket[order]
    nbuck = D.cores * 2 * CW * 2
    bounds = np.searchsorted(sb, np.arange(nbuck + 1))
    counts = bounds[1:] - bounds[:-1]
    assert counts.max() <= B_E, f"bucket overflow: {counts.max()} > {B_E}"
    rank = np.arange(len(sb)) - bounds[sb]
    c_b = sb // (2 * CW * 2)
    rem = sb % (2 * CW * 2)
    h_b = rem // (CW * 2)
    w_b = (rem % (CW * 2)) // 2
    j_b = rem % 2
    flat = (c_b * D.E_pad + h_b * EH + (w_b * 2 + j_b) * B_E + rank)

    src_pad = np.zeros(D.cores * D.E_pad, np.int64)
    drel_pad = np.full(D.cores * D.E_pad, -1.0, np.float32)
    w_pad = np.full(D.cores * D.E_pad, D.CUTOFF, np.float32)  # C(CUTOFF)=0
    a_pad = np.zeros((D.cores * D.E_pad, D.NG), np.float32)
    src_pad[flat] = inq[order]
    drel_pad[flat] = ((dst[order] % NS) - w_b * 128).astype(np.float32)
    w_pad[flat] = ew[order]
    a_pad[flat] = ea[order]

    AT = np.zeros((D.cores, D.NG, D.E_pad), np.float32)
    WT = np.zeros((D.cores, 128, D.E_pad // 128), np.float32)
    DRELC = np.zeros((D.cores, 128, D.E_pad // 128), np.float32)
    SRC = np.zeros((D.cores, 128, D.E_pad // 16), np.int16)
    for c in range(D.cores):
        sl = slice(c * D.E_pad, (c + 1) * D.E_pad)
        AT[c] = a_pad[sl].T
        WT[c] = _tile128(w_pad[sl])
        DRELC[c] = _tile128(drel_pad[sl])
        SRC[c] = _wrap16(src_pad[sl])

    # ---- graph edges: order by (core(dst), window(dst), dst) ----
    sg, dg = eig[0], eig[1]
    gcore = dg // GS
    gwin = (dg - gcore * GS) // D.gwin
    gorder = np.lexsort((dg, gwin, gcore))
    g_s, g_d, g_w, g_c = sg[gorder], dg[gorder], gwin[gorder], gcore[gorder]
    g_a = eag[gorder]

    SG = np.zeros((D.cores, 128, D.EG_pad // 16), np.int16)
    DREL = np.zeros((D.cores, 128, D.EG_pad // 128), np.float32)
    BHOT = np.zeros((D.cores, 3 * D.VOCAB, D.EG_pad), np.float32)

    gkeys = g_c * D.NWIN + g_w
    gbounds = np.searchsorted(gkeys, np.arange(D.cores * D.NWIN + 1))
    for c in range(D.cores):
        sg_pad = np.zeros(D.EG_pad, np.int64)
        dr_pad = np.full(D.EG_pad, -1.0, np.float32)  # -1 kills pads in one-hot
        bh_pad = np.zeros((3 * D.VOCAB, D.EG_pad), np.float32)
        for w in range(D.NWIN):
            lo, hi = gbounds[c * D.NWIN + w], gbounds[c * D.NWIN + w + 1]
            cnt = hi - lo
            assert cnt <= D.PW, f"PW overflow: core {c} win {w}: {cnt}"
            o = w * D.PW
            sg_pad[o:o + cnt] = g_s[lo:hi]
            dr_pad[o:o + cnt] = (g_d[lo:hi] - c * GS - w * D.gwin).astype(np.float32)
            for k in range(3):
                bh_pad[k * D.VOCAB + g_a[lo:hi, k], np.arange(o, o + cnt)] = 1.0
        SG[c] = _wrap16(sg_pad)
        DREL[c] = _tile128(dr_pad)
        BHOT[c] = bh_pad

    # ---- x^T shards ----
    xT = np.ascontiguousarray(x.T)  # [H, N]
    XT = xT.reshape(2, 128, D.N)

    # ---- weights (replicated) ----
    H2 = D.H // 128
    w = {k: np.asarray(inputs[k], np.float32) for k in (
        "mlp_w1", "mlp_b1", "mlp_w2", "mlp_b2", "cf_lin1", "cf_lin2",
        "cf_lin2_b", "lin_w", "lin_b", "bond_emb", "gin_eps", "gin_w1",
        "gin_w2", "bn1_g", "bn1_b", "bn2_g", "bn2_b")}
    const = dict(
        w1=w["mlp_w1"],                                   # [NG, NF]
        b1col=w["mlp_b1"].reshape(D.NF, 1),
        w2=w["mlp_w2"],                                   # [NF, NF]
        b2row=w["mlp_b2"].reshape(1, D.NF),
        b2row3=np.tile(w["mlp_b2"].reshape(1, D.NF), (1, 3)),
        ones1=np.ones((1, 128), np.float32),
        lin1=np.ascontiguousarray(w["cf_lin1"].reshape(H2, 128, D.NF)),
        lin2=w["cf_lin2"],                                # [NF, H]
        lin2b=w["cf_lin2_b"].reshape(H2, 128, 1),
        linw=np.ascontiguousarray(w["lin_w"].reshape(H2, 128, D.H)),
        linb=w["lin_b"].reshape(H2, 128, 1),
        gw1=np.ascontiguousarray(w["gin_w1"].reshape(H2, 128, D.H)),
        gw2=np.ascontiguousarray(w["gin_w2"].reshape(H2, 128, D.H)),
        bondcat=np.ascontiguousarray(
            w["bond_emb"].reshape(3 * D.VOCAB, D.H)),
        bn1g=w["bn1_g"].reshape(H2, 128, 1), bn1b=w["bn1_b"].reshape(H2, 128, 1),
        bn2g=w["bn2_g"].reshape(H2, 128, 1), bn2b=w["bn2_b"].reshape(H2, 128, 1),
        epsv=np.full((128, 1), 1.0 + float(w["gin_eps"]), np.float32),
        zerocol=np.zeros((128, 1), np.float32),
        eps5col=np.full((128, 1), 1e-5, np.float32),
        pihalf=np.full((128, 1), -math.pi / 2, np.float32),
        iota=np.tile(np.arange(128, dtype=np.float32), (128, 1)).copy(),
        iota6=np.tile(np.arange(128, dtype=np.float32), (128, 6)).copy(),
        ident=np.eye(128, dtype=np.float32),
    )

    in_maps = []
    for c in range(D.cores):
        m = dict(
            xT=np.ascontiguousarray(XT[:, :, c * NS:(c + 1) * NS]),
            AT=AT[c], WT=WT[c], SRC=SRC[c], DRELC=DRELC[c],
            SG=SG[c], DREL=DREL[c], BHOT=BHOT[c],
        )
        m.update(const)
        in_maps.append(m)
    return in_maps


def assemble(results, D: Dims):
    """Per-core outT [2,128,NS] -> full [N, H]."""
    parts = [r["outT"].reshape(D.H, D.NS) for r in results]
    outT = np.concatenate(parts, axis=1)  # [H, N]
    return np.ascontiguousarray(outT.T)


def _ts(i, n):
    return bass.ts(i, n)


def build_nc(D: Dims, flags: frozenset = frozenset()):
    nc = bacc.Bacc("TRN2", target_bir_lowering=False, debug=False,
                   num_devices=D.cores, num_swdge_queues=3)
    NS, GS, H, NF, NG = D.NS, D.GS, D.H, D.NF, D.NG
    H2 = H // 128

    I = {}
    def di(name, shape, dt=F32):
        I[name] = nc.dram_tensor(name, list(shape), dt, kind="ExternalInput")
        return I[name]

    di("xT", [2, 128, NS])
    di("AT", [NG, D.E_pad])
    di("WT", [128, D.E_pad // 128])
    di("SRC", [128, D.E_pad // 16], I16)
    di("DRELC", [128, D.E_pad // 128])
    di("SG", [128, D.EG_pad // 16], I16)
    di("DREL", [128, D.EG_pad // 128])
    di("BHOT", [3 * D.VOCAB, D.EG_pad])
    di("w1", [NG, NF]); di("b1col", [NF, 1]); di("w2", [NF, NF])
    di("b2row", [1, NF]); di("b2row3", [1, 3 * NF]); di("ones1", [1, 128])
    di("lin1", [H2, 128, NF]); di("lin2", [NF, H]); di("lin2b", [H2, 128, 1])
    di("linw", [H2, 128, H]); di("linb", [H2, 128, 1])
    di("gw1", [H2, 128, H]); di("gw2", [H2, 128, H])
    di("bondcat", [3 * D.VOCAB, H])
    di("bn1g", [H2, 128, 1]); di("bn1b", [H2, 128, 1])
    di("bn2g", [H2, 128, 1]); di("bn2b", [H2, 128, 1])
    di("epsv", [128, 1]); di("iota", [128, 128]); di("iota6", [128, 768])
    di("ident", [128, 128])
    di("zerocol", [128, 1]); di("eps5col", [128, 1]); di("pihalf", [128, 1])

    outT = nc.dram_tensor("outT", [2, 128, NS], F32, kind="ExternalOutput")

    groups = [list(range(D.cores))]

    with tile.TileContext(nc) as tc:
        with (
            tc.tile_pool(name="const", bufs=1) as cp,
            tc.tile_pool(name="work", bufs=2) as wp,
            tc.tile_pool(name="small", bufs=3) as sp,
            tc.tile_pool(name="gin", bufs=2) as ctx_gin_pool,
            tc.tile_pool(name="psum", bufs=2, space="PSUM") as pp,
            tc.tile_pool(name="dram", bufs=1, space="DRAM") as dp,
        ):
            # ---------- load constants ----------
            C = {}
            for nm, shp in [("w1", [NG, NF]), ("b1col", [NF, 1]),
                            ("w2", [NF, NF]), ("b2row", [1, NF]),
                            ("b2row3", [1, 3 * NF]),
                            ("ones1", [1, 128]), ("lin2", [NF, H]),
                            ("bondcat", [3 * D.VOCAB, H]),
                            ("epsv", [128, 1]), ("iota", [128, 128]),
                            ("iota6", [128, 768]),
                            ("ident", [128, 128]), ("zerocol", [128, 1]),
                            ("eps5col", [128, 1]), ("pihalf", [128, 1])]:
                t = cp.tile(shp, F32, name=f"c_{nm}")
                nc.sync.dma_start(t[:], I[nm].ap())
                C[nm] = t
            nc.const_aps.aps[(F32, 0.0)] = C["zerocol"][:]
            # [H2,128,*] constants: load as per-half tiles
            for nm in ("lin1", "lin2b", "linw", "linb", "gw1", "gw2",
                       "bn1g", "bn1b", "bn2g", "bn2b"):
                C[nm] = []
                inner = I[nm].shape[2]
                for k in range(H2):
                    t = cp.tile([128, inner], F32, name=f"c_{nm}{k}")
                    nc.sync.dma_start(t[:], I[nm].ap()[k])
                    C[nm].append(t)

            # ---------- DRAM scratch ----------
            # xf shard/full split in half so the AllGather pipelines with
            # compute: AG#h gathers every core's half-h shard; the gathered
            # layout is permuted (half-major), host_prep permutes src indices
            HSH = NS // 2
            xf_shard = [dp.tile([HSH, NF], F32, name=f"xf_shard{h}")
                        for h in range(2)]
            xf_full = [dp.tile([D.cores * HSH, NF], F32, name=f"xf_full{h}",
                               addr_space="Shared") for h in range(2)]
            xagg_shard = dp.tile([GS, H], F32, name="xagg_shard")
            xagg_full = dp.tile([D.cores * GS, H], F32, name="xagg_full",
                                addr_space="Shared")
            agg_ab = [dp.tile([NS, NF], F32, name=f"agg_{h}")
                      for h in range(2)]
            st1_in = dp.tile([128, 4], F32, name="st1_in")
            st1_out = dp.tile([128, 4], F32, name="st1_out", addr_space="Shared")
            st2_in = dp.tile([128, 4], F32, name="st2_in")
            st2_out = dp.tile([128, 4], F32, name="st2_out", addr_space="Shared")

            # =========== Phase A: segment-max pool first, then xf ===========
            # pass 1 pools so the (small) xagg AllGather is issued as early
            # as possible; pass 2 re-reads x and computes xf, issuing each
            # half's AllGather as soon as that half of the shard is written
            rep = D.N // D.G
            PCH = 250
            n_pch = NS // PCH
            half_chunks = n_pch // 2
            xaggT = [cp.tile([128, GS], F32, name=f"xaggT{k}") for k in range(2)]
            for j in range(n_pch):
                xt = [wp.tile([128, PCH], F32, tag=f"ph_a_xt{k}", name=f"ph_a_xt{k}")
                      for k in range(2)]
                for k in range(2):
                    nc.sync.dma_start(xt[k][:], I["xT"].ap()[k, :, _ts(j, PCH)])
                # pool: max over groups of 10 cols
                for k in range(2):
                    nc.vector.tensor_reduce(
                        xaggT[k][:, _ts(j, PCH // rep)],
                        xt[k][:].rearrange("p (g t) -> p g t", t=rep),
                        AX.X, ALU.max)

            # transpose x_aggT -> node-major x_agg shard
            GT = (GS + 127) // 128
            for t in range(GT):
                m = min(128, GS - t * 128)
                for k in range(2):
                    pst = pp.tile([128, 128], F32, tag="ps_tr", name="ps_tr")
                    nc.tensor.transpose(pst[:m, :], xaggT[k][:, t * 128:t * 128 + m],
                                        C["ident"][:])
                    sb = sp.tile([128, 128], F32, tag="ph_a_trsb", name="ph_a_trsb")
                    nc.scalar.copy(sb[:m, :], pst[:m, :])
                    nc.sync.dma_start(
                        xagg_shard[t * 128:t * 128 + m, _ts(k, 128)], sb[:m, :])

            if "no_coll" not in flags and "no_coll_xagg" not in flags:
                nc.gpsimd.collective_compute(
                    "AllGather", ALU.bypass, replica_groups=groups,
                    ins=[xagg_shard.opt()], outs=[xagg_full.opt()])

            # pass 2: xf = x @ cf_lin1, node-major tiles of <=128
            for j in range(n_pch):
                xt = [wp.tile([128, PCH], F32, tag=f"ph_a_xt{k}", name=f"ph_a2_xt{k}")
                      for k in range(2)]
                for k in range(2):
                    nc.sync.dma_start(xt[k][:], I["xT"].ap()[k, :, _ts(j, PCH)])
                h = j // half_chunks
                r0 = (j - h * half_chunks) * PCH
                nt = (PCH + 127) // 128
                for t in range(nt):
                    m = min(128, PCH - t * 128)
                    ps = pp.tile([128, NF], F32, tag="ps_mm", name="ps_mm")
                    for k in range(2):
                        nc.tensor.matmul(ps[:m, :], xt[k][:, t * 128:t * 128 + m],
                                         C["lin1"][k][:], start=(k == 0),
                                         stop=(k == 1))
                    sb = sp.tile([128, NF], F32, tag="ph_a_sb", name="ph_a_sb")
                    nc.scalar.copy(sb[:m, :], ps[:m, :])
                    nc.sync.dma_start(
                        xf_shard[h][r0 + t * 128: r0 + t * 128 + m, :],
                        sb[:m, :])
                if (j == half_chunks - 1 and "no_coll" not in flags
                        and "no_coll_xf" not in flags):
                    nc.gpsimd.collective_compute(
                        "AllGather", ALU.bypass, replica_groups=groups,
                        ins=[xf_shard[0].opt()], outs=[xf_full[0].opt()])
            # AG#1 is issued later (after st1's AllReduce) so the GIN stats
            # reduction isn't queued behind it on the collective engine

            # =========== Phase B: GIN branch (sharded by graph node) =========
            # gather x_agg[sg], edge_emb via bond one-hot matmul, relu,
            # one-hot scatter into agg_g windows
            sgidx = cp.tile([128, D.EG_pad // 16], I16, name="sgidx_sb")
            nc.sync.dma_start(sgidx[:], I["SG"].ap())
            drel = cp.tile([128, D.EG_pad // 128], F32, name="drel_sb")
            nc.sync.dma_start(drel[:], I["DREL"].ap())

            # t-buffer (node-major (1+eps)x_agg + agg_g), then transposed halves
            gp = ctx_gin_pool
            tT = [gp.tile([128, GS], F32, tag=f"ginbuf{k}", name=f"tT{k}")
                  for k in range(2)]

            assert D.EG_pad % 128 == 0
            tiles_per_win = D.PW // 128
            for w in range(0 if "no_b" in flags else D.NWIN):
                m = min(D.gwin, GS - w * D.gwin)
                # gather this window's source rows
                gath_g = wp.tile([128, tiles_per_win, H], F32,
                                 tag="ph_b_gath", name="ph_b_gath")
                nc.gpsimd.dma_gather(
                    gath_g[:], xagg_full[:],
                    sgidx[:, w * D.PW // 16:(w + 1) * D.PW // 16],
                    num_idxs=D.PW, num_idxs_reg=D.PW, elem_size=H)
                bhot = wp.tile([3 * D.VOCAB, D.PW], F32, tag="ph_b_bhot",
                               name="ph_b_bhot")
                nc.sync.dma_start(bhot[:],
                                  I["BHOT"].ap()[:, w * D.PW:(w + 1) * D.PW])
                ps_agg = pp.tile([128, H], F32, tag="ps_agg", name="ps_agg")
                for i in range(tiles_per_win):
                    t = w * tiles_per_win + i
                    # edge embedding: one-hot bond matmul (K=15)
                    ps_emb = pp.tile([128, H], F32, tag="ps_mm", name="ps_mm")
                    nc.tensor.matmul(ps_emb[:], bhot[:, _ts(i, 128)],
                                     C["bondcat"][:], start=True, stop=True)
                    # msg = relu(gathered + emb)
                    msg = sp.tile([128, H], F32, tag="ph_b_msg", name="ph_b_msg")
                    nc.vector.tensor_tensor(msg[:], gath_g[:, i, :], ps_emb[:],
                                            ALU.add)
                    nc.scalar.activation(msg[:], msg[:], ACTF.Relu)
                    # one-hot scatter
                    oh = sp.tile([128, D.gwin], F32, tag="ph_b_oh", name="ph_b_oh")
                    nc.vector.tensor_scalar(oh[:], C["iota"][:, :D.gwin],
                                            drel[:, t:t + 1], None, ALU.is_equal)
                    nc.tensor.matmul(ps_agg[:m, :], oh[:, :m], msg[:],
                                     start=(i == 0), stop=(i == tiles_per_win - 1))
                # t = (1+eps) * x_agg + agg_g  (node-major window rows)
                xa = sp.tile([128, H], F32, tag="ph_b_xa", name="ph_b_xa")
                nc.sync.dma_start(
                    xa[:m, :], xagg_shard[w * D.gwin:w * D.gwin + m, :])
                tn = sp.tile([128, H], F32, tag="ph_b_tn", name="ph_b_tn")
                nc.vector.tensor_scalar(tn[:m, :], xa[:m, :], C["epsv"][:m, :],
                                        None, ALU.mult)
                nc.vector.tensor_tensor(tn[:m, :], tn[:m, :], ps_agg[:m, :],
                                        ALU.add)
                # transpose to feature-major tT
                for k in range(2):
                    pst = pp.tile([128, 128], F32, tag="ps_tr", name="ps_tr")
                    nc.tensor.transpose(pst[:, :m], tn[:m, _ts(k, 128)],
                                        C["ident"][:m, :m])
                    nc.vector.tensor_copy(tT[k][:, w * D.gwin:w * D.gwin + m],
                                          pst[:, :m])

            def gin_mm_and_stats_issue(inT, Wc, uT, stats_in, stats_out, label):
                """u = in @ W (node-major tiles), transpose to uT, stats;
                issues the stats AllReduce but does NOT read the result."""
                for t in range(GT):
                    m = min(128, GS - t * 128)
                    ps = pp.tile([128, H], F32, tag="ps_mm", name="ps_mm")
                    for k in range(2):
                        nc.tensor.matmul(ps[:m, :],
                                         inT[k][:, t * 128:t * 128 + m],
                                         Wc[k][:], start=(k == 0), stop=(k == 1))
                    sb = sp.tile([128, H], F32, tag=f"{label}_sb", name=f"{label}_sb")
                    nc.scalar.copy(sb[:m, :], ps[:m, :])
                    for k in range(2):
                        pst = pp.tile([128, 128], F32, tag="ps_tr", name="ps_tr")
                        nc.tensor.transpose(pst[:, :m], sb[:m, _ts(k, 128)],
                                            C["ident"][:m, :m])
                        nc.vector.tensor_copy(uT[k][:, t * 128:t * 128 + m],
                                              pst[:, :m])
                st = sp.tile([128, 4], F32, tag=f"{label}_st", name=f"{label}_st")
                sq = sp.tile([128, GS], F32, tag="gin_sq", name="gin_sq",
                             bufs=1)
                for k in range(2):
                    nc.vector.tensor_reduce(st[:, 2 * k:2 * k + 1], uT[k][:],
                                            AX.X, ALU.add)
                    nc.vector.tensor_tensor(sq[:], uT[k][:], uT[k][:], ALU.mult)
                    nc.vector.tensor_reduce(st[:, 2 * k + 1:2 * k + 2], sq[:],
                                            AX.X, ALU.add)
                nc.sync.dma_start(stats_in[:], st[:])
                if "no_b_ar" not in flags:
                    nc.gpsimd.collective_compute(
                        "AllReduce", ALU.add, replica_groups=groups,
                        ins=[stats_in.opt()], outs=[stats_out.opt()])

            def gin_read_stf(stats_in, stats_out, label):
                stf = sp.tile([128, 4], F32, tag=f"{label}_stf", name=f"{label}_stf")
                nc.sync.dma_start(
                    stf[:], stats_in[:] if "no_b_ar" in flags else stats_out[:])
                return stf

            def bn_apply(stf, uT, g_c, b_c, outT_t, relu, label):
                """out = func((u - mu) * g / sqrt(var+eps) + b), feature-major."""
                inv_n = 1.0 / float(D.G)
                for k in range(2):
                    mu = sp.tile([128, 1], F32, tag=f"{label}_mu{k}", name=f"{label}_mu{k}")
                    nc.vector.tensor_scalar(mu[:], stf[:, 2 * k:2 * k + 1],
                                            inv_n, None, ALU.mult)
                    var = sp.tile([128, 1], F32, tag=f"{label}_va{k}", name=f"{label}_va{k}")
                    nc.vector.tensor_scalar(var[:], stf[:, 2 * k + 1:2 * k + 2],
                                            inv_n, None, ALU.mult)
                    mu2 = sp.tile([128, 1], F32, tag=f"{label}_m2{k}", name=f"{label}_m2{k}")
                    nc.vector.tensor_tensor(mu2[:], mu[:], mu[:], ALU.mult)
                    nc.vector.tensor_tensor(var[:], var[:], mu2[:], ALU.subtract)
                    sd = sp.tile([128, 1], F32, tag=f"{label}_sd{k}", name=f"{label}_sd{k}")
                    nc.scalar.activation(sd[:], var[:], ACTF.Sqrt,
                                         bias=C["eps5col"][:])
                    rs = sp.tile([128, 1], F32, tag=f"{label}_rs{k}", name=f"{label}_rs{k}")
                    nc.vector.reciprocal(rs[:], sd[:])
                    sc = sp.tile([128, 1], F32, tag=f"{label}_sc{k}", name=f"{label}_sc{k}")
                    nc.vector.tensor_tensor(sc[:], g_c[k][:], rs[:], ALU.mult)
                    sh = sp.tile([128, 1], F32, tag=f"{label}_sh{k}", name=f"{label}_sh{k}")
                    nc.vector.tensor_tensor(sh[:], mu[:], sc[:], ALU.mult)
                    nc.vector.tensor_tensor(sh[:], b_c[k][:], sh[:], ALU.subtract)
                    nc.scalar.activation(outT_t[k][:], uT[k][:],
                                         ACTF.Relu if relu else ACTF.Identity,
                                         bias=sh[:], scale=sc[:])

            def gin_buf(nm):
                return [gp.tile([128, GS], F32, tag=f"ginbuf{k}",
                                name=f"{nm}{k}") for k in range(2)]

            bstate = {}
            if "no_b" in flags:
                ginT = gin_buf("ginT")
                for k in range(2):
                    nc.vector.memset(ginT[k][:], 0.0)
                bstate["ginT"] = ginT
            else:
                uT = gin_buf("uT")
                gin_mm_and_stats_issue(tT, C["gw1"], uT, st1_in, st1_out,
                                       "gmm1")

            # xf AG#1 queued on the collective engine AFTER st1's AllReduce
            # so the (tiny) stats reduce isn't stuck behind the 25MB gather
            if "no_coll" not in flags and "no_coll_xf" not in flags:
                nc.gpsimd.collective_compute(
                    "AllGather", ALU.bypass, replica_groups=groups,
                    ins=[xf_shard[1].opt()], outs=[xf_full[1].opt()])

            def emit_b2():
                """bn1 + gmm2 + st2 AllReduce issue (mid phase C)."""
                if "no_b" in flags or "t2T" in bstate:
                    return
                stf1 = gin_read_stf(st1_in, st1_out, "gmm1")
                t1T = gin_buf("t1T")
                bn_apply(stf1, uT, C["bn1g"], C["bn1b"], t1T, True, "bn1")
                t2T = gin_buf("t2T")
                gin_mm_and_stats_issue(t1T, C["gw2"], t2T, st2_in, st2_out,
                                       "gmm2")
                bstate["t2T"] = t2T

            def emit_b3():
                """bn2 -> ginT (late in phase C)."""
                if "no_b" in flags or "ginT" in bstate:
                    return
                stf2 = gin_read_stf(st2_in, st2_out, "gmm2")
                ginT = gin_buf("ginT")
                bn_apply(stf2, bstate["t2T"], C["bn2g"], C["bn2b"], ginT,
                         False, "bn2")
                bstate["ginT"] = ginT

            # =========== Phase C: conformer edge pipeline ===========
            # window-major: per (src-half stream h, dst-window w of 128
            # nodes): gather the window's edges (one call per src quad
            # bucket of B_E), compute msg = (xf[src]) * (mlp(A) + b2), then
            # aggregate over dst via one-hot matmuls into PSUM (the one-hot
            # rows carry the cosine-cutoff C so no separate C-multiply),
            # and flush the window's 128 agg rows with a plain DMA write.
            # No scatter-add: each agg row is written exactly once.
            # resident: C row (cosine cutoff) and dst-rel row per edge
            crow = cp.tile([128, D.E_pad // 128], F32, name="crow_sb")
            for s0 in range(0, D.E_pad // 128, 512):
                sw = min(512, D.E_pad // 128 - s0)
                wt = wp.tile([128, 512], F32, tag="ph_c_wt", name="ph_c_wt")
                nc.sync.dma_start(wt[:, :sw], I["WT"].ap()[:, s0:s0 + sw])
                nc.scalar.activation(wt[:, :sw], wt[:, :sw], ACTF.Sin,
                                     bias=C["pihalf"][:],
                                     scale=math.pi / D.CUTOFF)
                nc.scalar.activation(crow[:, s0:s0 + sw], wt[:, :sw],
                                     ACTF.Copy, bias=0.5, scale=-0.5)
            drelc = cp.tile([128, D.E_pad // 128], F32, name="drelc_sb")
            nc.sync.dma_start(drelc[:], I["DRELC"].ap())

            B_E, CW, EH = D.B_E, D.CW, D.EH
            WE = 2 * B_E           # edges per (stream, window)
            SW = 2 * WE            # edges per super-window (pair of windows)
            NTS = SW // 128        # tiles per super-window (12)
            assert CW % 2 == 0
            wstream = ([] if "no_c" in flags else
                       [(h, w) for h in range(2) for w in range(0, CW, 2)])
            PFD = 3  # gather prefetch depth (super-windows issued ahead)
            gat_fifo = []

            def emit_gather(idx):
                h, w = wstream[idx]
                e0 = h * EH + w * WE
                if "no_gather" in flags:
                    if "no_cmm" in flags:
                        return None
                    gat = wp.tile([128, NTS, NF], F32, tag="ph_c_gat",
                                  name="ph_c_gat", bufs=PFD + 2)
                    nc.vector.memset(gat[:], 0.0)
                    return gat
                gat = wp.tile([128, NTS, NF], F32, tag="ph_c_gat",
                              name="ph_c_gat", bufs=PFD + 2)
                si = wp.tile([128, SW // 16], I16, tag="ph_c_si",
                             name="ph_c_si", bufs=PFD + 2)
                nc.sync.dma_start(
                    si[:], I["SRC"].ap()[:, e0 // 16:(e0 + SW) // 16])
                for b in range(4):
                    j = b % 2
                    nc.gpsimd.dma_gather(
                        gat[:, b * 3:(b + 1) * 3, :],
                        xf_full[h][j * D.qsize:(j + 1) * D.qsize, :],
                        si[:, b * B_E // 16:(b + 1) * B_E // 16],
                        num_idxs=B_E, num_idxs_reg=B_E, elem_size=NF,
                        queue_num=(4 * idx + b) % 3)
                return gat

            def emit_compute_flush(idx, gat):
                h, w = wstream[idx]
                e0 = h * EH + w * WE
                c0col = e0 // 128
                msg = None
                if "no_cmm" not in flags:
                    msg = wp.tile([128, NTS, NF], F32, tag="ph_c_msg",
                                  name="ph_c_msg")
                    at = wp.tile([NG, SW], F32, tag="ph_c_at",
                                 name="ph_c_at")
                    nc.sync.dma_start(at[:], I["AT"].ap()[:, e0:e0 + SW])
                    # per 512-edge group: mm1+relu, mm2 (+b2) packed 4 tiles
                    # into one PSUM bank; 512-wide msg-mul
                    for g in range(3):
                        s0 = g * 512
                        ps1 = pp.tile([128, 512], F32, tag="ps_mm", name="ps_mm")
                        nc.tensor.matmul(ps1[:], C["w1"][:],
                                         at[:, s0:s0 + 512],
                                         start=True, stop=True)
                        h1 = wp.tile([128, 512], F32, tag="ph_c_h1",
                                     name="ph_c_h1")
                        nc.scalar.activation(h1[:], ps1[:],
                                             ACTF.Relu, bias=C["b1col"][:])
                        psw = pp.tile([128, 4, NF], F32, tag="ps_w", name="ps_w")
                        for t4 in range(4):
                            nc.tensor.matmul(psw[:, t4, :], h1[:, _ts(t4, 128)],
                                             C["w2"][:], start=True,
                                             stop="b2zero" in flags)
                            if "b2zero" not in flags:
                                nc.tensor.matmul(psw[:, t4, :], C["ones1"][:],
                                                 C["b2row"][:], start=False,
                                                 stop=True)
                        nc.vector.tensor_tensor(msg[:, 4 * g:4 * g + 4, :],
                                                gat[:, 4 * g:4 * g + 4, :],
                                                psw[:], ALU.mult)
                if "no_scatter" in flags or "no_cmm" in flags:
                    return
                # one-hot aggregation: rows carry C; accumulate over tiles,
                # one PSUM half-bank per window of the pair
                ohc = sp.tile([128, NTS, 128], F32, tag="ph_c_oh",
                              name="ph_c_oh", bufs=2)
                for p2 in range(2):
                    view = ohc[:, p2 * 6:(p2 + 1) * 6, :]
                    cc = c0col + p2 * 6
                    nc.vector.tensor_tensor(
                        view,
                        C["iota6"][:].rearrange("p (t j) -> p t j", j=128),
                        drelc[:, cc:cc + 6].broadcast_to((128, 6, 128)),
                        ALU.is_equal)
                    nc.vector.tensor_tensor(
                        view, view,
                        crow[:, cc:cc + 6].broadcast_to((128, 6, 128)),
                        ALU.mult)
                psA = pp.tile([128, 2, NF], F32, tag="ps_agg", name="ps_cagg")
                for t in range(NTS):
                    nc.tensor.matmul(psA[:, t // 6, :], ohc[:, t, :],
                                     msg[:, t, :],
                                     start=(t % 6 == 0), stop=(t % 6 == 5))
                for p2 in range(2):
                    base = (w + p2) * 128
                    m = min(128, NS - base)
                    stg = sp.tile([128, NF], F32, tag="ph_c_stg",
                                  name="ph_c_stg", bufs=2)
                    nc.scalar.copy(stg[:m, :], psA[:m, p2, :])
                    nc.sync.dma_start(
                        agg_ab[h][base:base + m, :].rearrange(
                            "(t p) f -> p t f", p=m),
                        stg[:m, :].rearrange("p (t f) -> p t f", f=NF))

            # =========== Phase D: h = relu(agg@lin2+b)@linw+b, residual =====
            # emitted as a closure so node chunks interleave into the tail
            # of the window stream (chunk j only needs agg rows already
            # flushed by both streams, plus ginT)
            NCH = D.nchunk
            n_nch = NS // NCH

            def emit_d(j):
                r0 = j * NCH
                # load agg rows, transpose to feature-major aggT [NF, NCH]
                aggT = wp.tile([NF, NCH], F32, tag="ph_d_aggT", name="ph_d_aggT")
                ntt = (NCH + 127) // 128
                for t in range(ntt):
                    m = min(128, NCH - t * 128)
                    asb = sp.tile([128, NF], F32, tag="ph_d_asb", name="ph_d_asb")
                    nc.sync.dma_start(asb[:m, :],
                                      agg_ab[0][r0 + t * 128:r0 + t * 128 + m, :])
                    bsb = sp.tile([128, NF], F32, tag="ph_d_bsb", name="ph_d_bsb")
                    nc.sync.dma_start(bsb[:m, :],
                                      agg_ab[1][r0 + t * 128:r0 + t * 128 + m, :])
                    nc.vector.tensor_tensor(asb[:m, :], asb[:m, :], bsb[:m, :],
                                            ALU.add)
                    pst = pp.tile([128, 128], F32, tag="ps_tr", name="ps_tr")
                    nc.tensor.transpose(pst[:, :m], asb[:m, :], C["ident"][:m, :m])
                    nc.vector.tensor_copy(aggT[:, t * 128:t * 128 + m],
                                          pst[:, :m])
                # h1T = relu(lin2^T @ aggT + b)  [2][128, NCH]
                h1T = [wp.tile([128, NCH], F32, tag=f"ph_d_h1T{k}", name=f"ph_d_h1T{k}")
                       for k in range(2)]
                for k in range(2):
                    ps = pp.tile([128, NCH], F32, tag="ps_mm", name="ps_mm")
                    nc.tensor.matmul(ps[:], C["lin2"][:, _ts(k, 128)], aggT[:],
                                     start=True, stop=True)
                    nc.scalar.activation(h1T[k][:], ps[:], ACTF.Relu,
                                         bias=C["lin2b"][k][:])
                # outT = linw^T @ h1T + linb + xT + gin[batch]
                for k in range(2):
                    ps = pp.tile([128, NCH], F32, tag="ps_mm", name="ps_mm")
                    for kk in range(2):
                        nc.tensor.matmul(ps[:], C["linw"][kk][:, _ts(k, 128)],
                                         h1T[kk][:], start=(kk == 0),
                                         stop=(kk == 1))
                    ob = sp.tile([128, NCH], F32, tag="ph_d_ob", name="ph_d_ob")
                    nc.scalar.activation(ob[:], ps[:], ACTF.Identity,
                                         bias=C["linb"][k][:])
                    xtc = sp.tile([128, NCH], F32, tag="ph_d_xtc", name="ph_d_xtc")
                    nc.sync.dma_start(xtc[:], I["xT"].ap()[k, :, r0:r0 + NCH])
                    nc.vector.tensor_tensor(ob[:], ob[:], xtc[:], ALU.add)
                    # + gin, each graph col repeated `rep` times
                    rep = D.N // D.G
                    g0 = r0 // rep
                    gin_rep = bstate["ginT"][k][:, g0:g0 + NCH // rep] \
                        .broadcast_to((128, NCH // rep, rep))
                    nc.vector.tensor_tensor(
                        ob[:].rearrange("p (g t) -> p g t", t=rep),
                        ob[:].rearrange("p (g t) -> p g t", t=rep),
                        gin_rep, ALU.add)
                    nc.sync.dma_start(outT.ap()[k, :, r0:r0 + NCH], ob[:])

            next_d = [0]

            def emit_d_ready(done):
                """Emit phase D chunks whose agg rows are fully flushed."""
                if "no_d" in flags or "ginT" not in bstate:
                    return
                while (next_d[0] < n_nch
                       and (done - 48) * 256 >= NCH * (next_d[0] + 1)):
                    emit_d(next_d[0])
                    next_d[0] += 1

            for i in range(len(wstream) + PFD):
                if i < len(wstream):
                    gat_fifo.append(emit_gather(i))
                if i >= PFD:
                    done = i - PFD
                    emit_compute_flush(done, gat_fifo[done])
                    # GIN stage hooks: latency of the stats AllReduces and
                    # the serial BN chains hides under the window stream
                    if done == 12:
                        emit_b2()
                    if done == 61:
                        emit_b3()
                    if done > 61:
                        emit_d_ready(done)
            emit_b2()  # no-op unless phase C was skipped
            emit_b3()
            if "no_d" not in flags:
                for j in range(next_d[0], n_nch):
                    emit_d(j)

    nc.compile()
    return nc


_CACHE = {}


def _get_nc(D: Dims, flags: frozenset = frozenset()):
    key = ("nc", D, flags)
    if key not in _CACHE:
        _CACHE[key] = build_nc(D, flags)
    return _CACHE[key]


def run_on_hw(inputs, D: Dims = REAL):
    flags = (frozenset({"b2zero"})
             if not np.any(np.asarray(inputs["mlp_b2"])) else frozenset())
    nc = _get_nc(D, flags)
    in_maps = host_prep(inputs, D)
    res = bass_utils.run_bass_kernel_spmd(nc, in_maps,
                                          core_ids=list(range(D.cores)))
    return assemble(res.results, D)


def kernel(**inputs):
    return run_on_hw(inputs, REAL)



# revision 61
# speedup vs baseline: 2.1553x; 1.0090x over previous
"""Trainium2 Bass kernel for nn_DSSConf (DSS conformer GNN message passing).

Self-contained: hardcodes shapes/sharding for the real problem; exposes
kernel(**inputs) -> np.ndarray.
"""
import sys
import math
from dataclasses import dataclass

sys.path.insert(0, "/opt/trn_rl_repo")

import numpy as np
from concourse import bass, bacc, tile, mybir, bass_utils

F32 = mybir.dt.float32
I16 = mybir.dt.int16
ALU = mybir.AluOpType
ACTF = mybir.ActivationFunctionType
AX = mybir.AxisListType


@dataclass(frozen=True)
class Dims:
    N: int = 100000        # conformer nodes
    H: int = 256           # hidden
    NF: int = 128          # num filters
    NG: int = 50           # num gaussians
    G: int = 10000         # graph nodes
    E: int = 1000000       # conformer edges
    EG: int = 30000        # graph edges
    VOCAB: int = 5
    CUTOFF: float = 10.0
    cores: int = 8
    qsize: int = 25000     # src quadrant size for int16 gather indices
    B_E: int = 384         # fixed edges per (dst-window, src-quad) bucket
    gwin: int = 128        # GIN scatter window (<=128 segments)
    PW: int = 640          # padded GIN edges per (core, window) (multiple of 128)
    nchunk: int = 500      # node chunk for the h/out stage (divides NS, mult of 10)

    @property
    def NS(self):
        return self.N // self.cores

    @property
    def GS(self):
        return self.G // self.cores

    @property
    def CW(self):
        """dst windows of 128 nodes per core shard."""
        return (self.NS + 127) // 128

    @property
    def EH(self):
        """padded edges per src-half stream (2 quad buckets per window)."""
        return self.CW * 2 * self.B_E

    @property
    def E_pad(self):
        return 2 * self.EH

    @property
    def NWIN(self):
        return (self.GS + self.gwin - 1) // self.gwin

    @property
    def EG_pad(self):
        return self.NWIN * self.PW


REAL = Dims()


def _wrap16(arr, dtype=np.int16):
    """Edge i -> [i % 16, i // 16], replicated to 128 partitions."""
    a = np.asarray(arr).reshape(-1, 16).T.astype(dtype)  # [16, n/16]
    return np.tile(a, (8, 1)).copy()  # [128, n/16]


def _tile128(arr, dtype=np.float32):
    """Edge i -> [i % 128, i // 128] (per-partition scalar layout)."""
    return np.ascontiguousarray(np.asarray(arr).reshape(-1, 128).T.astype(dtype))


def host_prep(inputs, D: Dims):
    """Build per-core in_maps (list of dicts) for the SPMD kernel."""
    x = np.asarray(inputs["x"], np.float32)
    cnb = np.asarray(inputs["conf_node_batch"]).astype(np.int64)
    ei = np.asarray(inputs["edge_index_conf"]).astype(np.int64)
    ew = np.asarray(inputs["edge_weight_conf"], np.float32)
    ea = np.asarray(inputs["edge_attr_conf"], np.float32)
    eig = np.asarray(inputs["edge_index_graph"]).astype(np.int64)
    eag = np.asarray(inputs["edge_attr_graph"]).astype(np.int64)

    rep = D.N // D.G
    assert np.array_equal(cnb, np.repeat(np.arange(D.G), rep)), \
        "conf_node_batch structure mismatch"

    NS, GS = D.NS, D.GS
    src, dst = ei[0], ei[1]

    # ---- conformer edges: window-major one-hot layout ----
    # per core: two streams by src half (matching the split AllGather's
    # permuted xf layout); within a stream, buckets of fixed B_E edges per
    # (dst-window of 128, src quad within the half); in-quad src offset =
    # (src_core % 4)*NS/2 + (src % NS/2)
    B_E, CW, EH = D.B_E, D.CW, D.EH
    core = dst // NS
    sc = src // NS
    so = src % NS
    sh = so // (NS // 2)
    sj = (sc >= 4).astype(np.int64)
    inq = (sc % 4) * (NS // 2) + (so % (NS // 2))
    win = (dst % NS) // 128
    bucket = ((core * 2 + sh) * CW + win) * 2 + sj
    order = np.argsort(bucket, kind="stable")
    sb = bucket[order]
    nbuck = D.cores * 2 * CW * 2
    bounds = np.searchsorted(sb, np.arange(nbuck + 1))
    counts = bounds[1:] - bounds[:-1]
    assert counts.max() <= B_E, f"bucket overflow: {counts.max()} > {B_E}"
    rank = np.arange(len(sb)) - bounds[sb]
    c_b = sb // (2 * CW * 2)
    rem = sb % (2 * CW * 2)
    h_b = rem // (CW * 2)
    w_b = (rem % (CW * 2)) // 2
    j_b = rem % 2
    s_b, p_b = w_b // 2, w_b % 2
    flat = (c_b * D.E_pad + h_b * EH
            + (s_b * 4 + j_b * 2 + p_b) * B_E + rank)

    src_pad = np.zeros(D.cores * D.E_pad, np.int64)
    drel_pad = np.full(D.cores * D.E_pad, -1.0, np.float32)
    w_pad = np.full(D.cores * D.E_pad, D.CUTOFF, np.float32)  # C(CUTOFF)=0
    a_pad = np.zeros((D.cores * D.E_pad, D.NG), np.float32)
    src_pad[flat] = inq[order]
    drel_pad[flat] = ((dst[order] % NS) - w_b * 128).astype(np.float32)
    w_pad[flat] = ew[order]
    a_pad[flat] = ea[order]

    AT = np.zeros((D.cores, D.NG, D.E_pad), np.float32)
    WT = np.zeros((D.cores, 128, D.E_pad // 128), np.float32)
    DRELC = np.zeros((D.cores, 128, D.E_pad // 128), np.float32)
    SRC = np.zeros((D.cores, 128, D.E_pad // 16), np.int16)
    for c in range(D.cores):
        sl = slice(c * D.E_pad, (c + 1) * D.E_pad)
        AT[c] = a_pad[sl].T
        WT[c] = _tile128(w_pad[sl])
        DRELC[c] = _tile128(drel_pad[sl])
        SRC[c] = _wrap16(src_pad[sl])

    # ---- graph edges: order by (core(dst), window(dst), dst) ----
    sg, dg = eig[0], eig[1]
    gcore = dg // GS
    gwin = (dg - gcore * GS) // D.gwin
    gorder = np.lexsort((dg, gwin, gcore))
    g_s, g_d, g_w, g_c = sg[gorder], dg[gorder], gwin[gorder], gcore[gorder]
    g_a = eag[gorder]

    SG = np.zeros((D.cores, 128, D.EG_pad // 16), np.int16)
    DREL = np.zeros((D.cores, 128, D.EG_pad // 128), np.float32)
    BHOT = np.zeros((D.cores, 3 * D.VOCAB, D.EG_pad), np.float32)

    gkeys = g_c * D.NWIN + g_w
    gbounds = np.searchsorted(gkeys, np.arange(D.cores * D.NWIN + 1))
    for c in range(D.cores):
        sg_pad = np.zeros(D.EG_pad, np.int64)
        dr_pad = np.full(D.EG_pad, -1.0, np.float32)  # -1 kills pads in one-hot
        bh_pad = np.zeros((3 * D.VOCAB, D.EG_pad), np.float32)
        for w in range(D.NWIN):
            lo, hi = gbounds[c * D.NWIN + w], gbounds[c * D.NWIN + w + 1]
            cnt = hi - lo
            assert cnt <= D.PW, f"PW overflow: core {c} win {w}: {cnt}"
            o = w * D.PW
            sg_pad[o:o + cnt] = g_s[lo:hi]
            dr_pad[o:o + cnt] = (g_d[lo:hi] - c * GS - w * D.gwin).astype(np.float32)
            for k in range(3):
                bh_pad[k * D.VOCAB + g_a[lo:hi, k], np.arange(o, o + cnt)] = 1.0
        SG[c] = _wrap16(sg_pad)
        DREL[c] = _tile128(dr_pad)
        BHOT[c] = bh_pad

    # ---- x^T shards ----
    xT = np.ascontiguousarray(x.T)  # [H, N]
    XT = xT.reshape(2, 128, D.N)

    # ---- weights (replicated) ----
    H2 = D.H // 128
    w = {k: np.asarray(inputs[k], np.float32) for k in (
        "mlp_w1", "mlp_b1", "mlp_w2", "mlp_b2", "cf_lin1", "cf_lin2",
        "cf_lin2_b", "lin_w", "lin_b", "bond_emb", "gin_eps", "gin_w1",
        "gin_w2", "bn1_g", "bn1_b", "bn2_g", "bn2_b")}
    const = dict(
        w1=w["mlp_w1"],                                   # [NG, NF]
        b1col=w["mlp_b1"].reshape(D.NF, 1),
        w2=w["mlp_w2"],                                   # [NF, NF]
        b2row=w["mlp_b2"].reshape(1, D.NF),
        b2row3=np.tile(w["mlp_b2"].reshape(1, D.NF), (1, 3)),
        ones1=np.ones((1, 128), np.float32),
        lin1=np.ascontiguousarray(w["cf_lin1"].reshape(H2, 128, D.NF)),
        lin2=w["cf_lin2"],                                # [NF, H]
        lin2b=w["cf_lin2_b"].reshape(H2, 128, 1),
        linw=np.ascontiguousarray(w["lin_w"].reshape(H2, 128, D.H)),
        linb=w["lin_b"].reshape(H2, 128, 1),
        gw1=np.ascontiguousarray(w["gin_w1"].reshape(H2, 128, D.H)),
        gw2=np.ascontiguousarray(w["gin_w2"].reshape(H2, 128, D.H)),
        bondcat=np.ascontiguousarray(
            w["bond_emb"].reshape(3 * D.VOCAB, D.H)),
        bn1g=w["bn1_g"].reshape(H2, 128, 1), bn1b=w["bn1_b"].reshape(H2, 128, 1),
        bn2g=w["bn2_g"].reshape(H2, 128, 1), bn2b=w["bn2_b"].reshape(H2, 128, 1),
        epsv=np.full((128, 1), 1.0 + float(w["gin_eps"]), np.float32),
        zerocol=np.zeros((128, 1), np.float32),
        eps5col=np.full((128, 1), 1e-5, np.float32),
        pihalf=np.full((128, 1), -math.pi / 2, np.float32),
        iota=np.tile(np.arange(128, dtype=np.float32), (128, 1)).copy(),
        iota6=np.tile(np.arange(128, dtype=np.float32), (128, 6)).copy(),
        ident=np.eye(128, dtype=np.float32),
    )

    in_maps = []
    for c in range(D.cores):
        m = dict(
            xT=np.ascontiguousarray(XT[:, :, c * NS:(c + 1) * NS]),
            AT=AT[c], WT=WT[c], SRC=SRC[c], DRELC=DRELC[c],
            SG=SG[c], DREL=DREL[c], BHOT=BHOT[c],
        )
        m.update(const)
        in_maps.append(m)
    return in_maps


def assemble(results, D: Dims):
    """Per-core outT [2,128,NS] -> full [N, H]."""
    parts = [r["outT"].reshape(D.H, D.NS) for r in results]
    outT = np.concatenate(parts, axis=1)  # [H, N]
    return np.ascontiguousarray(outT.T)


def _ts(i, n):
    return bass.ts(i, n)


def build_nc(D: Dims, flags: frozenset = frozenset()):
    nc = bacc.Bacc("TRN2", target_bir_lowering=False, debug=False,
                   num_devices=D.cores, num_swdge_queues=3)
    NS, GS, H, NF, NG = D.NS, D.GS, D.H, D.NF, D.NG
    H2 = H // 128

    I = {}
    def di(name, shape, dt=F32):
        I[name] = nc.dram_tensor(name, list(shape), dt, kind="ExternalInput")
        return I[name]

    di("xT", [2, 128, NS])
    di("AT", [NG, D.E_pad])
    di("WT", [128, D.E_pad // 128])
    di("SRC", [128, D.E_pad // 16], I16)
    di("DRELC", [128, D.E_pad // 128])
    di("SG", [128, D.EG_pad // 16], I16)
    di("DREL", [128, D.EG_pad // 128])
    di("BHOT", [3 * D.VOCAB, D.EG_pad])
    di("w1", [NG, NF]); di("b1col", [NF, 1]); di("w2", [NF, NF])
    di("b2row", [1, NF]); di("b2row3", [1, 3 * NF]); di("ones1", [1, 128])
    di("lin1", [H2, 128, NF]); di("lin2", [NF, H]); di("lin2b", [H2, 128, 1])
    di("linw", [H2, 128, H]); di("linb", [H2, 128, 1])
    di("gw1", [H2, 128, H]); di("gw2", [H2, 128, H])
    di("bondcat", [3 * D.VOCAB, H])
    di("bn1g", [H2, 128, 1]); di("bn1b", [H2, 128, 1])
    di("bn2g", [H2, 128, 1]); di("bn2b", [H2, 128, 1])
    di("epsv", [128, 1]); di("iota", [128, 128]); di("iota6", [128, 768])
    di("ident", [128, 128])
    di("zerocol", [128, 1]); di("eps5col", [128, 1]); di("pihalf", [128, 1])

    outT = nc.dram_tensor("outT", [2, 128, NS], F32, kind="ExternalOutput")

    groups = [list(range(D.cores))]

    with tile.TileContext(nc) as tc:
        with (
            tc.tile_pool(name="const", bufs=1) as cp,
            tc.tile_pool(name="work", bufs=2) as wp,
            tc.tile_pool(name="small", bufs=3) as sp,
            tc.tile_pool(name="gin", bufs=2) as ctx_gin_pool,
            tc.tile_pool(name="psum", bufs=2, space="PSUM") as pp,
            tc.tile_pool(name="dram", bufs=1, space="DRAM") as dp,
        ):
            # ---------- load constants ----------
            C = {}
            for nm, shp in [("w1", [NG, NF]), ("b1col", [NF, 1]),
                            ("w2", [NF, NF]), ("b2row", [1, NF]),
                            ("b2row3", [1, 3 * NF]),
                            ("ones1", [1, 128]), ("lin2", [NF, H]),
                            ("bondcat", [3 * D.VOCAB, H]),
                            ("epsv", [128, 1]), ("iota", [128, 128]),
                            ("iota6", [128, 768]),
                            ("ident", [128, 128]), ("zerocol", [128, 1]),
                            ("eps5col", [128, 1]), ("pihalf", [128, 1])]:
                t = cp.tile(shp, F32, name=f"c_{nm}")
                nc.sync.dma_start(t[:], I[nm].ap())
                C[nm] = t
            nc.const_aps.aps[(F32, 0.0)] = C["zerocol"][:]
            # [H2,128,*] constants: load as per-half tiles
            for nm in ("lin1", "lin2b", "linw", "linb", "gw1", "gw2",
                       "bn1g", "bn1b", "bn2g", "bn2b"):
                C[nm] = []
                inner = I[nm].shape[2]
                for k in range(H2):
                    t = cp.tile([128, inner], F32, name=f"c_{nm}{k}")
                    nc.sync.dma_start(t[:], I[nm].ap()[k])
                    C[nm].append(t)

            # ---------- DRAM scratch ----------
            # xf shard/full split in half so the AllGather pipelines with
            # compute: AG#h gathers every core's half-h shard; the gathered
            # layout is permuted (half-major), host_prep permutes src indices
            HSH = NS // 2
            xf_shard = [dp.tile([HSH, NF], F32, name=f"xf_shard{h}")
                        for h in range(2)]
            xf_full = [dp.tile([D.cores * HSH, NF], F32, name=f"xf_full{h}",
                               addr_space="Shared") for h in range(2)]
            xagg_shard = dp.tile([GS, H], F32, name="xagg_shard")
            xagg_full = dp.tile([D.cores * GS, H], F32, name="xagg_full",
                                addr_space="Shared")
            agg_ab = [dp.tile([NS, NF], F32, name=f"agg_{h}")
                      for h in range(2)]
            st1_in = dp.tile([128, 4], F32, name="st1_in")
            st1_out = dp.tile([128, 4], F32, name="st1_out", addr_space="Shared")
            st2_in = dp.tile([128, 4], F32, name="st2_in")
            st2_out = dp.tile([128, 4], F32, name="st2_out", addr_space="Shared")

            # =========== Phase A: xf half 0 FIRST, then pool, then half 1 ==
            # AG#0 (which gates the conformer window stream) is the first
            # collective issued; the pooling pass + xagg AllGather follow,
            # then xf half 1 + AG#1. GIN windows are deferred into the
            # window-stream driver so their xagg wait cannot HOL-block Pool.
            rep = D.N // D.G
            PCH = 250
            n_pch = NS // PCH
            half_chunks = n_pch // 2
            GT = (GS + 127) // 128

            def emit_xf_half(h):
                for jj in range(half_chunks):
                    j = h * half_chunks + jj
                    xt = [wp.tile([128, PCH], F32, tag=f"ph_a_xt{k}",
                                  name=f"ph_a2_xt{k}") for k in range(2)]
                    for k in range(2):
                        nc.sync.dma_start(xt[k][:],
                                          I["xT"].ap()[k, :, _ts(j, PCH)])
                    r0 = jj * PCH
                    nt = (PCH + 127) // 128
                    for t in range(nt):
                        m = min(128, PCH - t * 128)
                        ps = pp.tile([128, NF], F32, tag="ps_mm", name="ps_mm")
                        for k in range(2):
                            nc.tensor.matmul(ps[:m, :],
                                             xt[k][:, t * 128:t * 128 + m],
                                             C["lin1"][k][:], start=(k == 0),
                                             stop=(k == 1))
                        sb = sp.tile([128, NF], F32, tag="ph_a_sb",
                                     name="ph_a_sb")
                        nc.scalar.copy(sb[:m, :], ps[:m, :])
                        nc.sync.dma_start(
                            xf_shard[h][r0 + t * 128: r0 + t * 128 + m, :],
                            sb[:m, :])
                if "no_coll" not in flags and "no_coll_xf" not in flags:
                    nc.gpsimd.collective_compute(
                        "AllGather", ALU.bypass, replica_groups=groups,
                        ins=[xf_shard[h].opt()], outs=[xf_full[h].opt()])

            emit_xf_half(0)

            # pooling pass + xagg AllGather
            xaggT = [cp.tile([128, GS], F32, name=f"xaggT{k}") for k in range(2)]
            for j in range(n_pch):
                xt = [wp.tile([128, PCH], F32, tag=f"ph_a_xt{k}", name=f"ph_a_xt{k}")
                      for k in range(2)]
                for k in range(2):
                    nc.sync.dma_start(xt[k][:], I["xT"].ap()[k, :, _ts(j, PCH)])
                # pool: max over groups of 10 cols
                for k in range(2):
                    nc.vector.tensor_reduce(
                        xaggT[k][:, _ts(j, PCH // rep)],
                        xt[k][:].rearrange("p (g t) -> p g t", t=rep),
                        AX.X, ALU.max)
            for t in range(GT):
                m = min(128, GS - t * 128)
                for k in range(2):
                    pst = pp.tile([128, 128], F32, tag="ps_tr", name="ps_tr")
                    nc.tensor.transpose(pst[:m, :], xaggT[k][:, t * 128:t * 128 + m],
                                        C["ident"][:])
                    sb = sp.tile([128, 128], F32, tag="ph_a_trsb", name="ph_a_trsb")
                    nc.scalar.copy(sb[:m, :], pst[:m, :])
                    nc.sync.dma_start(
                        xagg_shard[t * 128:t * 128 + m, _ts(k, 128)], sb[:m, :])
            if "no_coll" not in flags and "no_coll_xagg" not in flags:
                nc.gpsimd.collective_compute(
                    "AllGather", ALU.bypass, replica_groups=groups,
                    ins=[xagg_shard.opt()], outs=[xagg_full.opt()])

            emit_xf_half(1)

            # =========== Phase B: GIN branch (sharded by graph node) =========
            # gather x_agg[sg], edge_emb via bond one-hot matmul, relu,
            # one-hot scatter into agg_g windows
            sgidx = cp.tile([128, D.EG_pad // 16], I16, name="sgidx_sb")
            nc.sync.dma_start(sgidx[:], I["SG"].ap())
            drel = cp.tile([128, D.EG_pad // 128], F32, name="drel_sb")
            nc.sync.dma_start(drel[:], I["DREL"].ap())

            # t-buffer (node-major (1+eps)x_agg + agg_g), then transposed halves
            gp = ctx_gin_pool
            assert D.EG_pad % 128 == 0
            tiles_per_win = D.PW // 128
            bstate = {}

            def emit_b1_windows(tT):
              for w in range(D.NWIN):
                m = min(D.gwin, GS - w * D.gwin)
                # gather this window's source rows
                gath_g = wp.tile([128, tiles_per_win, H], F32,
                                 tag="ph_b_gath", name="ph_b_gath")
                nc.gpsimd.dma_gather(
                    gath_g[:], xagg_full[:],
                    sgidx[:, w * D.PW // 16:(w + 1) * D.PW // 16],
                    num_idxs=D.PW, num_idxs_reg=D.PW, elem_size=H)
                bhot = wp.tile([3 * D.VOCAB, D.PW], F32, tag="ph_b_bhot",
                               name="ph_b_bhot")
                nc.sync.dma_start(bhot[:],
                                  I["BHOT"].ap()[:, w * D.PW:(w + 1) * D.PW])
                ps_agg = pp.tile([128, H], F32, tag="ps_agg", name="ps_agg")
                for i in range(tiles_per_win):
                    t = w * tiles_per_win + i
                    # edge embedding: one-hot bond matmul (K=15)
                    ps_emb = pp.tile([128, H], F32, tag="ps_mm", name="ps_mm")
                    nc.tensor.matmul(ps_emb[:], bhot[:, _ts(i, 128)],
                                     C["bondcat"][:], start=True, stop=True)
                    # msg = relu(gathered + emb)
                    msg = sp.tile([128, H], F32, tag="ph_b_msg", name="ph_b_msg")
                    nc.vector.tensor_tensor(msg[:], gath_g[:, i, :], ps_emb[:],
                                            ALU.add)
                    nc.scalar.activation(msg[:], msg[:], ACTF.Relu)
                    # one-hot scatter
                    oh = sp.tile([128, D.gwin], F32, tag="ph_b_oh", name="ph_b_oh")
                    nc.vector.tensor_scalar(oh[:], C["iota"][:, :D.gwin],
                                            drel[:, t:t + 1], None, ALU.is_equal)
                    nc.tensor.matmul(ps_agg[:m, :], oh[:, :m], msg[:],
                                     start=(i == 0), stop=(i == tiles_per_win - 1))
                # t = (1+eps) * x_agg + agg_g  (node-major window rows)
                xa = sp.tile([128, H], F32, tag="ph_b_xa", name="ph_b_xa")
                nc.sync.dma_start(
                    xa[:m, :], xagg_shard[w * D.gwin:w * D.gwin + m, :])
                tn = sp.tile([128, H], F32, tag="ph_b_tn", name="ph_b_tn")
                nc.vector.tensor_scalar(tn[:m, :], xa[:m, :], C["epsv"][:m, :],
                                        None, ALU.mult)
                nc.vector.tensor_tensor(tn[:m, :], tn[:m, :], ps_agg[:m, :],
                                        ALU.add)
                # transpose to feature-major tT
                for k in range(2):
                    pst = pp.tile([128, 128], F32, tag="ps_tr", name="ps_tr")
                    nc.tensor.transpose(pst[:, :m], tn[:m, _ts(k, 128)],
                                        C["ident"][:m, :m])
                    nc.vector.tensor_copy(tT[k][:, w * D.gwin:w * D.gwin + m],
                                          pst[:, :m])

            def gin_mm_and_stats_issue(inT, Wc, uT, stats_in, stats_out, label):
                """u = in @ W (node-major tiles), transpose to uT, stats;
                issues the stats AllReduce but does NOT read the result."""
                for t in range(GT):
                    m = min(128, GS - t * 128)
                    ps = pp.tile([128, H], F32, tag="ps_mm", name="ps_mm")
                    for k in range(2):
                        nc.tensor.matmul(ps[:m, :],
                                         inT[k][:, t * 128:t * 128 + m],
                                         Wc[k][:], start=(k == 0), stop=(k == 1))
                    sb = sp.tile([128, H], F32, tag=f"{label}_sb", name=f"{label}_sb")
                    nc.scalar.copy(sb[:m, :], ps[:m, :])
                    for k in range(2):
                        pst = pp.tile([128, 128], F32, tag="ps_tr", name="ps_tr")
                        nc.tensor.transpose(pst[:, :m], sb[:m, _ts(k, 128)],
                                            C["ident"][:m, :m])
                        nc.vector.tensor_copy(uT[k][:, t * 128:t * 128 + m],
                                              pst[:, :m])
                st = sp.tile([128, 4], F32, tag=f"{label}_st", name=f"{label}_st")
                sq = sp.tile([128, GS], F32, tag="gin_sq", name="gin_sq",
                             bufs=1)
                for k in range(2):
                    nc.vector.tensor_reduce(st[:, 2 * k:2 * k + 1], uT[k][:],
                                            AX.X, ALU.add)
                    nc.vector.tensor_tensor(sq[:], uT[k][:], uT[k][:], ALU.mult)
                    nc.vector.tensor_reduce(st[:, 2 * k + 1:2 * k + 2], sq[:],
                                            AX.X, ALU.add)
                nc.sync.dma_start(stats_in[:], st[:])
                if "no_b_ar" not in flags:
                    nc.gpsimd.collective_compute(
                        "AllReduce", ALU.add, replica_groups=groups,
                        ins=[stats_in.opt()], outs=[stats_out.opt()])

            def gin_read_stf(stats_in, stats_out, label):
                stf = sp.tile([128, 4], F32, tag=f"{label}_stf", name=f"{label}_stf")
                nc.sync.dma_start(
                    stf[:], stats_in[:] if "no_b_ar" in flags else stats_out[:])
                return stf

            def bn_apply(stf, uT, g_c, b_c, outT_t, relu, label):
                """out = func((u - mu) * g / sqrt(var+eps) + b), feature-major."""
                inv_n = 1.0 / float(D.G)
                for k in range(2):
                    mu = sp.tile([128, 1], F32, tag=f"{label}_mu{k}", name=f"{label}_mu{k}")
                    nc.vector.tensor_scalar(mu[:], stf[:, 2 * k:2 * k + 1],
                                            inv_n, None, ALU.mult)
                    var = sp.tile([128, 1], F32, tag=f"{label}_va{k}", name=f"{label}_va{k}")
                    nc.vector.tensor_scalar(var[:], stf[:, 2 * k + 1:2 * k + 2],
                                            inv_n, None, ALU.mult)
                    mu2 = sp.tile([128, 1], F32, tag=f"{label}_m2{k}", name=f"{label}_m2{k}")
                    nc.vector.tensor_tensor(mu2[:], mu[:], mu[:], ALU.mult)
                    nc.vector.tensor_tensor(var[:], var[:], mu2[:], ALU.subtract)
                    sd = sp.tile([128, 1], F32, tag=f"{label}_sd{k}", name=f"{label}_sd{k}")
                    nc.scalar.activation(sd[:], var[:], ACTF.Sqrt,
                                         bias=C["eps5col"][:])
                    rs = sp.tile([128, 1], F32, tag=f"{label}_rs{k}", name=f"{label}_rs{k}")
                    nc.vector.reciprocal(rs[:], sd[:])
                    sc = sp.tile([128, 1], F32, tag=f"{label}_sc{k}", name=f"{label}_sc{k}")
                    nc.vector.tensor_tensor(sc[:], g_c[k][:], rs[:], ALU.mult)
                    sh = sp.tile([128, 1], F32, tag=f"{label}_sh{k}", name=f"{label}_sh{k}")
                    nc.vector.tensor_tensor(sh[:], mu[:], sc[:], ALU.mult)
                    nc.vector.tensor_tensor(sh[:], b_c[k][:], sh[:], ALU.subtract)
                    nc.scalar.activation(outT_t[k][:], uT[k][:],
                                         ACTF.Relu if relu else ACTF.Identity,
                                         bias=sh[:], scale=sc[:])

            def gin_buf(nm):
                return [gp.tile([128, GS], F32, tag=f"ginbuf{k}",
                                name=f"{nm}{k}") for k in range(2)]

            if "no_b" in flags:
                ginT = gin_buf("ginT")
                for k in range(2):
                    nc.vector.memset(ginT[k][:], 0.0)
                bstate["ginT"] = ginT

            def emit_b1():
                """GIN window aggregation + gmm1 + st1 issue (early phase C,
                after the xagg AllGather has had time to land)."""
                if "no_b" in flags or "uT" in bstate:
                    return
                tT = [gp.tile([128, GS], F32, tag=f"ginbuf{k}", name=f"tT{k}")
                      for k in range(2)]
                emit_b1_windows(tT)
                uT = gin_buf("uT")
                gin_mm_and_stats_issue(tT, C["gw1"], uT, st1_in, st1_out,
                                       "gmm1")
                bstate["uT"] = uT

            def emit_b2():
                """bn1 + gmm2 + st2 AllReduce issue (mid phase C)."""
                if "no_b" in flags or "t2T" in bstate or "uT" not in bstate:
                    return
                stf1 = gin_read_stf(st1_in, st1_out, "gmm1")
                t1T = gin_buf("t1T")
                bn_apply(stf1, bstate["uT"], C["bn1g"], C["bn1b"], t1T,
                         True, "bn1")
                t2T = gin_buf("t2T")
                gin_mm_and_stats_issue(t1T, C["gw2"], t2T, st2_in, st2_out,
                                       "gmm2")
                bstate["t2T"] = t2T

            def emit_b3():
                """bn2 -> ginT (late in phase C)."""
                if "no_b" in flags or "ginT" in bstate:
                    return
                stf2 = gin_read_stf(st2_in, st2_out, "gmm2")
                ginT = gin_buf("ginT")
                bn_apply(stf2, bstate["t2T"], C["bn2g"], C["bn2b"], ginT,
                         False, "bn2")
                bstate["ginT"] = ginT

            # =========== Phase C: conformer edge pipeline ===========
            # window-major: per (src-half stream h, dst-window w of 128
            # nodes): gather the window's edges (one call per src quad
            # bucket of B_E), compute msg = (xf[src]) * (mlp(A) + b2), then
            # aggregate over dst via one-hot matmuls into PSUM (the one-hot
            # rows carry the cosine-cutoff C so no separate C-multiply),
            # and flush the window's 128 agg rows with a plain DMA write.
            # No scatter-add: each agg row is written exactly once.
            # resident: C row (cosine cutoff) and dst-rel row per edge
            crow = cp.tile([128, D.E_pad // 128], F32, name="crow_sb")
            for s0 in range(0, D.E_pad // 128, 512):
                sw = min(512, D.E_pad // 128 - s0)
                wt = wp.tile([128, 512], F32, tag="ph_c_wt", name="ph_c_wt")
                nc.sync.dma_start(wt[:, :sw], I["WT"].ap()[:, s0:s0 + sw])
                nc.scalar.activation(wt[:, :sw], wt[:, :sw], ACTF.Sin,
                                     bias=C["pihalf"][:],
                                     scale=math.pi / D.CUTOFF)
                nc.scalar.activation(crow[:, s0:s0 + sw], wt[:, :sw],
                                     ACTF.Copy, bias=0.5, scale=-0.5)
            drelc = cp.tile([128, D.E_pad // 128], F32, name="drelc_sb")
            nc.sync.dma_start(drelc[:], I["DRELC"].ap())

            B_E, CW, EH = D.B_E, D.CW, D.EH
            WE = 2 * B_E           # edges per (stream, window)
            SW = 2 * WE            # edges per super-window (pair of windows)
            NTS = SW // 128        # tiles per super-window (12)
            assert CW % 2 == 0
            wstream = ([] if "no_c" in flags else
                       [(h, w) for h in range(2) for w in range(0, CW, 2)])
            PFD = 3  # gather prefetch depth (super-windows issued ahead)
            gat_fifo = []

            def emit_gather(idx):
                h, w = wstream[idx]
                e0 = h * EH + w * WE
                if "no_gather" in flags:
                    if "no_cmm" in flags:
                        return None
                    gat = wp.tile([128, NTS, NF], F32, tag="ph_c_gat",
                                  name="ph_c_gat", bufs=PFD + 2)
                    nc.vector.memset(gat[:], 0.0)
                    return gat
                gat = wp.tile([128, NTS, NF], F32, tag="ph_c_gat",
                              name="ph_c_gat", bufs=PFD + 2)
                si = wp.tile([128, SW // 16], I16, tag="ph_c_si",
                             name="ph_c_si", bufs=PFD + 2)
                nc.sync.dma_start(
                    si[:], I["SRC"].ap()[:, e0 // 16:(e0 + SW) // 16])
                for j in range(2):
                    nc.gpsimd.dma_gather(
                        gat[:, j * 6:(j + 1) * 6, :],
                        xf_full[h][j * D.qsize:(j + 1) * D.qsize, :],
                        si[:, j * 2 * B_E // 16:(j + 1) * 2 * B_E // 16],
                        num_idxs=2 * B_E, num_idxs_reg=2 * B_E, elem_size=NF,
                        queue_num=(2 * idx + j) % 3)
                return gat

            def emit_compute_flush(idx, gat):
                h, w = wstream[idx]
                e0 = h * EH + w * WE
                c0col = e0 // 128
                msg = None
                if "no_cmm" not in flags:
                    msg = wp.tile([128, NTS, NF], F32, tag="ph_c_msg",
                                  name="ph_c_msg")
                    at = wp.tile([NG, SW], F32, tag="ph_c_at",
                                 name="ph_c_at")
                    nc.sync.dma_start(at[:], I["AT"].ap()[:, e0:e0 + SW])
                    # per 512-edge group: mm1+relu, mm2 (+b2) packed 4 tiles
                    # into one PSUM bank; 512-wide msg-mul
                    for g in range(3):
                        s0 = g * 512
                        ps1 = pp.tile([128, 512], F32, tag="ps_mm", name="ps_mm")
                        nc.tensor.matmul(ps1[:], C["w1"][:],
                                         at[:, s0:s0 + 512],
                                         start=True, stop=True)
                        h1 = wp.tile([128, 512], F32, tag="ph_c_h1",
                                     name="ph_c_h1")
                        nc.scalar.activation(h1[:], ps1[:],
                                             ACTF.Relu, bias=C["b1col"][:])
                        psw = pp.tile([128, 4, NF], F32, tag="ps_w", name="ps_w")
                        for t4 in range(4):
                            nc.tensor.matmul(psw[:, t4, :], h1[:, _ts(t4, 128)],
                                             C["w2"][:], start=True,
                                             stop="b2zero" in flags)
                            if "b2zero" not in flags:
                                nc.tensor.matmul(psw[:, t4, :], C["ones1"][:],
                                                 C["b2row"][:], start=False,
                                                 stop=True)
                        nc.vector.tensor_tensor(msg[:, 4 * g:4 * g + 4, :],
                                                gat[:, 4 * g:4 * g + 4, :],
                                                psw[:], ALU.mult)
                if "no_scatter" in flags or "no_cmm" in flags:
                    return
                # one-hot aggregation: rows carry C; accumulate over tiles,
                # one PSUM half-bank per window of the pair
                ohc = sp.tile([128, NTS, 128], F32, tag="ph_c_oh",
                              name="ph_c_oh", bufs=2)
                for p2 in range(2):
                    view = ohc[:, p2 * 6:(p2 + 1) * 6, :]
                    cc = c0col + p2 * 6
                    nc.vector.tensor_tensor(
                        view,
                        C["iota6"][:].rearrange("p (t j) -> p t j", j=128),
                        drelc[:, cc:cc + 6].broadcast_to((128, 6, 128)),
                        ALU.is_equal)
                    nc.vector.tensor_tensor(
                        view, view,
                        crow[:, cc:cc + 6].broadcast_to((128, 6, 128)),
                        ALU.mult)
                psA = pp.tile([128, 2, NF], F32, tag="ps_agg", name="ps_cagg")
                for p2 in range(2):
                    tl = [t for t in range(NTS) if (t // 3) % 2 == p2]
                    for i, t in enumerate(tl):
                        nc.tensor.matmul(psA[:, p2, :], ohc[:, t, :],
                                         msg[:, t, :], start=(i == 0),
                                         stop=(i == len(tl) - 1))
                for p2 in range(2):
                    base = (w + p2) * 128
                    m = min(128, NS - base)
                    stg = sp.tile([128, NF], F32, tag="ph_c_stg",
                                  name="ph_c_stg", bufs=2)
                    nc.scalar.copy(stg[:m, :], psA[:m, p2, :])
                    nc.sync.dma_start(
                        agg_ab[h][base:base + m, :].rearrange(
                            "(t p) f -> p t f", p=m),
                        stg[:m, :].rearrange("p (t f) -> p t f", f=NF))

            # =========== Phase D: h = relu(agg@lin2+b)@linw+b, residual =====
            # emitted as a closure so node chunks interleave into the tail
            # of the window stream (chunk j only needs agg rows already
            # flushed by both streams, plus ginT)
            NCH = D.nchunk
            n_nch = NS // NCH

            def emit_d(j):
                r0 = j * NCH
                # load agg rows, transpose to feature-major aggT [NF, NCH]
                aggT = wp.tile([NF, NCH], F32, tag="ph_d_aggT", name="ph_d_aggT")
                ntt = (NCH + 127) // 128
                for t in range(ntt):
                    m = min(128, NCH - t * 128)
                    asb = sp.tile([128, NF], F32, tag="ph_d_asb", name="ph_d_asb")
                    nc.sync.dma_start(asb[:m, :],
                                      agg_ab[0][r0 + t * 128:r0 + t * 128 + m, :])
                    bsb = sp.tile([128, NF], F32, tag="ph_d_bsb", name="ph_d_bsb")
                    nc.sync.dma_start(bsb[:m, :],
                                      agg_ab[1][r0 + t * 128:r0 + t * 128 + m, :])
                    nc.vector.tensor_tensor(asb[:m, :], asb[:m, :], bsb[:m, :],
                                            ALU.add)
                    pst = pp.tile([128, 128], F32, tag="ps_tr", name="ps_tr")
                    nc.tensor.transpose(pst[:, :m], asb[:m, :], C["ident"][:m, :m])
                    nc.vector.tensor_copy(aggT[:, t * 128:t * 128 + m],
                                          pst[:, :m])
                # h1T = relu(lin2^T @ aggT + b)  [2][128, NCH]
                h1T = [wp.tile([128, NCH], F32, tag=f"ph_d_h1T{k}", name=f"ph_d_h1T{k}")
                       for k in range(2)]
                for k in range(2):
                    ps = pp.tile([128, NCH], F32, tag="ps_mm", name="ps_mm")
                    nc.tensor.matmul(ps[:], C["lin2"][:, _ts(k, 128)], aggT[:],
                                     start=True, stop=True)
                    nc.scalar.activation(h1T[k][:], ps[:], ACTF.Relu,
                                         bias=C["lin2b"][k][:])
                # outT = linw^T @ h1T + linb + xT + gin[batch]
                for k in range(2):
                    ps = pp.tile([128, NCH], F32, tag="ps_mm", name="ps_mm")
                    for kk in range(2):
                        nc.tensor.matmul(ps[:], C["linw"][kk][:, _ts(k, 128)],
                                         h1T[kk][:], start=(kk == 0),
                                         stop=(kk == 1))
                    ob = sp.tile([128, NCH], F32, tag="ph_d_ob", name="ph_d_ob")
                    nc.scalar.activation(ob[:], ps[:], ACTF.Identity,
                                         bias=C["linb"][k][:])
                    xtc = sp.tile([128, NCH], F32, tag="ph_d_xtc", name="ph_d_xtc")
                    nc.sync.dma_start(xtc[:], I["xT"].ap()[k, :, r0:r0 + NCH])
                    nc.vector.tensor_tensor(ob[:], ob[:], xtc[:], ALU.add)
                    # + gin, each graph col repeated `rep` times
                    rep = D.N // D.G
                    g0 = r0 // rep
                    gin_rep = bstate["ginT"][k][:, g0:g0 + NCH // rep] \
                        .broadcast_to((128, NCH // rep, rep))
                    nc.vector.tensor_tensor(
                        ob[:].rearrange("p (g t) -> p g t", t=rep),
                        ob[:].rearrange("p (g t) -> p g t", t=rep),
                        gin_rep, ALU.add)
                    nc.sync.dma_start(outT.ap()[k, :, r0:r0 + NCH], ob[:])

            next_d = [0]

            def emit_d_ready(done):
                """Emit phase D chunks whose agg rows are fully flushed
                (at most 3 per window to avoid emission bursts)."""
                if "no_d" in flags or "ginT" not in bstate:
                    return
                burst = 0
                while (next_d[0] < n_nch and burst < 3
                       and (done - 48) * 256 >= NCH * (next_d[0] + 1)):
                    emit_d(next_d[0])
                    next_d[0] += 1
                    burst += 1

            for i in range(len(wstream) + PFD):
                if i < len(wstream):
                    gat_fifo.append(emit_gather(i))
                if i >= PFD:
                    done = i - PFD
                    emit_compute_flush(done, gat_fifo[done])
                    # GIN stage hooks: latency of the stats AllReduces and
                    # the serial BN chains hides under the window stream
                    if done == 28:
                        emit_b1()
                    if done == 61:
                        emit_b2()
                    if done == 80:
                        emit_b3()
                    if done > 80:
                        emit_d_ready(done)
            emit_b1()  # no-ops unless phase C was skipped
            emit_b2()
            emit_b3()
            if "no_d" not in flags:
                for j in range(next_d[0], n_nch):
                    emit_d(j)

    nc.compile()
    return nc


_CACHE = {}


def _get_nc(D: Dims, flags: frozenset = frozenset()):
    key = ("nc", D, flags)
    if key not in _CACHE:
        _CACHE[key] = build_nc(D, flags)
    return _CACHE[key]


def run_on_hw(inputs, D: Dims = REAL):
    flags = (frozenset({"b2zero"})
             if not np.any(np.asarray(inputs["mlp_b2"])) else frozenset())
    nc = _get_nc(D, flags)
    in_maps = host_prep(inputs, D)
    res = bass_utils.run_bass_kernel_spmd(nc, in_maps,
                                          core_ids=list(range(D.cores)))
    return assemble(res.results, D)


def kernel(**inputs):
    return run_on_hw(inputs, REAL)



# revision 64
# speedup vs baseline: 2.1925x; 1.0173x over previous
"""Trainium2 Bass kernel for nn_DSSConf (DSS conformer GNN message passing).

Self-contained: hardcodes shapes/sharding for the real problem; exposes
kernel(**inputs) -> np.ndarray.
"""
import sys
import math
from dataclasses import dataclass

sys.path.insert(0, "/opt/trn_rl_repo")

import numpy as np
from concourse import bass, bacc, tile, mybir, bass_utils

F32 = mybir.dt.float32
I16 = mybir.dt.int16
ALU = mybir.AluOpType
ACTF = mybir.ActivationFunctionType
AX = mybir.AxisListType


@dataclass(frozen=True)
class Dims:
    N: int = 100000        # conformer nodes
    H: int = 256           # hidden
    NF: int = 128          # num filters
    NG: int = 50           # num gaussians
    G: int = 10000         # graph nodes
    E: int = 1000000       # conformer edges
    EG: int = 30000        # graph edges
    VOCAB: int = 5
    CUTOFF: float = 10.0
    cores: int = 8
    qsize: int = 25000     # src quadrant size for int16 gather indices
    B_E: int = 384         # fixed edges per (dst-window, src-quad) bucket
    gwin: int = 128        # GIN scatter window (<=128 segments)
    PW: int = 640          # padded GIN edges per (core, window) (multiple of 128)
    nchunk: int = 500      # node chunk for the h/out stage (divides NS, mult of 10)

    @property
    def NS(self):
        return self.N // self.cores

    @property
    def GS(self):
        return self.G // self.cores

    @property
    def CW(self):
        """dst windows of 128 nodes per core shard."""
        return (self.NS + 127) // 128

    @property
    def EH(self):
        """padded edges per src-half stream (2 quad buckets per window)."""
        return self.CW * 2 * self.B_E

    @property
    def E_pad(self):
        return 2 * self.EH

    @property
    def NWIN(self):
        return (self.GS + self.gwin - 1) // self.gwin

    @property
    def EG_pad(self):
        return self.NWIN * self.PW


REAL = Dims()


def _wrap16(arr, dtype=np.int16):
    """Edge i -> [i % 16, i // 16], replicated to 128 partitions."""
    a = np.asarray(arr).reshape(-1, 16).T.astype(dtype)  # [16, n/16]
    return np.tile(a, (8, 1)).copy()  # [128, n/16]


def _tile128(arr, dtype=np.float32):
    """Edge i -> [i % 128, i // 128] (per-partition scalar layout)."""
    return np.ascontiguousarray(np.asarray(arr).reshape(-1, 128).T.astype(dtype))


def host_prep(inputs, D: Dims):
    """Build per-core in_maps (list of dicts) for the SPMD kernel."""
    x = np.asarray(inputs["x"], np.float32)
    cnb = np.asarray(inputs["conf_node_batch"]).astype(np.int64)
    ei = np.asarray(inputs["edge_index_conf"]).astype(np.int64)
    ew = np.asarray(inputs["edge_weight_conf"], np.float32)
    ea = np.asarray(inputs["edge_attr_conf"], np.float32)
    eig = np.asarray(inputs["edge_index_graph"]).astype(np.int64)
    eag = np.asarray(inputs["edge_attr_graph"]).astype(np.int64)

    rep = D.N // D.G
    assert np.array_equal(cnb, np.repeat(np.arange(D.G), rep)), \
        "conf_node_batch structure mismatch"

    NS, GS = D.NS, D.GS
    src, dst = ei[0], ei[1]

    # ---- conformer edges: window-major one-hot layout ----
    # per core: two streams by src half (matching the split AllGather's
    # permuted xf layout); within a stream, buckets of fixed B_E edges per
    # (dst-window of 128, src quad within the half); in-quad src offset =
    # (src_core % 4)*NS/2 + (src % NS/2)
    B_E, CW, EH = D.B_E, D.CW, D.EH
    core = dst // NS
    sc = src // NS
    so = src % NS
    sh = so // (NS // 2)
    sj = (sc >= 4).astype(np.int64)
    inq = (sc % 4) * (NS // 2) + (so % (NS // 2))
    win = (dst % NS) // 128
    bucket = ((core * 2 + sh) * CW + win) * 2 + sj
    order = np.argsort(bucket, kind="stable")
    sb = bucket[order]
    nbuck = D.cores * 2 * CW * 2
    bounds = np.searchsorted(sb, np.arange(nbuck + 1))
    counts = bounds[1:] - bounds[:-1]
    assert counts.max() <= B_E, f"bucket overflow: {counts.max()} > {B_E}"
    rank = np.arange(len(sb)) - bounds[sb]
    c_b = sb // (2 * CW * 2)
    rem = sb % (2 * CW * 2)
    h_b = rem // (CW * 2)
    w_b = (rem % (CW * 2)) // 2
    j_b = rem % 2
    s_b, p_b = w_b // 2, w_b % 2
    flat = (c_b * D.E_pad + h_b * EH
            + (s_b * 4 + j_b * 2 + p_b) * B_E + rank)

    src_pad = np.zeros(D.cores * D.E_pad, np.int64)
    drel_pad = np.full(D.cores * D.E_pad, -1.0, np.float32)
    w_pad = np.full(D.cores * D.E_pad, D.CUTOFF, np.float32)  # C(CUTOFF)=0
    a_pad = np.zeros((D.cores * D.E_pad, D.NG), np.float32)
    src_pad[flat] = inq[order]
    drel_pad[flat] = ((dst[order] % NS) - w_b * 128).astype(np.float32)
    w_pad[flat] = ew[order]
    a_pad[flat] = ea[order]

    AT = np.zeros((D.cores, D.NG, D.E_pad), np.float32)
    WT = np.zeros((D.cores, 128, D.E_pad // 128), np.float32)
    DRELC = np.zeros((D.cores, 128, D.E_pad // 128), np.float32)
    SRC = np.zeros((D.cores, 128, D.E_pad // 16), np.int16)
    for c in range(D.cores):
        sl = slice(c * D.E_pad, (c + 1) * D.E_pad)
        AT[c] = a_pad[sl].T
        WT[c] = _tile128(w_pad[sl])
        DRELC[c] = _tile128(drel_pad[sl])
        SRC[c] = _wrap16(src_pad[sl])

    # ---- graph edges: order by (core(dst), window(dst), dst) ----
    sg, dg = eig[0], eig[1]
    gcore = dg // GS
    gwin = (dg - gcore * GS) // D.gwin
    gorder = np.lexsort((dg, gwin, gcore))
    g_s, g_d, g_w, g_c = sg[gorder], dg[gorder], gwin[gorder], gcore[gorder]
    g_a = eag[gorder]

    SG = np.zeros((D.cores, 128, D.EG_pad // 16), np.int16)
    DREL = np.zeros((D.cores, 128, D.EG_pad // 128), np.float32)
    BHOT = np.zeros((D.cores, 3 * D.VOCAB, D.EG_pad), np.float32)

    gkeys = g_c * D.NWIN + g_w
    gbounds = np.searchsorted(gkeys, np.arange(D.cores * D.NWIN + 1))
    for c in range(D.cores):
        sg_pad = np.zeros(D.EG_pad, np.int64)
        dr_pad = np.full(D.EG_pad, -1.0, np.float32)  # -1 kills pads in one-hot
        bh_pad = np.zeros((3 * D.VOCAB, D.EG_pad), np.float32)
        for w in range(D.NWIN):
            lo, hi = gbounds[c * D.NWIN + w], gbounds[c * D.NWIN + w + 1]
            cnt = hi - lo
            assert cnt <= D.PW, f"PW overflow: core {c} win {w}: {cnt}"
            o = w * D.PW
            sg_pad[o:o + cnt] = g_s[lo:hi]
            dr_pad[o:o + cnt] = (g_d[lo:hi] - c * GS - w * D.gwin).astype(np.float32)
            for k in range(3):
                bh_pad[k * D.VOCAB + g_a[lo:hi, k], np.arange(o, o + cnt)] = 1.0
        SG[c] = _wrap16(sg_pad)
        DREL[c] = _tile128(dr_pad)
        BHOT[c] = bh_pad

    # ---- x^T shards ----
    xT = np.ascontiguousarray(x.T)  # [H, N]
    XT = xT.reshape(2, 128, D.N)

    # ---- weights (replicated) ----
    H2 = D.H // 128
    w = {k: np.asarray(inputs[k], np.float32) for k in (
        "mlp_w1", "mlp_b1", "mlp_w2", "mlp_b2", "cf_lin1", "cf_lin2",
        "cf_lin2_b", "lin_w", "lin_b", "bond_emb", "gin_eps", "gin_w1",
        "gin_w2", "bn1_g", "bn1_b", "bn2_g", "bn2_b")}
    const = dict(
        w1=w["mlp_w1"],                                   # [NG, NF]
        b1col=w["mlp_b1"].reshape(D.NF, 1),
        w2=w["mlp_w2"],                                   # [NF, NF]
        b2row=w["mlp_b2"].reshape(1, D.NF),
        b2row3=np.tile(w["mlp_b2"].reshape(1, D.NF), (1, 3)),
        ones1=np.ones((1, 128), np.float32),
        lin1=np.ascontiguousarray(w["cf_lin1"].reshape(H2, 128, D.NF)),
        lin2=w["cf_lin2"],                                # [NF, H]
        lin2b=w["cf_lin2_b"].reshape(H2, 128, 1),
        linw=np.ascontiguousarray(w["lin_w"].reshape(H2, 128, D.H)),
        linb=w["lin_b"].reshape(H2, 128, 1),
        gw1=np.ascontiguousarray(w["gin_w1"].reshape(H2, 128, D.H)),
        gw2=np.ascontiguousarray(w["gin_w2"].reshape(H2, 128, D.H)),
        bondcat=np.ascontiguousarray(
            w["bond_emb"].reshape(3 * D.VOCAB, D.H)),
        bn1g=w["bn1_g"].reshape(H2, 128, 1), bn1b=w["bn1_b"].reshape(H2, 128, 1),
        bn2g=w["bn2_g"].reshape(H2, 128, 1), bn2b=w["bn2_b"].reshape(H2, 128, 1),
        epsv=np.full((128, 1), 1.0 + float(w["gin_eps"]), np.float32),
        zerocol=np.zeros((128, 1), np.float32),
        eps5col=np.full((128, 1), 1e-5, np.float32),
        pihalf=np.full((128, 1), -math.pi / 2, np.float32),
        iota=np.tile(np.arange(128, dtype=np.float32), (128, 1)).copy(),
        iota6=np.tile(np.arange(128, dtype=np.float32), (128, 6)).copy(),
        ident=np.eye(128, dtype=np.float32),
    )

    in_maps = []
    for c in range(D.cores):
        m = dict(
            xT=np.ascontiguousarray(XT[:, :, c * NS:(c + 1) * NS]),
            AT=AT[c], WT=WT[c], SRC=SRC[c], DRELC=DRELC[c],
            SG=SG[c], DREL=DREL[c], BHOT=BHOT[c],
        )
        m.update(const)
        in_maps.append(m)
    return in_maps


def assemble(results, D: Dims):
    """Per-core outT [2,128,NS] -> full [N, H]."""
    parts = [r["outT"].reshape(D.H, D.NS) for r in results]
    outT = np.concatenate(parts, axis=1)  # [H, N]
    return np.ascontiguousarray(outT.T)


def _ts(i, n):
    return bass.ts(i, n)


def build_nc(D: Dims, flags: frozenset = frozenset()):
    nc = bacc.Bacc("TRN2", target_bir_lowering=False, debug=False,
                   num_devices=D.cores, num_swdge_queues=3)
    NS, GS, H, NF, NG = D.NS, D.GS, D.H, D.NF, D.NG
    H2 = H // 128

    I = {}
    def di(name, shape, dt=F32):
        I[name] = nc.dram_tensor(name, list(shape), dt, kind="ExternalInput")
        return I[name]

    di("xT", [2, 128, NS])
    di("AT", [NG, D.E_pad])
    di("WT", [128, D.E_pad // 128])
    di("SRC", [128, D.E_pad // 16], I16)
    di("DRELC", [128, D.E_pad // 128])
    di("SG", [128, D.EG_pad // 16], I16)
    di("DREL", [128, D.EG_pad // 128])
    di("BHOT", [3 * D.VOCAB, D.EG_pad])
    di("w1", [NG, NF]); di("b1col", [NF, 1]); di("w2", [NF, NF])
    di("b2row", [1, NF]); di("b2row3", [1, 3 * NF]); di("ones1", [1, 128])
    di("lin1", [H2, 128, NF]); di("lin2", [NF, H]); di("lin2b", [H2, 128, 1])
    di("linw", [H2, 128, H]); di("linb", [H2, 128, 1])
    di("gw1", [H2, 128, H]); di("gw2", [H2, 128, H])
    di("bondcat", [3 * D.VOCAB, H])
    di("bn1g", [H2, 128, 1]); di("bn1b", [H2, 128, 1])
    di("bn2g", [H2, 128, 1]); di("bn2b", [H2, 128, 1])
    di("epsv", [128, 1]); di("iota", [128, 128]); di("iota6", [128, 768])
    di("ident", [128, 128])
    di("zerocol", [128, 1]); di("eps5col", [128, 1]); di("pihalf", [128, 1])

    outT = nc.dram_tensor("outT", [2, 128, NS], F32, kind="ExternalOutput")

    groups = [list(range(D.cores))]

    with tile.TileContext(nc) as tc:
        with (
            tc.tile_pool(name="const", bufs=1) as cp,
            tc.tile_pool(name="work", bufs=2) as wp,
            tc.tile_pool(name="small", bufs=3) as sp,
            tc.tile_pool(name="gin", bufs=2) as ctx_gin_pool,
            tc.tile_pool(name="psum", bufs=2, space="PSUM") as pp,
            tc.tile_pool(name="dram", bufs=1, space="DRAM") as dp,
        ):
            # ---------- load constants ----------
            C = {}
            for nm, shp in [("w1", [NG, NF]), ("b1col", [NF, 1]),
                            ("w2", [NF, NF]), ("b2row", [1, NF]),
                            ("ones1", [1, 128]), ("lin2", [NF, H]),
                            ("bondcat", [3 * D.VOCAB, H]),
                            ("epsv", [128, 1]), ("iota", [128, 128]),
                            ("iota6", [128, 768]),
                            ("ident", [128, 128]), ("zerocol", [128, 1]),
                            ("eps5col", [128, 1]), ("pihalf", [128, 1])]:
                t = cp.tile(shp, F32, name=f"c_{nm}")
                nc.sync.dma_start(t[:], I[nm].ap())
                C[nm] = t
            nc.const_aps.aps[(F32, 0.0)] = C["zerocol"][:]
            # [H2,128,*] constants: load as per-half tiles
            for nm in ("lin1", "lin2b", "linw", "linb", "gw1", "gw2",
                       "bn1g", "bn1b", "bn2g", "bn2b"):
                C[nm] = []
                inner = I[nm].shape[2]
                for k in range(H2):
                    t = cp.tile([128, inner], F32, name=f"c_{nm}{k}")
                    nc.sync.dma_start(t[:], I[nm].ap()[k])
                    C[nm].append(t)

            # ---------- DRAM scratch ----------
            # xf shard/full split in half so the AllGather pipelines with
            # compute: AG#h gathers every core's half-h shard; the gathered
            # layout is permuted (half-major), host_prep permutes src indices
            HSH = NS // 2
            xf_shard = [dp.tile([HSH, NF], F32, name=f"xf_shard{h}")
                        for h in range(2)]
            xf_full = [dp.tile([D.cores * HSH, NF], F32, name=f"xf_full{h}",
                               addr_space="Shared") for h in range(2)]
            xagg_shard = dp.tile([GS, H], F32, name="xagg_shard")
            xagg_full = dp.tile([D.cores * GS, H], F32, name="xagg_full",
                                addr_space="Shared")
            agg_ab = [dp.tile([NS, NF], F32, name=f"agg_{h}")
                      for h in range(2)]
            st1_in = dp.tile([128, 4], F32, name="st1_in")
            st1_out = dp.tile([128, 4], F32, name="st1_out", addr_space="Shared")
            st2_in = dp.tile([128, 4], F32, name="st2_in")
            st2_out = dp.tile([128, 4], F32, name="st2_out", addr_space="Shared")

            # =========== Phase A: xf half 0 FIRST, then pool, then half 1 ==
            # AG#0 (which gates the conformer window stream) is the first
            # collective issued; the pooling pass + xagg AllGather follow,
            # then xf half 1 + AG#1. GIN windows are deferred into the
            # window-stream driver so their xagg wait cannot HOL-block Pool.
            rep = D.N // D.G
            PCH = 250
            n_pch = NS // PCH
            half_chunks = n_pch // 2
            GT = (GS + 127) // 128

            def emit_xf_half(h):
                for jj in range(half_chunks):
                    j = h * half_chunks + jj
                    xt = [wp.tile([128, PCH], F32, tag=f"ph_a_xt{k}",
                                  name=f"ph_a2_xt{k}") for k in range(2)]
                    for k in range(2):
                        nc.sync.dma_start(xt[k][:],
                                          I["xT"].ap()[k, :, _ts(j, PCH)])
                    r0 = jj * PCH
                    nt = (PCH + 127) // 128
                    for t in range(nt):
                        m = min(128, PCH - t * 128)
                        ps = pp.tile([128, NF], F32, tag="ps_mm", name="ps_mm")
                        for k in range(2):
                            nc.tensor.matmul(ps[:m, :],
                                             xt[k][:, t * 128:t * 128 + m],
                                             C["lin1"][k][:], start=(k == 0),
                                             stop=(k == 1))
                        sb = sp.tile([128, NF], F32, tag="ph_a_sb",
                                     name="ph_a_sb")
                        nc.scalar.copy(sb[:m, :], ps[:m, :])
                        nc.sync.dma_start(
                            xf_shard[h][r0 + t * 128: r0 + t * 128 + m, :],
                            sb[:m, :])
                if "no_coll" not in flags and "no_coll_xf" not in flags:
                    nc.gpsimd.collective_compute(
                        "AllGather", ALU.bypass, replica_groups=groups,
                        ins=[xf_shard[h].opt()], outs=[xf_full[h].opt()])

            emit_xf_half(0)

            # pooling pass + xagg AllGather
            xaggT = [cp.tile([128, GS], F32, name=f"xaggT{k}") for k in range(2)]
            for j in range(n_pch):
                xt = [wp.tile([128, PCH], F32, tag=f"ph_a_xt{k}", name=f"ph_a_xt{k}")
                      for k in range(2)]
                for k in range(2):
                    nc.sync.dma_start(xt[k][:], I["xT"].ap()[k, :, _ts(j, PCH)])
                # pool: max over groups of 10 cols
                for k in range(2):
                    nc.vector.tensor_reduce(
                        xaggT[k][:, _ts(j, PCH // rep)],
                        xt[k][:].rearrange("p (g t) -> p g t", t=rep),
                        AX.X, ALU.max)
            for t in range(GT):
                m = min(128, GS - t * 128)
                for k in range(2):
                    pst = pp.tile([128, 128], F32, tag="ps_tr", name="ps_tr")
                    nc.tensor.transpose(pst[:m, :], xaggT[k][:, t * 128:t * 128 + m],
                                        C["ident"][:])
                    sb = sp.tile([128, 128], F32, tag="ph_a_trsb", name="ph_a_trsb",
             bufs=2)
                    nc.scalar.copy(sb[:m, :], pst[:m, :])
                    nc.sync.dma_start(
                        xagg_shard[t * 128:t * 128 + m, _ts(k, 128)], sb[:m, :])
            if "no_coll" not in flags and "no_coll_xagg" not in flags:
                nc.gpsimd.collective_compute(
                    "AllGather", ALU.bypass, replica_groups=groups,
                    ins=[xagg_shard.opt()], outs=[xagg_full.opt()])

            emit_xf_half(1)

            # =========== Phase B: GIN branch (sharded by graph node) =========
            # gather x_agg[sg], edge_emb via bond one-hot matmul, relu,
            # one-hot scatter into agg_g windows
            sgidx = cp.tile([128, D.EG_pad // 16], I16, name="sgidx_sb")
            nc.sync.dma_start(sgidx[:], I["SG"].ap())
            drel = cp.tile([128, D.EG_pad // 128], F32, name="drel_sb")
            nc.sync.dma_start(drel[:], I["DREL"].ap())

            # t-buffer (node-major (1+eps)x_agg + agg_g), then transposed halves
            gp = ctx_gin_pool
            assert D.EG_pad % 128 == 0
            tiles_per_win = D.PW // 128
            bstate = {}

            def emit_b1_windows(tT):
              for w in range(D.NWIN):
                m = min(D.gwin, GS - w * D.gwin)
                # gather this window's source rows
                gath_g = wp.tile([128, tiles_per_win, H], F32,
                                 tag="ph_b_gath", name="ph_b_gath")
                nc.gpsimd.dma_gather(
                    gath_g[:], xagg_full[:],
                    sgidx[:, w * D.PW // 16:(w + 1) * D.PW // 16],
                    num_idxs=D.PW, num_idxs_reg=D.PW, elem_size=H)
                bhot = wp.tile([3 * D.VOCAB, D.PW], F32, tag="ph_b_bhot",
                               name="ph_b_bhot")
                nc.sync.dma_start(bhot[:],
                                  I["BHOT"].ap()[:, w * D.PW:(w + 1) * D.PW])
                ps_agg = pp.tile([128, H], F32, tag="ps_agg", name="ps_agg")
                for i in range(tiles_per_win):
                    t = w * tiles_per_win + i
                    # edge embedding: one-hot bond matmul (K=15)
                    ps_emb = pp.tile([128, H], F32, tag="ps_mm", name="ps_mm")
                    nc.tensor.matmul(ps_emb[:], bhot[:, _ts(i, 128)],
                                     C["bondcat"][:], start=True, stop=True)
                    # msg = relu(gathered + emb)
                    msg = sp.tile([128, H], F32, tag="ph_b_msg", name="ph_b_msg")
                    nc.vector.tensor_tensor(msg[:], gath_g[:, i, :], ps_emb[:],
                                            ALU.add)
                    nc.scalar.activation(msg[:], msg[:], ACTF.Relu)
                    # one-hot scatter
                    oh = sp.tile([128, D.gwin], F32, tag="ph_b_oh", name="ph_b_oh")
                    nc.vector.tensor_scalar(oh[:], C["iota"][:, :D.gwin],
                                            drel[:, t:t + 1], None, ALU.is_equal)
                    nc.tensor.matmul(ps_agg[:m, :], oh[:, :m], msg[:],
                                     start=(i == 0), stop=(i == tiles_per_win - 1))
                # t = (1+eps) * x_agg + agg_g  (node-major window rows)
                xa = sp.tile([128, H], F32, tag="ph_b_xa", name="ph_b_xa")
                nc.sync.dma_start(
                    xa[:m, :], xagg_shard[w * D.gwin:w * D.gwin + m, :])
                tn = sp.tile([128, H], F32, tag="ph_b_tn", name="ph_b_tn")
                nc.vector.tensor_scalar(tn[:m, :], xa[:m, :], C["epsv"][:m, :],
                                        None, ALU.mult)
                nc.vector.tensor_tensor(tn[:m, :], tn[:m, :], ps_agg[:m, :],
                                        ALU.add)
                # transpose to feature-major tT
                for k in range(2):
                    pst = pp.tile([128, 128], F32, tag="ps_tr", name="ps_tr")
                    nc.tensor.transpose(pst[:, :m], tn[:m, _ts(k, 128)],
                                        C["ident"][:m, :m])
                    nc.vector.tensor_copy(tT[k][:, w * D.gwin:w * D.gwin + m],
                                          pst[:, :m])

            def gin_mm_and_stats_issue(inT, Wc, uT, stats_in, stats_out, label):
                """u = in @ W (node-major tiles), transpose to uT, stats;
                issues the stats AllReduce but does NOT read the result."""
                for t in range(GT):
                    m = min(128, GS - t * 128)
                    ps = pp.tile([128, H], F32, tag="ps_mm", name="ps_mm")
                    for k in range(2):
                        nc.tensor.matmul(ps[:m, :],
                                         inT[k][:, t * 128:t * 128 + m],
                                         Wc[k][:], start=(k == 0), stop=(k == 1))
                    sb = sp.tile([128, H], F32, tag=f"{label}_sb", name=f"{label}_sb")
                    nc.scalar.copy(sb[:m, :], ps[:m, :])
                    for k in range(2):
                        pst = pp.tile([128, 128], F32, tag="ps_tr", name="ps_tr")
                        nc.tensor.transpose(pst[:, :m], sb[:m, _ts(k, 128)],
                                            C["ident"][:m, :m])
                        nc.vector.tensor_copy(uT[k][:, t * 128:t * 128 + m],
                                              pst[:, :m])
                st = sp.tile([128, 4], F32, tag=f"{label}_st", name=f"{label}_st")
                sq = sp.tile([128, GS], F32, tag="gin_sq", name="gin_sq",
                             bufs=1)
                for k in range(2):
                    nc.vector.tensor_reduce(st[:, 2 * k:2 * k + 1], uT[k][:],
                                            AX.X, ALU.add)
                    nc.vector.tensor_tensor(sq[:], uT[k][:], uT[k][:], ALU.mult)
                    nc.vector.tensor_reduce(st[:, 2 * k + 1:2 * k + 2], sq[:],
                                            AX.X, ALU.add)
                nc.sync.dma_start(stats_in[:], st[:])
                if "no_b_ar" not in flags:
                    nc.gpsimd.collective_compute(
                        "AllReduce", ALU.add, replica_groups=groups,
                        ins=[stats_in.opt()], outs=[stats_out.opt()])

            def gin_read_stf(stats_in, stats_out, label):
                stf = sp.tile([128, 4], F32, tag=f"{label}_stf", name=f"{label}_stf")
                nc.sync.dma_start(
                    stf[:], stats_in[:] if "no_b_ar" in flags else stats_out[:])
                return stf

            def bn_apply(stf, uT, g_c, b_c, outT_t, relu, label):
                """out = func((u - mu) * g / sqrt(var+eps) + b), feature-major."""
                inv_n = 1.0 / float(D.G)
                for k in range(2):
                    mu = sp.tile([128, 1], F32, tag=f"{label}_mu{k}", name=f"{label}_mu{k}")
                    nc.vector.tensor_scalar(mu[:], stf[:, 2 * k:2 * k + 1],
                                            inv_n, None, ALU.mult)
                    var = sp.tile([128, 1], F32, tag=f"{label}_va{k}", name=f"{label}_va{k}")
                    nc.vector.tensor_scalar(var[:], stf[:, 2 * k + 1:2 * k + 2],
                                            inv_n, None, ALU.mult)
                    mu2 = sp.tile([128, 1], F32, tag=f"{label}_m2{k}", name=f"{label}_m2{k}")
                    nc.vector.tensor_tensor(mu2[:], mu[:], mu[:], ALU.mult)
                    nc.vector.tensor_tensor(var[:], var[:], mu2[:], ALU.subtract)
                    sd = sp.tile([128, 1], F32, tag=f"{label}_sd{k}", name=f"{label}_sd{k}")
                    nc.scalar.activation(sd[:], var[:], ACTF.Sqrt,
                                         bias=C["eps5col"][:])
                    rs = sp.tile([128, 1], F32, tag=f"{label}_rs{k}", name=f"{label}_rs{k}")
                    nc.vector.reciprocal(rs[:], sd[:])
                    sc = sp.tile([128, 1], F32, tag=f"{label}_sc{k}", name=f"{label}_sc{k}")
                    nc.vector.tensor_tensor(sc[:], g_c[k][:], rs[:], ALU.mult)
                    sh = sp.tile([128, 1], F32, tag=f"{label}_sh{k}", name=f"{label}_sh{k}")
                    nc.vector.tensor_tensor(sh[:], mu[:], sc[:], ALU.mult)
                    nc.vector.tensor_tensor(sh[:], b_c[k][:], sh[:], ALU.subtract)
                    nc.scalar.activation(outT_t[k][:], uT[k][:],
                                         ACTF.Relu if relu else ACTF.Identity,
                                         bias=sh[:], scale=sc[:])

            def gin_buf(nm):
                return [gp.tile([128, GS], F32, tag=f"ginbuf{k}",
                                name=f"{nm}{k}") for k in range(2)]

            if "no_b" in flags:
                ginT = gin_buf("ginT")
                for k in range(2):
                    nc.vector.memset(ginT[k][:], 0.0)
                bstate["ginT"] = ginT

            def emit_b1():
                """GIN window aggregation + gmm1 + st1 issue (early phase C,
                after the xagg AllGather has had time to land)."""
                if "no_b" in flags or "uT" in bstate:
                    return
                tT = [gp.tile([128, GS], F32, tag=f"ginbuf{k}", name=f"tT{k}")
                      for k in range(2)]
                emit_b1_windows(tT)
                uT = gin_buf("uT")
                gin_mm_and_stats_issue(tT, C["gw1"], uT, st1_in, st1_out,
                                       "gmm1")
                bstate["uT"] = uT

            def emit_b2():
                """bn1 + gmm2 + st2 AllReduce issue (mid phase C)."""
                if "no_b" in flags or "t2T" in bstate or "uT" not in bstate:
                    return
                stf1 = gin_read_stf(st1_in, st1_out, "gmm1")
                t1T = gin_buf("t1T")
                bn_apply(stf1, bstate["uT"], C["bn1g"], C["bn1b"], t1T,
                         True, "bn1")
                t2T = gin_buf("t2T")
                gin_mm_and_stats_issue(t1T, C["gw2"], t2T, st2_in, st2_out,
                                       "gmm2")
                bstate["t2T"] = t2T

            def emit_b3():
                """bn2 -> ginT (late in phase C)."""
                if "no_b" in flags or "ginT" in bstate:
                    return
                stf2 = gin_read_stf(st2_in, st2_out, "gmm2")
                ginT = gin_buf("ginT")
                bn_apply(stf2, bstate["t2T"], C["bn2g"], C["bn2b"], ginT,
                         False, "bn2")
                bstate["ginT"] = ginT

            # =========== Phase C: conformer edge pipeline ===========
            # window-major: per (src-half stream h, dst-window w of 128
            # nodes): gather the window's edges (one call per src quad
            # bucket of B_E), compute msg = (xf[src]) * (mlp(A) + b2), then
            # aggregate over dst via one-hot matmuls into PSUM (the one-hot
            # rows carry the cosine-cutoff C so no separate C-multiply),
            # and flush the window's 128 agg rows with a plain DMA write.
            # No scatter-add: each agg row is written exactly once.
            # resident: C row (cosine cutoff) and dst-rel row per edge
            crow = cp.tile([128, D.E_pad // 128], F32, name="crow_sb")
            for s0 in range(0, D.E_pad // 128, 512):
                sw = min(512, D.E_pad // 128 - s0)
                wt = wp.tile([128, 512], F32, tag="ph_c_wt", name="ph_c_wt",
             bufs=1)
                nc.sync.dma_start(wt[:, :sw], I["WT"].ap()[:, s0:s0 + sw])
                nc.scalar.activation(wt[:, :sw], wt[:, :sw], ACTF.Sin,
                                     bias=C["pihalf"][:],
                                     scale=math.pi / D.CUTOFF)
                nc.scalar.activation(crow[:, s0:s0 + sw], wt[:, :sw],
                                     ACTF.Copy, bias=0.5, scale=-0.5)
            drelc = cp.tile([128, D.E_pad // 128], F32, name="drelc_sb")
            nc.sync.dma_start(drelc[:], I["DRELC"].ap())

            B_E, CW, EH = D.B_E, D.CW, D.EH
            WE = 2 * B_E           # edges per (stream, window)
            SW = 2 * WE            # edges per super-window (pair of windows)
            NTS = SW // 128        # tiles per super-window (12)
            assert CW % 2 == 0
            wstream = ([] if "no_c" in flags else
                       [(h, w) for h in range(2) for w in range(0, CW, 2)])
            PFD = 4  # gather prefetch depth (super-windows issued ahead)
            gat_fifo = []

            def emit_gather(idx):
                h, w = wstream[idx]
                e0 = h * EH + w * WE
                if "no_gather" in flags:
                    if "no_cmm" in flags:
                        return None
                    gat = wp.tile([128, NTS, NF], F32, tag="ph_c_gat",
                                  name="ph_c_gat", bufs=PFD + 2)
                    nc.vector.memset(gat[:], 0.0)
                    return gat
                gat = wp.tile([128, NTS, NF], F32, tag="ph_c_gat",
                              name="ph_c_gat", bufs=PFD + 2)
                si = wp.tile([128, SW // 16], I16, tag="ph_c_si",
                             name="ph_c_si", bufs=PFD + 2)
                nc.sync.dma_start(
                    si[:], I["SRC"].ap()[:, e0 // 16:(e0 + SW) // 16])
                for j in range(2):
                    nc.gpsimd.dma_gather(
                        gat[:, j * 6:(j + 1) * 6, :],
                        xf_full[h][j * D.qsize:(j + 1) * D.qsize, :],
                        si[:, j * 2 * B_E // 16:(j + 1) * 2 * B_E // 16],
                        num_idxs=2 * B_E, num_idxs_reg=2 * B_E, elem_size=NF,
                        queue_num=(2 * idx + j) % 3)
                return gat

            def emit_compute_flush(idx, gat):
                h, w = wstream[idx]
                e0 = h * EH + w * WE
                c0col = e0 // 128
                msg = None
                if "no_cmm" not in flags:
                    msg = wp.tile([128, NTS, NF], F32, tag="ph_c_msg",
                                  name="ph_c_msg")
                    at = wp.tile([NG, SW], F32, tag="ph_c_at",
                                 name="ph_c_at")
                    nc.sync.dma_start(at[:], I["AT"].ap()[:, e0:e0 + SW])
                    # per 512-edge group: mm1+relu, mm2 (+b2) packed 4 tiles
                    # into one PSUM bank; 512-wide msg-mul
                    for g in range(3):
                        s0 = g * 512
                        ps1 = pp.tile([128, 512], F32, tag="ps_mm", name="ps_mm")
                        nc.tensor.matmul(ps1[:], C["w1"][:],
                                         at[:, s0:s0 + 512],
                                         start=True, stop=True)
                        h1 = wp.tile([128, 512], F32, tag="ph_c_h1",
                                     name="ph_c_h1")
                        nc.scalar.activation(h1[:], ps1[:],
                                             ACTF.Relu, bias=C["b1col"][:])
                        psw = pp.tile([128, 4, NF], F32, tag="ps_w", name="ps_w")
                        for t4 in range(4):
                            nc.tensor.matmul(psw[:, t4, :], h1[:, _ts(t4, 128)],
                                             C["w2"][:], start=True,
                                             stop="b2zero" in flags)
                            if "b2zero" not in flags:
                                nc.tensor.matmul(psw[:, t4, :], C["ones1"][:],
                                                 C["b2row"][:], start=False,
                                                 stop=True)
                        nc.vector.tensor_tensor(msg[:, 4 * g:4 * g + 4, :],
                                                gat[:, 4 * g:4 * g + 4, :],
                                                psw[:], ALU.mult)
                if "no_scatter" in flags or "no_cmm" in flags:
                    return
                # one-hot aggregation: rows carry C; accumulate over tiles,
                # one PSUM half-bank per window of the pair
                ohc = sp.tile([128, NTS, 128], F32, tag="ph_c_oh",
                              name="ph_c_oh", bufs=2)
                for p2 in range(2):
                    view = ohc[:, p2 * 6:(p2 + 1) * 6, :]
                    cc = c0col + p2 * 6
                    nc.vector.tensor_tensor(
                        view,
                        C["iota6"][:].rearrange("p (t j) -> p t j", j=128),
                        drelc[:, cc:cc + 6].broadcast_to((128, 6, 128)),
                        ALU.is_equal)
                    nc.vector.tensor_tensor(
                        view, view,
                        crow[:, cc:cc + 6].broadcast_to((128, 6, 128)),
                        ALU.mult)
                psA = pp.tile([128, 2, NF], F32, tag="ps_agg", name="ps_cagg")
                for p2 in range(2):
                    tl = [t for t in range(NTS) if (t // 3) % 2 == p2]
                    for i, t in enumerate(tl):
                        nc.tensor.matmul(psA[:, p2, :], ohc[:, t, :],
                                         msg[:, t, :], start=(i == 0),
                                         stop=(i == len(tl) - 1))
                for p2 in range(2):
                    base = (w + p2) * 128
                    m = min(128, NS - base)
                    stg = sp.tile([128, NF], F32, tag="ph_c_stg",
                                  name="ph_c_stg", bufs=2)
                    nc.scalar.copy(stg[:m, :], psA[:m, p2, :])
                    nc.sync.dma_start(
                        agg_ab[h][base:base + m, :].rearrange(
                            "(t p) f -> p t f", p=m),
                        stg[:m, :].rearrange("p (t f) -> p t f", f=NF))

            # =========== Phase D: h = relu(agg@lin2+b)@linw+b, residual =====
            # emitted as a closure so node chunks interleave into the tail
            # of the window stream (chunk j only needs agg rows already
            # flushed by both streams, plus ginT)
            NCH = D.nchunk
            n_nch = NS // NCH

            def emit_d(j):
                r0 = j * NCH
                # load agg rows, transpose to feature-major aggT [NF, NCH]
                aggT = wp.tile([NF, NCH], F32, tag="ph_d_aggT", name="ph_d_aggT")
                ntt = (NCH + 127) // 128
                for t in range(ntt):
                    m = min(128, NCH - t * 128)
                    asb = sp.tile([128, NF], F32, tag="ph_d_asb", name="ph_d_asb")
                    nc.sync.dma_start(asb[:m, :],
                                      agg_ab[0][r0 + t * 128:r0 + t * 128 + m, :])
                    bsb = sp.tile([128, NF], F32, tag="ph_d_bsb", name="ph_d_bsb")
                    nc.sync.dma_start(bsb[:m, :],
                                      agg_ab[1][r0 + t * 128:r0 + t * 128 + m, :])
                    nc.vector.tensor_tensor(asb[:m, :], asb[:m, :], bsb[:m, :],
                                            ALU.add)
                    pst = pp.tile([128, 128], F32, tag="ps_tr", name="ps_tr")
                    nc.tensor.transpose(pst[:, :m], asb[:m, :], C["ident"][:m, :m])
                    nc.vector.tensor_copy(aggT[:, t * 128:t * 128 + m],
                                          pst[:, :m])
                # h1T = relu(lin2^T @ aggT + b)  [2][128, NCH]
                h1T = [wp.tile([128, NCH], F32, tag=f"ph_d_h1T{k}", name=f"ph_d_h1T{k}")
                       for k in range(2)]
                for k in range(2):
                    ps = pp.tile([128, NCH], F32, tag="ps_mm", name="ps_mm")
                    nc.tensor.matmul(ps[:], C["lin2"][:, _ts(k, 128)], aggT[:],
                                     start=True, stop=True)
                    nc.scalar.activation(h1T[k][:], ps[:], ACTF.Relu,
                                         bias=C["lin2b"][k][:])
                # outT = linw^T @ h1T + linb + xT + gin[batch]
                for k in range(2):
                    ps = pp.tile([128, NCH], F32, tag="ps_mm", name="ps_mm")
                    for kk in range(2):
                        nc.tensor.matmul(ps[:], C["linw"][kk][:, _ts(k, 128)],
                                         h1T[kk][:], start=(kk == 0),
                                         stop=(kk == 1))
                    ob = sp.tile([128, NCH], F32, tag="ph_d_ob", name="ph_d_ob")
                    nc.scalar.activation(ob[:], ps[:], ACTF.Identity,
                                         bias=C["linb"][k][:])
                    xtc = sp.tile([128, NCH], F32, tag="ph_d_xtc", name="ph_d_xtc")
                    nc.sync.dma_start(xtc[:], I["xT"].ap()[k, :, r0:r0 + NCH])
                    nc.vector.tensor_tensor(ob[:], ob[:], xtc[:], ALU.add)
                    # + gin, each graph col repeated `rep` times
                    rep = D.N // D.G
                    g0 = r0 // rep
                    gin_rep = bstate["ginT"][k][:, g0:g0 + NCH // rep] \
                        .broadcast_to((128, NCH // rep, rep))
                    nc.vector.tensor_tensor(
                        ob[:].rearrange("p (g t) -> p g t", t=rep),
                        ob[:].rearrange("p (g t) -> p g t", t=rep),
                        gin_rep, ALU.add)
                    nc.sync.dma_start(outT.ap()[k, :, r0:r0 + NCH], ob[:])

            next_d = [0]

            def emit_d_ready(done):
                """Emit phase D chunks whose agg rows are fully flushed
                (at most 3 per window to avoid emission bursts)."""
                if "no_d" in flags or "ginT" not in bstate:
                    return
                burst = 0
                while (next_d[0] < n_nch and burst < 3
                       and (done - 48) * 256 >= NCH * (next_d[0] + 1)):
                    emit_d(next_d[0])
                    next_d[0] += 1
                    burst += 1

            for i in range(len(wstream) + PFD):
                if i < len(wstream):
                    gat_fifo.append(emit_gather(i))
                if i >= PFD:
                    done = i - PFD
                    emit_compute_flush(done, gat_fifo[done])
                    # GIN stage hooks: latency of the stats AllReduces and
                    # the serial BN chains hides under the window stream
                    if done == 28:
                        emit_b1()
                    if done == 61:
                        emit_b2()
                    if done == 80:
                        emit_b3()
                    if done > 80:
                        emit_d_ready(done)
            emit_b1()  # no-ops unless phase C was skipped
            emit_b2()
            emit_b3()
            if "no_d" not in flags:
                for j in range(next_d[0], n_nch):
                    emit_d(j)

    nc.compile()
    return nc


_CACHE = {}


def _get_nc(D: Dims, flags: frozenset = frozenset()):
    key = ("nc", D, flags)
    if key not in _CACHE:
        _CACHE[key] = build_nc(D, flags)
    return _CACHE[key]


def run_on_hw(inputs, D: Dims = REAL):
    flags = (frozenset({"b2zero"})
             if not np.any(np.asarray(inputs["mlp_b2"])) else frozenset())
    nc = _get_nc(D, flags)
    in_maps = host_prep(inputs, D)
    res = bass_utils.run_bass_kernel_spmd(nc, in_maps,
                                          core_ids=list(range(D.cores)))
    return assemble(res.results, D)


def kernel(**inputs):
    return run_on_hw(inputs, REAL)

